# revision 16
# baseline (speedup 1.0000x reference)
"""Self-contained Trainium2 Bass kernel for the DeepseekV2 decoder layer problem.

Sharding (8 cores): core c owns the contiguous 128-token block [128c, 128c+128).
KV-side projections are computed per-own-token and AllGathered as one bundle
(kpe^T / ik^T / kv_latent^T).  Indexer scores + top-k + MLA attention + o_proj
run on own rows.  h2 is transported feature-major (h2^T) in two half-token
AllGathers (second half carries the router weights); MoE is expert-parallel
(1 routed expert per core, dense over all tokens) plus the shared expert on
own tokens.  Host sums the per-core partials.
"""
import sys
sys.path.insert(0, "/opt/trn_rl_repo")
import numpy as np
import ml_dtypes

import concourse.bass as bass
import concourse.mybir as mybir
from concourse import bacc, tile
from concourse.bass_utils import run_bass_kernel_spmd
from concourse.masks import make_identity

f32 = mybir.dt.float32
bf16 = mybir.dt.bfloat16
AF = mybir.ActivationFunctionType
ALU = mybir.AluOpType
AX = mybir.AxisListType
BF = ml_dtypes.bfloat16

# dims
T = 1024; H = 2048; NH = 16; DN = 128; DR = 64; DQ = DN + DR; DV = 128
QL = 1536; KL = 512
INH = 16; IHD = 128; TOPK = 256
NE = 8; MI = 1024; SI = 1024
BASE = 10000.0; EPS = 1e-6
SCALE = DQ ** -0.5
IDX_SCALE = IHD ** -0.5
FP8_MAX = 448.0
NCORES = 8
TPC = T // NCORES        # 128 tokens per core
NEG = -1e30
import os
SKIP_CC = os.environ.get("SKIP_CC") == "1"
SKIP_TOPK = os.environ.get("SKIP_TOPK") == "1"

KB = 16   # H/128 k-chunks
QB = 12   # QL/128
RG = [list(range(NCORES))]
CCL = DR + IHD + KL          # merged latent collective rows (704)
HLF = TPC // 2               # 64 tokens per h2 half


def build():
    nc = bacc.Bacc("TRN2", target_bir_lowering=False,
                   debug=os.environ.get("BASS_DEBUG") == "1",
                   enable_asserts=False, num_devices=NCORES)

    def din(name, shape, dt=bf16):
        return nc.dram_tensor(name, shape, dt, kind="ExternalInput").ap()

    # ---- per-core inputs ----
    XO = din("XO", [TPC, H], f32)              # x_in own rows
    CAUS = din("CAUS", [TPC, T], f32)          # causal01 over global keys
    CSQ = din("CSQ", [TPC, 512], f32)          # cos*SCALE tiled 16x
    SNQ = din("SNQ", [TPC, 512], f32)
    CSR = din("CSR", [TPC, 512], f32)          # cos tiled 16x (unscaled)
    SNR = din("SNR", [TPC, 512], f32)
    OH = din("OH", [1, NE], f32)               # own-expert one-hot
    KNW = din("KNW", [1, IHD], f32)            # idx_kn_w
    KNB = din("KNB", [1, IHD], f32)
    WPB = din("WPB", [1, INH], f32)            # idx_wp_b
    WA = din("WA", [H, QL + KL + DR])          # bf16, ln-folded
    WQB = din("WQB", [QL, NH * DQ])
    WIQ = din("WIQ", [QL, INH * IHD])
    WIK = din("WIK", [H, IHD])
    WIP = din("WIP", [H, INH])
    WKN = din("WKN", [KL, NH * DN])
    WV = din("WV", [KL, NH * DV])
    WO = din("WO", [NH * DV, H])
    WG = din("WG", [H, NE], f32)
    WEG = din("WEG", [MI // 128, 128, KB * 128])   # [m][p][k*128+mi'] host-relaid
    WEU = din("WEU", [MI // 128, 128, KB * 128])
    WED = din("WED", [MI // 128, 128, H])          # [m][p=mi-in-chunk][H]
    WSG = din("WSG", [H, SI])
    WSU = din("WSU", [H, SI])
    WSD = din("WSD", [SI, H])

    OUT_P = nc.dram_tensor("OUT_P", [T, H], bf16, kind="ExternalOutput").ap()
    OUT_X = nc.dram_tensor("OUT_X", [TPC, H], f32, kind="ExternalOutput").ap()

    with tile.TileContext(nc) as tc:
        with tc.tile_pool(name="const", bufs=1) as Pc, \
             tc.tile_pool(name="dram", bufs=1, space="DRAM") as Pd:
            idf = Pc.tile([128, 128], f32)
            make_identity(nc, idf[:])
            idb = Pc.tile([128, 128], bf16)
            nc.vector.tensor_copy(idb[:], idf[:])
            eps_b = Pc.tile([128, 1], f32)
            nc.vector.memset(eps_b[:], EPS)

            xo = Pc.tile([TPC, H], f32)
            nc.sync.dma_start(xo[:], XO[:])
            caus = Pc.tile([TPC, T], f32)
            nc.sync.dma_start(caus[:], CAUS[:])
            csq = Pc.tile([TPC, 512], f32); nc.sync.dma_start(csq[:], CSQ[:])
            snq = Pc.tile([TPC, 512], f32); nc.sync.dma_start(snq[:], SNQ[:])
            csr = Pc.tile([TPC, 512], f32); nc.sync.dma_start(csr[:], CSR[:])
            snr = Pc.tile([TPC, 512], f32); nc.sync.dma_start(snr[:], SNR[:])
            knw_r = Pc.tile([1, IHD], f32); nc.sync.dma_start(knw_r[:], KNW[:])
            knb_r = Pc.tile([1, IHD], f32); nc.sync.dma_start(knb_r[:], KNB[:])
            wpb_r = Pc.tile([1, INH], f32); nc.sync.dma_start(wpb_r[:], WPB[:])
            knw_bc = Pc.tile([128, IHD], f32)
            nc.gpsimd.partition_broadcast(knw_bc[:], knw_r[:])
            knb_bc = Pc.tile([128, IHD], f32)
            nc.gpsimd.partition_broadcast(knb_bc[:], knb_r[:])
            wpb_bc = Pc.tile([128, INH], f32)
            nc.gpsimd.partition_broadcast(wpb_bc[:], wpb_r[:])
            wg_sb = Pc.tile([128, KB, NE], f32)
            nc.sync.dma_start(wg_sb[:], WG[:].rearrange("(k p) n -> p k n", p=128))
            oh_r = Pc.tile([1, NE], f32); nc.sync.dma_start(oh_r[:], OH[:])
            oh_bc = Pc.tile([128, NE], f32)
            nc.gpsimd.partition_broadcast(oh_bc[:], oh_r[:])

            # collective buffers
            cc1_in = Pd.tile([CCL, TPC], bf16)
            cc1_out = Pd.tile([NCORES, CCL, TPC], bf16, addr_space="Shared")
            HRW = HLF * KB + 2 * NE        # half-token h2T cols + rw bf16 pairs
            cch0_in = Pd.tile([128, HRW], bf16)
            cch0_out = Pd.tile([NCORES, 128, HRW], bf16, addr_space="Shared")
            cch1_in = Pd.tile([128, HLF * KB], bf16)
            cch1_out = Pd.tile([NCORES, 128, HLF * KB], bf16, addr_space="Shared")

            with tc.tile_pool(name="att", bufs=1) as Pa, \
                 tc.tile_pool(name="wstream", bufs=2) as Pw:
                Pe = tc.alloc_tile_pool(name="early", bufs=1)
                # rmsnorm scale r1 for own rows
                sq = Pa.tile([TPC, H], f32, name="sq_scratch", tag="sq2")
                ssq = Pa.tile([TPC, 1], f32)
                nc.scalar.activation(sq[:], xo[:], AF.Square, accum_out=ssq[:])
                r1 = Pa.tile([TPC, 1], f32)
                nc.scalar.activation(r1[:], ssq[:], AF.Sqrt, bias=eps_b[:], scale=1.0 / H)
                nc.vector.reciprocal(r1[:], r1[:])
                hn_own = Pe.tile([TPC, H], bf16)
                nc.vector.tensor_scalar(hn_own[:], xo[:], r1[:], None, op0=ALU.mult)
                hnT = Pe.tile([128, KB, TPC], bf16)
                with tc.tile_pool(name="ps_tr", bufs=2, space="PSUM") as Pp:
                    for k in range(KB):
                        tp = Pp.tile([128, 128], bf16, name="tp")
                        nc.tensor.transpose(tp[:], hn_own[:, k * 128:(k + 1) * 128], idb[:])
                        nc.scalar.copy(hnT[:, k, :], tp[:])

                # ---- qkv_a: kv+kpe columns FIRST so CC1 can launch early ----
                with tc.tile_pool(name="ps_qkv", bufs=1, space="PSUM") as Pp:
                    kvp_ps = Pp.tile([TPC, KL + DR], f32)
                    for k in range(KB):
                        wakv_k = Pw.tile([128, KL + DR], bf16, name="wakv", tag="wknh", bufs=3)
                        with tc.high_priority():
                            nc.sync.dma_start(wakv_k[:], WA[:].rearrange("(k p) n -> p k n", p=128)[:, k, QL:])
                        nc.tensor.matmul(kvp_ps[:, 0:512], hnT[:, k, :], wakv_k[:, 0:512],
                                         start=(k == 0), stop=(k == KB - 1))
                        nc.tensor.matmul(kvp_ps[:, 512:], hnT[:, k, :], wakv_k[:, 512:],
                                         start=(k == 0), stop=(k == KB - 1))
                    # kv_c rmsnorm -> bf16
                    ksq = Pa.tile([TPC, KL], f32, name="ksq", tag="sq2")
                    kss = Pa.tile([TPC, 1], f32)
                    nc.scalar.activation(ksq[:], kvp_ps[:, :KL], AF.Square, accum_out=kss[:])
                    rkv = Pa.tile([TPC, 1], f32)
                    nc.scalar.activation(rkv[:], kss[:], AF.Sqrt, bias=eps_b[:], scale=1.0 / KL)
                    nc.vector.reciprocal(rkv[:], rkv[:])
                    kvn = Pa.tile([TPC, KL], bf16)
                    nc.vector.tensor_scalar(kvn[:], kvp_ps[:, :KL], rkv[:], None, op0=ALU.mult)

                    # k_pe rope (unscaled tables) -> bf16 [TPC, 64]
                    kpe = Pa.tile([TPC, DR], bf16)
                    t1 = Pa.tile([TPC, 32], f32, name="rt1", tag="rt1")
                    t2 = Pa.tile([TPC, 32], f32, name="rt2", tag="rt2")
                    pe_src = kvp_ps[:, KL:].rearrange("p (n two) -> p n two", two=2)
                    x1, x2 = pe_src[:, :, 0], pe_src[:, :, 1]
                    ko = kpe[:].rearrange("p (n two) -> p n two", two=2)
                    nc.vector.tensor_tensor(t1[:], x1, csr[:, :32], op=ALU.mult)
                    nc.vector.tensor_tensor(t2[:], x2, snr[:, :32], op=ALU.mult)
                    nc.vector.tensor_sub(ko[:, :, 0], t1[:], t2[:])
                    nc.vector.tensor_tensor(t1[:], x1, snr[:, :32], op=ALU.mult)
                    nc.vector.tensor_tensor(t2[:], x2, csr[:, :32], op=ALU.mult)
                    nc.vector.tensor_add(ko[:, :, 1], t1[:], t2[:])

                # ---- ik own: layernorm(hn @ Wik) + rope ----
                ikn = Pa.tile([TPC, IHD], bf16)
                with tc.tile_pool(name="ps_ik", bufs=1, space="PSUM") as Pp:
                    wik_sb = Pe.tile([128, KB, IHD], bf16)
                    with tc.high_priority():
                        nc.sync.dma_start(wik_sb[:], WIK[:].rearrange("(k p) n -> p k n", p=128))
                    ik_ps = Pp.tile([TPC, IHD], f32)
                    for k in range(KB):
                        nc.tensor.matmul(ik_ps[:], hnT[:, k, :], wik_sb[:, k, :],
                                         start=(k == 0), stop=(k == KB - 1))
                    negm = Pa.tile([TPC, 1], f32)
                    nc.vector.tensor_reduce(negm[:], ik_ps[:], AX.X, ALU.add, negate=True)
                    nc.vector.tensor_scalar(negm[:], negm[:], 1.0 / IHD, None, op0=ALU.mult)
                    xm = Pa.tile([TPC, IHD], f32)
                    nc.vector.tensor_scalar(xm[:], ik_ps[:], negm[:], None, op0=ALU.add)
                    xms = Pa.tile([TPC, IHD], f32)
                    vss = Pa.tile([TPC, 1], f32)
                    nc.scalar.activation(xms[:], xm[:], AF.Square, accum_out=vss[:])
                    rstd = Pa.tile([TPC, 1], f32)
                    nc.scalar.activation(rstd[:], vss[:], AF.Sqrt, bias=eps_b[:], scale=1.0 / IHD)
                    nc.vector.reciprocal(rstd[:], rstd[:])
                    ikf = Pa.tile([TPC, IHD], f32)
                    nc.vector.scalar_tensor_tensor(ikf[:], xm[:], rstd[:], knw_bc[:],
                                                   op0=ALU.mult, op1=ALU.mult)
                    nc.vector.tensor_add(ikf[:], ikf[:], knb_bc[:])
                    pe2 = ikf[:, :DR].rearrange("p (n two) -> p n two", two=2)
                    iko2 = ikn[:, :DR].rearrange("p (n two) -> p n two", two=2)
                    it1 = Pa.tile([TPC, 32], f32, name="it1", tag="rt1")
                    it2 = Pa.tile([TPC, 32], f32, name="it2", tag="rt2")
                    nc.vector.tensor_tensor(it1[:], pe2[:, :, 0], csr[:, :32], op=ALU.mult)
                    nc.vector.tensor_tensor(it2[:], pe2[:, :, 1], snr[:, :32], op=ALU.mult)
                    nc.vector.tensor_sub(iko2[:, :, 0], it1[:], it2[:])
                    nc.vector.tensor_tensor(it1[:], pe2[:, :, 0], snr[:, :32], op=ALU.mult)
                    nc.vector.tensor_tensor(it2[:], pe2[:, :, 1], csr[:, :32], op=ALU.mult)
                    nc.vector.tensor_add(iko2[:, :, 1], it1[:], it2[:])
                    nc.vector.tensor_copy(ikn[:, DR:], ikf[:, DR:])

                # transposes of kpe, ikn, kvn -> merged CC1 input
                with tc.tile_pool(name="ps_tr2", bufs=2, space="PSUM") as Pp:
                    kpeT_o = Pa.tile([DR, TPC], bf16)
                    tpp = Pp.tile([DR, 128], bf16, name="tpp", tag="tp")
                    nc.tensor.transpose(tpp[:], kpe[:], idb[:])
                    nc.scalar.copy(kpeT_o[:], tpp[:])
                    nc.sync.dma_start(cc1_in[:DR, :], kpeT_o[:])
                    iknT_o = Pa.tile([IHD, TPC], bf16)
                    tpi = Pp.tile([IHD, TPC], bf16, name="tpi", tag="tp")
                    nc.tensor.transpose(tpi[:], ikn[:], idb[:])
                    nc.scalar.copy(iknT_o[:], tpi[:])
                    nc.sync.dma_start(cc1_in[DR:DR + IHD, :], iknT_o[:])
                    kvT_o = Pa.tile([128, 4, TPC], bf16)
                    for k in range(4):
                        tpk = Pp.tile([128, 128], bf16, name="tpk", tag="tp")
                        nc.tensor.transpose(tpk[:], kvn[:, k * 128:(k + 1) * 128], idb[:])
                        nc.scalar.copy(kvT_o[:, k, :], tpk[:])
                    nc.sync.dma_start(
                        cc1_in[DR + IHD:, :].rearrange("(k p) t -> p k t", p=128), kvT_o[:])
                if not SKIP_CC:
                    nc.gpsimd.collective_compute("AllGather", ALU.bypass, replica_groups=RG,
                                                 ins=[cc1_in[:].opt()], outs=[cc1_out[:].opt()])

                # ---- q-part of qkv_a (overlaps CC1) ----
                with tc.tile_pool(name="ps_qp", bufs=1, space="PSUM") as Pp:
                    qc_ps2 = Pp.tile([TPC, QL], f32)
                    for k in range(KB):
                        waq_k = Pw.tile([128, QL], bf16, name="waq", tag="wstream")
                        nc.sync.dma_start(waq_k[:], WA[:].rearrange("(k p) n -> p k n", p=128)[:, k, :QL])
                        for j in range(3):
                            nc.tensor.matmul(qc_ps2[:, j * 512:(j + 1) * 512],
                                             hnT[:, k, :], waq_k[:, j * 512:(j + 1) * 512],
                                             start=(k == 0), stop=(k == KB - 1))
                    qsq = Pa.tile([TPC, QL], f32, name="qsq", tag="sq2")
                    qss = Pa.tile([TPC, 1], f32)
                    nc.scalar.activation(qsq[:], qc_ps2[:], AF.Square, accum_out=qss[:])
                    rq = Pa.tile([TPC, 1], f32)
                    nc.scalar.activation(rq[:], qss[:], AF.Sqrt, bias=eps_b[:], scale=1.0 / QL)
                    nc.vector.reciprocal(rq[:], rq[:])
                    qcn = Pe.tile([TPC, QL], bf16)
                    nc.vector.tensor_scalar(qcn[:], qc_ps2[:], rq[:], None, op0=ALU.mult)
                qcT = Pe.tile([128, QB, TPC], bf16)
                with tc.tile_pool(name="ps_qct", bufs=2, space="PSUM") as Pp:
                    for k in range(QB):
                        tpq = Pp.tile([128, 128], bf16, name="tpq", tag="tp")
                        nc.tensor.transpose(tpq[:], qcn[:, k * 128:(k + 1) * 128], idb[:])
                        nc.scalar.copy(qcT[:, k, :], tpq[:])

                # ---- iq (indexer q) FIRST: it gates the topk long pole ----
                iq_bf = Pe.tile([TPC, INH, IHD], bf16)
                qscale = Pa.tile([TPC, INH], f32)
                with tc.tile_pool(name="ps_iq", bufs=1, space="PSUM") as Pp:
                    iq_ps = Pp.tile([TPC, INH * IHD], f32)
                    for k in range(QB):
                        wiq_k = Pw.tile([128, INH * IHD], bf16, name="wiq", tag="wstream")
                        nc.sync.dma_start(wiq_k[:], WIQ[:].rearrange("(k p) n -> p k n", p=128)[:, k, :])
                        for j in range(4):
                            nc.tensor.matmul(iq_ps[:, j * 512:(j + 1) * 512], qcT[:, k, :],
                                             wiq_k[:, j * 512:(j + 1) * 512],
                                             start=(k == 0), stop=(k == QB - 1))
                    iqv = iq_ps[:].rearrange("p (h d) -> p h d", h=INH)
                    ipe = iqv[:, :, :DR].rearrange("p h (n two) -> p h n two", two=2)
                    ioe = iq_bf[:, :, :DR].rearrange("p h (n two) -> p h n two", two=2)
                    c3r = csr[:].rearrange("p (h n) -> p h n", h=NH)
                    s3r = snr[:].rearrange("p (h n) -> p h n", h=NH)
                    iq1 = Pa.tile([TPC, INH, 32], f32, name="iq1", tag="qt1")
                    iq2 = Pa.tile([TPC, INH, 32], f32, name="iq2", tag="qt2")
                    nc.vector.tensor_tensor(iq1[:], ipe[:, :, :, 0], c3r, op=ALU.mult)
                    nc.vector.tensor_tensor(iq2[:], ipe[:, :, :, 1], s3r, op=ALU.mult)
                    nc.vector.tensor_sub(ioe[:, :, :, 0], iq1[:], iq2[:])
                    nc.vector.tensor_tensor(iq1[:], ipe[:, :, :, 0], s3r, op=ALU.mult)
                    nc.vector.tensor_tensor(iq2[:], ipe[:, :, :, 1], c3r, op=ALU.mult)
                    nc.vector.tensor_add(ioe[:, :, :, 1], iq1[:], iq2[:])
                    nc.vector.tensor_copy(iq_bf[:, :, DR:], iqv[:, :, DR:])
                    nc.vector.tensor_reduce(qscale[:], iq_bf[:], AX.X, ALU.max,
                                            apply_absolute_value=True)
                # q_scale = exp2(ceil(log2(max(amax,1e-12)/448)))
                zz = Pa.tile([TPC, INH], f32)
                nc.vector.tensor_scalar(zz[:], qscale[:], 1e-12, 1.0 / FP8_MAX, op0=ALU.max, op1=ALU.mult)
                man = Pa.tile([TPC, INH], mybir.dt.uint32)
                nc.vector.tensor_scalar(man[:], zz[:].bitcast(mybir.dt.uint32), 0x007FFFFF, None, op0=ALU.bitwise_and)
                exb = Pa.tile([TPC, INH], mybir.dt.uint32)
                nc.vector.tensor_scalar(exb[:], zz[:].bitcast(mybir.dt.uint32), 0xFF800000, None, op0=ALU.bitwise_and)
                nc.vector.tensor_scalar(man[:], man[:], 0, None, op0=ALU.not_equal)
                nc.vector.tensor_scalar(man[:], man[:], 23, None, op0=ALU.logical_shift_left)
                nc.vector.tensor_tensor(exb[:], exb[:], man[:], op=ALU.add)
                nc.vector.tensor_scalar(qscale[:], exb[:].bitcast(f32), IDX_SCALE * (INH ** -0.5), None, op0=ALU.mult)

                iqT = Pe.tile([IHD, INH, TPC], bf16)
                with tc.tile_pool(name="ps_iqt", bufs=2, space="PSUM") as Pp:
                    for h in range(INH):
                        ti = Pp.tile([IHD, TPC], bf16, name="ti", tag="tp")
                        nc.tensor.transpose(ti[:], iq_bf[:, h, :], idb[:])
                        nc.scalar.copy(iqT[:, h, :], ti[:])

                # wts = (hn @ Wip + b) * qscale_scaled ; then diag(wts_h) mats
                wts = Pa.tile([TPC, INH], f32)
                with tc.tile_pool(name="ps_wp", bufs=1, space="PSUM") as Pp:
                    wip_sb = Pe.tile([128, KB, INH], bf16)
                    nc.sync.dma_start(wip_sb[:], WIP[:].rearrange("(k p) n -> p k n", p=128))
                    wp_ps = Pp.tile([TPC, INH], f32)
                    for k in range(KB):
                        nc.tensor.matmul(wp_ps[:], hnT[:, k, :], wip_sb[:, k, :],
                                         start=(k == 0), stop=(k == KB - 1))
                    nc.vector.tensor_add(wts[:], wp_ps[:], wpb_bc[:])
                    nc.vector.tensor_tensor(wts[:], wts[:], qscale[:], op=ALU.mult)
                dgw = Pe.tile([128, INH, 128], bf16)      # diag(wts_h) per head
                for h in range(INH):
                    nc.vector.tensor_scalar(dgw[:, h, :], idb[:], wts[:, h:h + 1], None, op0=ALU.mult)

                # causal additive mask as bf16 (injected into score PSUM via idb matmul)
                cadd_bf = Pe.tile([TPC, T], bf16)
                nc.vector.tensor_scalar(cadd_bf[:], caus[:], 1.0, -NEG, op0=ALU.subtract, op1=ALU.mult)

                # ---- gathered latent -> SBUF (global token order) ----
                kpeT_all = Pa.tile([DR, T], bf16)
                nc.scalar.dma_start(kpeT_all[:].rearrange("d (c t) -> d c t", c=NCORES),
                                    cc1_out[:, :DR, :].rearrange("c d t -> d c t"))
                iknT_all = Pe.tile([IHD, T], bf16)
                nc.scalar.dma_start(iknT_all[:].rearrange("d (c t) -> d c t", c=NCORES),
                                    cc1_out[:, DR:DR + IHD, :].rearrange("c d t -> d c t"))
                kvcT = Pa.tile([128, 4, T], bf16)
                for k in range(4):
                    nc.scalar.dma_start(
                        kvcT[:, k, :].rearrange("p (c t) -> p c t", c=NCORES),
                        cc1_out[:, DR + IHD + k * 128:DR + IHD + (k + 1) * 128, :]
                        .rearrange("c p t -> p c t"))

                # ---- indexer scores on PE: s_acc = mask + sum_h diag(wts_h) @ relu(s_h) ----
                s_acc = Pe.tile([TPC, T], f32)
                with tc.tile_pool(name="ps_s", bufs=1, space="PSUM") as Pp:
                    sa_ps = Pp.tile([TPC, T], f32, name="sa_ps")
                    for j in range(2):
                        nc.tensor.matmul(sa_ps[:, j * 512:(j + 1) * 512], idb[:],
                                         cadd_bf[:, j * 512:(j + 1) * 512],
                                         start=True, stop=False)
                    with tc.tile_pool(name="ps_sh", bufs=3, space="PSUM") as Pp2:
                        for h in range(INH):
                            s_ps = Pp2.tile([TPC, T], f32, name="s_ps", tag="sps")
                            for j in range(2):
                                nc.tensor.matmul(s_ps[:, j * 512:(j + 1) * 512], iqT[:, h, :],
                                                 iknT_all[:, j * 512:(j + 1) * 512],
                                                 start=True, stop=True)
                            rel_h = Pa.tile([TPC, T], bf16, name="rel_h", tag="relh", bufs=3)
                            nc.scalar.activation(rel_h[:], s_ps[:], AF.Relu)
                            for j in range(2):
                                nc.tensor.matmul(sa_ps[:, j * 512:(j + 1) * 512], dgw[:, h, :],
                                                 rel_h[:, j * 512:(j + 1) * 512],
                                                 start=False, stop=(h == INH - 1 and j == 1))
                    nc.scalar.copy(s_acc[:], sa_ps[:])

                # ---- topk threshold scan (DVE serial) ----
                scr = Pe.tile([TPC, T], f32, tag="scrt")
                nc.vector.tensor_copy(scr[:], s_acc[:])
                m8 = Pa.tile([TPC, 8], f32)
                for it in range(1 if SKIP_TOPK else TOPK // 8):
                    nc.vector.max(m8[:], scr[:])
                    nc.vector.match_replace(scr[:], m8[:], scr[:], -3e38)

                # ---- mask from scan threshold ----
                mask01 = Pe.tile([TPC, T], f32, tag="scrt")
                nc.vector.tensor_scalar(mask01[:], s_acc[:], m8[:, 7:8], None, op0=ALU.is_ge)
                nc.vector.tensor_tensor(mask01[:], mask01[:], caus[:], op=ALU.mult)
                madd_bf = Pa.tile([TPC, T], bf16)
                nc.vector.tensor_scalar(madd_bf[:], mask01[:], 1.0, -NEG, op0=ALU.subtract, op1=ALU.mult)


                # ==== work that overlaps the scan: q_b, V, K^T ====
                qtn = Pe.tile([TPC, NH, DN], bf16)    # q_nope * SCALE
                qtp = Pe.tile([TPC, NH, DR], bf16)    # roped q_pe * SCALE
                with tc.tile_pool(name="ps_q", bufs=1, space="PSUM") as Pp:
                    q_ps = Pp.tile([TPC, NH * DQ], f32)
                    for k in range(QB):
                        wqb_k = Pw.tile([128, NH * DQ], bf16, name="wqb", tag="wstream")
                        nc.sync.dma_start(wqb_k[:], WQB[:].rearrange("(k p) n -> p k n", p=128)[:, k, :])
                        for j in range(6):
                            nc.tensor.matmul(q_ps[:, j * 512:(j + 1) * 512], qcT[:, k, :],
                                             wqb_k[:, j * 512:(j + 1) * 512],
                                             start=(k == 0), stop=(k == QB - 1))
                    qv = q_ps[:].rearrange("p (h d) -> p h d", h=NH)
                    nc.vector.tensor_scalar(qtn[:], qv[:, :, :DN], SCALE, None, op0=ALU.mult)
                    pe3 = qv[:, :, DN:].rearrange("p h (n two) -> p h n two", two=2)
                    qo3 = qtp[:].rearrange("p h (n two) -> p h n two", two=2)
                    c3 = csq[:].rearrange("p (h n) -> p h n", h=NH)
                    s3 = snq[:].rearrange("p (h n) -> p h n", h=NH)
                    qt1 = Pa.tile([TPC, NH, 32], f32, name="qt1", tag="qt1")
                    qt2 = Pa.tile([TPC, NH, 32], f32, name="qt2", tag="qt2")
                    nc.vector.tensor_tensor(qt1[:], pe3[:, :, :, 0], c3, op=ALU.mult)
                    nc.vector.tensor_tensor(qt2[:], pe3[:, :, :, 1], s3, op=ALU.mult)
                    nc.vector.tensor_sub(qo3[:, :, :, 0], qt1[:], qt2[:])
                    nc.vector.tensor_tensor(qt1[:], pe3[:, :, :, 0], s3, op=ALU.mult)
                    nc.vector.tensor_tensor(qt2[:], pe3[:, :, :, 1], c3, op=ALU.mult)
                    nc.vector.tensor_add(qo3[:, :, :, 1], qt1[:], qt2[:])

                qtnT = Pa.tile([DN, NH, TPC], bf16)
                qtpT = Pa.tile([DR, NH, TPC], bf16)
                with tc.tile_pool(name="ps_qt", bufs=2, space="PSUM") as Pp:
                    for h in range(NH):
                        tq1 = Pp.tile([DN, TPC], bf16, name="tq1", tag="tp")
                        nc.tensor.transpose(tq1[:], qtn[:, h, :], idb[:])
                        nc.scalar.copy(qtnT[:, h, :], tq1[:])
                        tq2 = Pp.tile([DR, TPC], bf16, name="tq2", tag="tp")
                        nc.tensor.transpose(tq2[:], qtp[:, h, :], idb[:])
                        nc.scalar.copy(qtpT[:, h, :], tq2[:])

                Pe.release()
                Pl = tc.alloc_tile_pool(name="late", bufs=1)
                # V for all tokens -> SBUF resident
                v_all = Pl.tile([128, NCORES, NH * DV], bf16)
                wv_sb = Pa.tile([128, 4, NH * DV], bf16)
                nc.sync.dma_start(wv_sb[:], WV[:].rearrange("(k p) n -> p k n", p=128))
                with tc.tile_pool(name="ps_vall", bufs=2, space="PSUM") as Pp:
                    for tch in range(NCORES):
                        v_ps = Pp.tile([128, NH * DV], f32, name="v_ps", tag="vps")
                        for k in range(4):
                            for j in range(4):
                                nc.tensor.matmul(v_ps[:, j * 512:(j + 1) * 512],
                                                 kvcT[:, k, tch * 128:(tch + 1) * 128],
                                                 wv_sb[:, k, j * 512:(j + 1) * 512],
                                                 start=(k == 0), stop=(k == 3))
                        nc.scalar.copy(v_all[:, tch, :], v_ps[:])

                # K^T for all heads -> SBUF resident
                kt_all = Pl.tile([DN, NH, T], bf16)
                with tc.tile_pool(name="ps_ktb", bufs=2, space="PSUM") as Pp:
                    for h in range(NH):
                        wkn_h = Pw.tile([128, 4, DN], bf16, name="wkn_h", tag="wknh", bufs=3)
                        nc.sync.dma_start(
                            wkn_h[:],
                            WKN[:, h * DN:(h + 1) * DN].rearrange("(k p) n -> p k n", p=128))
                        kt_ps = Pp.tile([DN, T], f32, name="kt_ps", tag="ktp")
                        for j in range(2):
                            for k in range(4):
                                nc.tensor.matmul(kt_ps[:, j * 512:(j + 1) * 512],
                                                 wkn_h[:, k, :],
                                                 kvcT[:, k, j * 512:(j + 1) * 512],
                                                 start=(k == 0), stop=(k == 3))
                        nc.scalar.copy(kt_all[:, h, :], kt_ps[:])

                # ---- MLA attention ----
                oT = Pa.tile([DV, NH, TPC], bf16)
                with tc.tile_pool(name="ps_att", bufs=1, space="PSUM") as Pp:
                    for h in range(NH):
                        a_ps = Pp.tile([TPC, T], f32, name="a_ps", tag="sps", bufs=3)
                        for j in range(2):
                            nc.tensor.matmul(a_ps[:, j * 512:(j + 1) * 512], qtnT[:, h, :],
                                             kt_all[:, h, j * 512:(j + 1) * 512],
                                             start=True, stop=False)
                            nc.tensor.matmul(a_ps[:, j * 512:(j + 1) * 512], qtpT[:, h, :],
                                             kpeT_all[:, j * 512:(j + 1) * 512],
                                             start=False, stop=False)
                            nc.tensor.matmul(a_ps[:, j * 512:(j + 1) * 512], idb[:],
                                             madd_bf[:, j * 512:(j + 1) * 512],
                                             start=False, stop=True)
                        pex = Pa.tile([TPC, T], bf16, name="pex")
                        rs = Pa.tile([TPC, 1], f32, name="rs")
                        nc.scalar.activation(pex[:], a_ps[:], AF.Exp, accum_out=rs[:])
                        nc.vector.reciprocal(rs[:], rs[:])
                        pb = Pa.tile([TPC, T], bf16, name="pb")
                        nc.vector.tensor_scalar(pb[:], pex[:], rs[:], None, op0=ALU.mult)
                        # transpose P in 8 chunks; copy alternating DVE/Act; accumulate O^T
                        o_ps = Pp.tile([DV, TPC], f32, name="o_ps", tag="ops")
                        for s in range(8):
                            pt = Pp.tile([128, TPC], bf16, name="pt", tag="tp")
                            nc.tensor.transpose(pt[:], pb[:, s * 128:(s + 1) * 128], idb[:])
                            pts = Pa.tile([128, TPC], bf16, name="pts", tag="pts", bufs=4)
                            if s % 2 == 0:
                                nc.vector.tensor_copy(pts[:], pt[:])
                            else:
                                nc.scalar.copy(pts[:], pt[:])
                            nc.tensor.matmul(o_ps[:], v_all[:, s, h * DV:(h + 1) * DV], pts[:],
                                             start=(s == 0), stop=(s == 7))
                        nc.vector.tensor_copy(oT[:, h, :], o_ps[:])

                # ---- o_proj + residual ----
                x_own = Pa.tile([TPC, H], f32)
                with tc.tile_pool(name="ps_op", bufs=1, space="PSUM") as Pp:
                    d_ps = Pp.tile([TPC, H], f32)
                    for h in range(NH):
                        wo_k = Pw.tile([128, H], bf16, name="wo_k", tag="wstream")
                        nc.sync.dma_start(wo_k[:], WO[:].rearrange("(k p) n -> p k n", p=128)[:, h, :])
                        for j in range(4):
                            nc.tensor.matmul(d_ps[:, j * 512:(j + 1) * 512], oT[:, h, :],
                                             wo_k[:, j * 512:(j + 1) * 512],
                                             start=(h == 0), stop=(h == NH - 1))
                    nc.vector.tensor_tensor(x_own[:], d_ps[:], xo[:], op=ALU.add)

                # ---- post-LN pieces: r2, gate logits, rw, h2T_own ----
                sq2 = Pa.tile([TPC, H], f32, name="sq2a", tag="sq2")
                ss2 = Pa.tile([TPC, 1], f32)
                nc.scalar.activation(sq2[:], x_own[:], AF.Square, accum_out=ss2[:])
                r2 = Pa.tile([TPC, 1], f32)
                nc.scalar.activation(r2[:], ss2[:], AF.Sqrt, bias=eps_b[:], scale=1.0 / H)
                nc.vector.reciprocal(r2[:], r2[:])
                xT_own = Pa.tile([128, KB, TPC], f32)
                with tc.tile_pool(name="ps_xt", bufs=2, space="PSUM") as Pp:
                    for k in range(KB):
                        tx = Pp.tile([128, TPC], f32, name="tx", tag="tpf")
                        nc.tensor.transpose(tx[:], x_own[:, k * 128:(k + 1) * 128], idf[:])
                        nc.scalar.copy(xT_own[:, k, :], tx[:])
                lg = Pa.tile([TPC, NE], f32)
                with tc.tile_pool(name="ps_g", bufs=1, space="PSUM") as Pp:
                    l_ps = Pp.tile([TPC, NE], f32)
                    for k in range(KB):
                        nc.tensor.matmul(l_ps[:], xT_own[:, k, :], wg_sb[:, k, :],
                                         start=(k == 0), stop=(k == KB - 1))
                    nc.scalar.activation(lg[:], l_ps[:], AF.Copy, scale=r2[:])
                gm8 = Pa.tile([TPC, 8], f32)
                nc.vector.max(gm8[:], lg[:])
                negm0 = Pa.tile([TPC, 1], f32)
                nc.vector.tensor_scalar(negm0[:], gm8[:, 0:1], -1.0, None, op0=ALU.mult)
                el = Pa.tile([TPC, NE], f32)
                nc.scalar.activation(el[:], lg[:], AF.Exp, bias=negm0[:])
                dn1 = Pa.tile([TPC, 1], f32)
                nc.vector.tensor_tensor(dn1[:], gm8[:, 1:2], gm8[:, 0:1], op=ALU.subtract)
                nc.scalar.activation(dn1[:], dn1[:], AF.Exp)
                nc.vector.tensor_scalar(dn1[:], dn1[:], 1.0, None, op0=ALU.add)
                nc.vector.reciprocal(dn1[:], dn1[:])
                sel = Pa.tile([TPC, NE], f32)
                nc.vector.tensor_scalar(sel[:], lg[:], gm8[:, 1:2], None, op0=ALU.is_ge)
                rw = Pa.tile([TPC, NE], f32)
                nc.vector.scalar_tensor_tensor(rw[:], el[:], dn1[:], sel[:],
                                               op0=ALU.mult, op1=ALU.mult)

                # h2T_own in [t', k] layout (feature-major transport)
                r2row = Pa.tile([1, TPC], f32)
                r2bc = Pa.tile([128, TPC], f32)
                with tc.tile_pool(name="ps_r2", bufs=1, space="PSUM") as Pp:
                    r2p = Pp.tile([1, TPC], f32)
                    nc.tensor.transpose(r2p[:], r2[:], idf[:])
                    nc.scalar.copy(r2row[:], r2p[:])
                nc.gpsimd.partition_broadcast(r2bc[:], r2row[:])
                h2T_own = Pa.tile([128, TPC, KB], bf16)
                for k in range(KB):
                    nc.vector.tensor_tensor(h2T_own[:, :, k], xT_own[:, k, :], r2bc[:], op=ALU.mult)

                # ---- CC2 in two half-token slabs (first carries rw) ----
                nc.scalar.dma_start(cch0_in[:, :HLF * KB],
                                    h2T_own[:, :HLF, :].rearrange("p t k -> p (t k)"))
                nc.scalar.dma_start(cch0_in[:, HLF * KB:], rw[:].bitcast(bf16))
                if not SKIP_CC:
                    nc.gpsimd.collective_compute("AllGather", ALU.bypass, replica_groups=RG,
                                                 ins=[cch0_in[:].opt()], outs=[cch0_out[:].opt()])
                nc.scalar.dma_start(cch1_in[:],
                                    h2T_own[:, HLF:, :].rearrange("p t k -> p (t k)"))
                if not SKIP_CC:
                    nc.gpsimd.collective_compute("AllGather", ALU.bypass, replica_groups=RG,
                                                 ins=[cch1_in[:].opt()], outs=[cch1_out[:].opt()])

                # ---- shared expert on own tokens (overlaps CC2) ----
                ss_own = Pa.tile([TPC, SI], bf16)
                with tc.tile_pool(name="ps_shx", bufs=1, space="PSUM") as Pp:
                    gs_ps = Pp.tile([TPC, SI], f32, name="gs_ps")
                    us_ps = Pp.tile([TPC, SI], f32, name="us_ps")
                    for k in range(KB):
                        wsg_k = Pw.tile([128, SI], bf16, name="wsg_k", tag="wstream")
                        nc.sync.dma_start(wsg_k[:], WSG[:].rearrange("(k p) n -> p k n", p=128)[:, k, :])
                        wsu_k = Pw.tile([128, SI], bf16, name="wsu_k", tag="wstream")
                        nc.sync.dma_start(wsu_k[:], WSU[:].rearrange("(k p) n -> p k n", p=128)[:, k, :])
                        for j in range(2):
                            nc.tensor.matmul(gs_ps[:, j * 512:(j + 1) * 512], h2T_own[:, :, k],
                                             wsg_k[:, j * 512:(j + 1) * 512],
                                             start=(k == 0), stop=(k == KB - 1))
                            nc.tensor.matmul(us_ps[:, j * 512:(j + 1) * 512], h2T_own[:, :, k],
                                             wsu_k[:, j * 512:(j + 1) * 512],
                                             start=(k == 0), stop=(k == KB - 1))
                    sgo = Pa.tile([TPC, SI], f32, name="sgo", tag="sq2")
                    nc.scalar.activation(sgo[:], gs_ps[:], AF.Silu)
                    nc.vector.tensor_tensor(ss_own[:], sgo[:], us_ps[:], op=ALU.mult)
                ssT = Pa.tile([128, 8, TPC], bf16)
                with tc.tile_pool(name="ps_st", bufs=2, space="PSUM") as Pp:
                    for m in range(8):
                        tss = Pp.tile([128, TPC], bf16, name="tss", tag="tp")
                        nc.tensor.transpose(tss[:], ss_own[:, m * 128:(m + 1) * 128], idb[:])
                        nc.vector.tensor_copy(ssT[:, m, :], tss[:])
                with tc.tile_pool(name="ps_sd", bufs=1, space="PSUM") as Pp:
                    sh_ps = Pp.tile([TPC, H], f32)
                    for m in range(8):
                        wsd_m = Pw.tile([128, H], bf16, name="wsd_m", tag="wstream")
                        nc.sync.dma_start(wsd_m[:], WSD[:].rearrange("(k p) n -> p k n", p=128)[:, m, :])
                        for j in range(4):
                            nc.tensor.matmul(sh_ps[:, j * 512:(j + 1) * 512], ssT[:, m, :],
                                             wsd_m[:, j * 512:(j + 1) * 512],
                                             start=(m == 0), stop=(m == 7))
                    outx = Pa.tile([TPC, H], f32, name="outx", tag="sq2")
                    nc.vector.tensor_tensor(outx[:], sh_ps[:], x_own[:], op=ALU.add)
                nc.scalar.dma_start(OUT_X[:], outx[:])
                Pl.release()

            # =================== MoE phase (expert-parallel, dense) ===================
            with tc.tile_pool(name="moe", bufs=1) as Pm:
                weg = Pm.tile([128, MI // 128, KB, 128], bf16)
                weu = Pm.tile([128, MI // 128, KB, 128], bf16)
                for m in range(MI // 128):
                    nc.sync.dma_start(weg[:, m, :, :].rearrange("p k n -> p (k n)"), WEG[:][m])
                    nc.sync.dma_start(weu[:, m, :, :].rearrange("p k n -> p (k n)"), WEU[:][m])
                wed = Pm.tile([128, MI // 128, H], bf16)
                for m in range(MI // 128):
                    nc.sync.dma_start(wed[:, m, :], WED[:][m])
                # gathered h2T halves [p, c, t'(64), k]
                h2h0 = Pm.tile([128, NCORES, HLF, KB], bf16)
                nc.scalar.dma_start(
                    h2h0[:].rearrange("p c t k -> p c (t k)"),
                    cch0_out[:, :, :HLF * KB].rearrange("c p n -> p c n"))
                h2h1 = Pm.tile([128, NCORES, HLF, KB], bf16)
                nc.scalar.dma_start(
                    h2h1[:].rearrange("p c t k -> p c (t k)"),
                    cch1_out[:].rearrange("c p n -> p c n"))
                # rw for all tokens: [p=token-in-chunk, c, 8] f32 (bitcast pairs);
                # select own-expert column via one-hot dot on DVE
                rw_sb = Pm.tile([128, NCORES, 2 * NE], bf16)
                nc.scalar.dma_start(rw_sb[:],
                                    cch0_out[:, :, HLF * KB:].rearrange("c p n -> p c n"))
                rwe = Pm.tile([128, NCORES], f32)
                rwt = Pm.tile([128, NE], f32, name="rwt")
                for tch in range(NCORES):
                    nc.vector.tensor_tensor(rwt[:], rw_sb[:, tch, :].bitcast(f32), oh_bc[:], op=ALU.mult)
                    nc.vector.tensor_reduce(rwe[:, tch:tch + 1], rwt[:], AX.X, ALU.add)

                su = Pm.tile([128, MI // 128, T], bf16)   # silu(g)*u  [mi, (c t')]
                suv = su[:].rearrange("p m (c t) -> p m c t", c=NCORES)
                with tc.tile_pool(name="ps_moe", bufs=2, space="PSUM") as Pp:
                    for half, h2h in ((0, h2h0), (1, h2h1)):
                        for m in range(MI // 128):
                            g_ps = Pp.tile([128, 512], f32, name="g_ps", tag="gps")
                            u_ps = Pp.tile([128, 512], f32, name="u_ps", tag="ups")
                            gv = g_ps[:].rearrange("p (c t) -> p c t", c=NCORES)
                            uv = u_ps[:].rearrange("p (c t) -> p c t", c=NCORES)
                            for k in range(KB):
                                nc.tensor.matmul(g_ps[:], weg[:, m, k, :],
                                                 h2h[:, :, :, k].rearrange("p c t -> p (c t)"),
                                                 start=(k == 0), stop=(k == KB - 1))
                                nc.tensor.matmul(u_ps[:], weu[:, m, k, :],
                                                 h2h[:, :, :, k].rearrange("p c t -> p (c t)"),
                                                 start=(k == 0), stop=(k == KB - 1))
                            sg = Pm.tile([128, 512], f32, name="sg", tag="sgs", bufs=2)
                            nc.scalar.activation(sg[:], g_ps[:], AF.Silu)
                            nc.vector.tensor_tensor(sg[:], sg[:], u_ps[:], op=ALU.mult)
                            nc.vector.tensor_copy(
                                suv[:, m, :, half * HLF:(half + 1) * HLF],
                                sg[:].rearrange("p (c t) -> p c t", c=NCORES))

                with tc.tile_pool(name="ps_dn", bufs=2, space="PSUM") as Pp:
                    for tch in range(8):
                        dn_ps = Pp.tile([128, H], f32, name="dn_ps", tag="dnp")
                        for m in range(8):
                            for j in range(4):
                                nc.tensor.matmul(dn_ps[:, j * 512:(j + 1) * 512],
                                                 su[:, m, tch * 128:(tch + 1) * 128],
                                                 wed[:, m, j * 512:(j + 1) * 512],
                                                 start=(m == 0), stop=(m == 7))
                        ob = Pm.tile([128, H], bf16, name="ob", tag="obs")
                        # scale rows by rw[token, own_expert] (per-partition ptr)
                        nc.scalar.activation(ob[:], dn_ps[:], AF.Copy,
                                             scale=rwe[:, tch:tch + 1])
                        nc.sync.dma_start(OUT_P[:].rearrange("(c p) n -> c p n", p=128)[tch], ob[:])

    nc.compile()
    return nc


_NC = None


def kernel(**inputs):
    global _NC
    inp = {k: np.asarray(v) for k, v in inputs.items()}
    pos = inp["positions"].astype(np.int64)
    x = inp["hidden_states"].astype(np.float32)

    # ---- fold layernorm weights into downstream mats (host prep) ----
    iw = inp["input_ln_w"].astype(np.float32)
    qw = inp["q_a_ln_w"].astype(np.float32)
    kw = inp["kv_a_ln_w"].astype(np.float32)
    pw = inp["post_ln_w"].astype(np.float32)
    Wa = (iw[:, None] * inp["W_qkv_a"]).astype(BF)
    Wik = (iw[:, None] * inp["idx_wk"]).astype(BF)
    Wip = (iw[:, None] * inp["idx_wp_w"]).astype(BF)
    Wqb = (qw[:, None] * inp["W_q_b"]).astype(BF)
    Wiq = (qw[:, None] * inp["idx_wq_b"]).astype(BF)
    Wkvb = (kw[:, None] * inp["W_kv_b"]).astype(np.float32).reshape(KL, NH, DN + DV)
    Wkn = np.ascontiguousarray(Wkvb[:, :, :DN].reshape(KL, NH * DN)).astype(BF)
    Wv = np.ascontiguousarray(Wkvb[:, :, DN:].reshape(KL, NH * DV)).astype(BF)
    Wo = inp["W_o"].astype(BF)
    Wg = (pw[:, None] * inp["W_gate"]).astype(np.float32)
    Weg = (pw[None, :, None] * inp["We_gate"]).astype(BF)
    Weu = (pw[None, :, None] * inp["We_up"]).astype(BF)
    Wed = inp["We_down"].astype(BF)
    Wsg = (pw[:, None] * inp["Ws_gate"]).astype(BF)
    Wsu = (pw[:, None] * inp["Ws_up"]).astype(BF)
    Wsd = inp["Ws_down"].astype(BF)

    # relayout expert weights: [H, MI] -> [m][p][k*128+mi'] with H=(k,p)
    def relay_up(W):   # [H, MI] -> [8, 128, 16*128]
        Wr = W.reshape(KB, 128, MI // 128, 128)          # k p m mi'
        return np.ascontiguousarray(Wr.transpose(2, 1, 0, 3).reshape(MI // 128, 128, KB * 128))

    def relay_dn(W):   # [MI, H] -> [8, 128, H]
        return np.ascontiguousarray(W.reshape(MI // 128, 128, H))

    inv = 1.0 / (BASE ** (np.arange(0, DR, 2, dtype=np.float32) / DR))
    ang = pos.astype(np.float32)[:, None] * inv           # [T, 32]
    cs_a, sn_a = np.cos(ang), np.sin(ang)

    in_maps = []
    for c in range(NCORES):
        rows = list(range(c * TPC, (c + 1) * TPC))
        posn = pos[rows]
        causm = (posn[:, None] >= pos[None, :]).astype(np.float32)
        cs = cs_a[rows]; sn = sn_a[rows]
        oh = np.zeros((1, NE), np.float32); oh[0, c] = 1.0
        in_maps.append({
            "OH": oh,
            "XO": np.ascontiguousarray(x[rows]),
            "CAUS": np.ascontiguousarray(causm),
            "CSQ": np.ascontiguousarray(np.tile(cs * SCALE, (1, NH)).astype(np.float32)),
            "SNQ": np.ascontiguousarray(np.tile(sn * SCALE, (1, NH)).astype(np.float32)),
            "CSR": np.ascontiguousarray(np.tile(cs, (1, NH)).astype(np.float32)),
            "SNR": np.ascontiguousarray(np.tile(sn, (1, NH)).astype(np.float32)),
            "KNW": inp["idx_kn_w"].astype(np.float32).reshape(1, IHD),
            "KNB": inp["idx_kn_b"].astype(np.float32).reshape(1, IHD),
            "WPB": inp["idx_wp_b"].astype(np.float32).reshape(1, INH),
            "WA": Wa, "WQB": Wqb, "WIQ": Wiq, "WIK": Wik, "WIP": Wip,
            "WKN": Wkn, "WV": Wv, "WO": Wo, "WG": Wg,
            "WEG": relay_up(Weg[c]),
            "WEU": relay_up(Weu[c]),
            "WED": relay_dn(Wed[c]),
            "WSG": Wsg, "WSU": Wsu, "WSD": Wsd,
        })

    if _NC is None:
        _NC = build()
    try:
        res = run_bass_kernel_spmd(_NC, in_maps, core_ids=list(range(NCORES)))
    except Exception:
        import time as _time
        _time.sleep(2.0)
        res = run_bass_kernel_spmd(_NC, in_maps, core_ids=list(range(NCORES)))

    out = np.zeros((T, H), np.float64)
    for c in range(NCORES):
        out += res.results[c]["OUT_P"].astype(np.float64)
    for c in range(NCORES):
        out[c * TPC:(c + 1) * TPC] += res.results[c]["OUT_X"].astype(np.float64)
    return out.astype(np.float32)


# revision 18
# speedup vs baseline: 1.0406x; 1.0406x over previous
"""Self-contained Trainium2 Bass kernel for the DeepseekV2 decoder layer problem.

Sharding (8 cores): core c owns the contiguous 128-token block [128c, 128c+128).
KV-side projections are computed per-own-token and AllGathered as one bundle
(kpe^T / ik^T / kv_latent^T).  Indexer scores + top-k + MLA attention + o_proj
run on own rows.  h2 is transported feature-major (h2^T) in two half-token
AllGathers (second half carries the router weights); MoE is expert-parallel
(1 routed expert per core, dense over all tokens) plus the shared expert on
own tokens.  Host sums the per-core partials.
"""
import sys
sys.path.insert(0, "/opt/trn_rl_repo")
import numpy as np
import ml_dtypes

import concourse.bass as bass
import concourse.mybir as mybir
from concourse import bacc, tile
from concourse.bass_utils import run_bass_kernel_spmd
from concourse.masks import make_identity

f32 = mybir.dt.float32
bf16 = mybir.dt.bfloat16
AF = mybir.ActivationFunctionType
ALU = mybir.AluOpType
AX = mybir.AxisListType
BF = ml_dtypes.bfloat16

# dims
T = 1024; H = 2048; NH = 16; DN = 128; DR = 64; DQ = DN + DR; DV = 128
QL = 1536; KL = 512
INH = 16; IHD = 128; TOPK = 256
NE = 8; MI = 1024; SI = 1024
BASE = 10000.0; EPS = 1e-6
SCALE = DQ ** -0.5
IDX_SCALE = IHD ** -0.5
FP8_MAX = 448.0
NCORES = 8
TPC = T // NCORES        # 128 tokens per core
NEG = -1e30
import os
SKIP_CC = os.environ.get("SKIP_CC") == "1"
SKIP_TOPK = os.environ.get("SKIP_TOPK") == "1"

KB = 16   # H/128 k-chunks
QB = 12   # QL/128
RG = [list(range(NCORES))]
CCL = DR + IHD + KL          # merged latent collective rows (704)
HLF = TPC // 2               # 64 tokens per h2 half


def build():
    nc = bacc.Bacc("TRN2", target_bir_lowering=False,
                   debug=os.environ.get("BASS_DEBUG") == "1",
                   enable_asserts=False, num_devices=NCORES)

    def din(name, shape, dt=bf16):
        return nc.dram_tensor(name, shape, dt, kind="ExternalInput").ap()

    # ---- per-core inputs ----
    XO = din("XO", [TPC, H], f32)              # x_in own rows
    CAUS = din("CAUS", [TPC, T], f32)          # causal01 over global keys
    CSR = din("CSR", [TPC, 512], f32)          # cos tiled 16x (unscaled)
    SNR = din("SNR", [TPC, 512], f32)
    OH = din("OH", [1, NE], f32)               # own-expert one-hot
    KNW = din("KNW", [1, IHD], f32)            # idx_kn_w
    KNB = din("KNB", [1, IHD], f32)
    WPB = din("WPB", [1, INH], f32)            # idx_wp_b
    WA = din("WA", [H, QL + KL + DR])          # bf16, ln-folded
    WQB = din("WQB", [QL, NH * DQ])
    WIQ = din("WIQ", [QL, INH * IHD])
    WIK = din("WIK", [H, IHD])
    WIP = din("WIP", [H, INH])
    WKN = din("WKN", [KL, NH * DN])
    WV = din("WV", [KL, NH * DV])
    WO = din("WO", [NH * DV, H])
    WG = din("WG", [H, NE], f32)
    WEG = din("WEG", [MI // 128, 128, KB * 128])   # [m][p][k*128+mi'] host-relaid
    WEU = din("WEU", [MI // 128, 128, KB * 128])
    WED = din("WED", [MI // 128, 128, H])          # [m][p=mi-in-chunk][H]
    WSG = din("WSG", [H, SI])
    WSU = din("WSU", [H, SI])
    WSD = din("WSD", [SI, H])

    OUT_P = nc.dram_tensor("OUT_P", [T, H], bf16, kind="ExternalOutput").ap()
    OUT_X = nc.dram_tensor("OUT_X", [TPC, H], f32, kind="ExternalOutput").ap()

    with tile.TileContext(nc) as tc:
        with tc.tile_pool(name="const", bufs=1) as Pc, \
             tc.tile_pool(name="dram", bufs=1, space="DRAM") as Pd:
            idf = Pc.tile([128, 128], f32)
            make_identity(nc, idf[:])
            idb = Pc.tile([128, 128], bf16)
            nc.vector.tensor_copy(idb[:], idf[:])
            eps_b = Pc.tile([128, 1], f32)
            nc.vector.memset(eps_b[:], EPS)

            xo = Pc.tile([TPC, H], f32)
            nc.sync.dma_start(xo[:], XO[:])
            caus = Pc.tile([TPC, T], f32)
            nc.sync.dma_start(caus[:], CAUS[:])
            csr = Pc.tile([TPC, 512], f32); nc.sync.dma_start(csr[:], CSR[:])
            snr = Pc.tile([TPC, 512], f32); nc.sync.dma_start(snr[:], SNR[:])
            knw_r = Pc.tile([1, IHD], f32); nc.sync.dma_start(knw_r[:], KNW[:])
            knb_r = Pc.tile([1, IHD], f32); nc.sync.dma_start(knb_r[:], KNB[:])
            wpb_r = Pc.tile([1, INH], f32); nc.sync.dma_start(wpb_r[:], WPB[:])
            knw_bc = Pc.tile([128, IHD], f32)
            nc.gpsimd.partition_broadcast(knw_bc[:], knw_r[:])
            knb_bc = Pc.tile([128, IHD], f32)
            nc.gpsimd.partition_broadcast(knb_bc[:], knb_r[:])
            wpb_bc = Pc.tile([128, INH], f32)
            nc.gpsimd.partition_broadcast(wpb_bc[:], wpb_r[:])
            wg_sb = Pc.tile([128, KB, NE], f32)
            nc.sync.dma_start(wg_sb[:], WG[:].rearrange("(k p) n -> p k n", p=128))
            oh_r = Pc.tile([1, NE], f32); nc.sync.dma_start(oh_r[:], OH[:])
            oh_bc = Pc.tile([128, NE], f32)
            nc.gpsimd.partition_broadcast(oh_bc[:], oh_r[:])

            # collective buffers
            cc1_in = Pd.tile([CCL, TPC], bf16)
            cc1_out = Pd.tile([NCORES, CCL, TPC], bf16, addr_space="Shared")
            HRW = HLF * KB + 2 * NE        # half-token h2T cols + rw bf16 pairs
            cch0_in = Pd.tile([128, HRW], bf16)
            cch0_out = Pd.tile([NCORES, 128, HRW], bf16, addr_space="Shared")
            cch1_in = Pd.tile([128, HLF * KB], bf16)
            cch1_out = Pd.tile([NCORES, 128, HLF * KB], bf16, addr_space="Shared")

            with tc.tile_pool(name="att", bufs=1) as Pa, \
                 tc.tile_pool(name="wstream", bufs=2) as Pw:
                Pe = tc.alloc_tile_pool(name="early", bufs=1)
                # rmsnorm scale r1 for own rows
                sq = Pa.tile([TPC, H], f32, name="sq_scratch", tag="sq2")
                ssq = Pa.tile([TPC, 1], f32)
                nc.scalar.activation(sq[:], xo[:], AF.Square, accum_out=ssq[:])
                r1 = Pa.tile([TPC, 1], f32)
                nc.scalar.activation(r1[:], ssq[:], AF.Sqrt, bias=eps_b[:], scale=1.0 / H)
                nc.vector.reciprocal(r1[:], r1[:])
                hn_own = Pe.tile([TPC, H], bf16)
                nc.vector.tensor_scalar(hn_own[:], xo[:], r1[:], None, op0=ALU.mult)
                hnT = Pe.tile([128, KB, TPC], bf16)
                with tc.tile_pool(name="ps_tr", bufs=2, space="PSUM") as Pp:
                    for k in range(KB):
                        tp = Pp.tile([128, 128], bf16, name="tp")
                        nc.tensor.transpose(tp[:], hn_own[:, k * 128:(k + 1) * 128], idb[:])
                        nc.scalar.copy(hnT[:, k, :], tp[:])

                # ---- qkv_a: kv+kpe columns FIRST so CC1 can launch early ----
                with tc.tile_pool(name="ps_qkv", bufs=1, space="PSUM") as Pp:
                    kvp_ps = Pp.tile([TPC, KL + DR], f32)
                    for k in range(KB):
                        wakv_k = Pw.tile([128, KL + DR], bf16, name="wakv", tag="wknh", bufs=3)
                        with tc.high_priority():
                            nc.sync.dma_start(wakv_k[:], WA[:].rearrange("(k p) n -> p k n", p=128)[:, k, QL:])
                        nc.tensor.matmul(kvp_ps[:, 0:512], hnT[:, k, :], wakv_k[:, 0:512],
                                         start=(k == 0), stop=(k == KB - 1))
                        nc.tensor.matmul(kvp_ps[:, 512:], hnT[:, k, :], wakv_k[:, 512:],
                                         start=(k == 0), stop=(k == KB - 1))
                    # kv_c rmsnorm -> bf16
                    ksq = Pa.tile([TPC, KL], f32, name="ksq", tag="sq2")
                    kss = Pa.tile([TPC, 1], f32)
                    nc.scalar.activation(ksq[:], kvp_ps[:, :KL], AF.Square, accum_out=kss[:])
                    rkv = Pa.tile([TPC, 1], f32)
                    nc.scalar.activation(rkv[:], kss[:], AF.Sqrt, bias=eps_b[:], scale=1.0 / KL)
                    nc.vector.reciprocal(rkv[:], rkv[:])
                    kvn = Pa.tile([TPC, KL], bf16)
                    nc.vector.tensor_scalar(kvn[:], kvp_ps[:, :KL], rkv[:], None, op0=ALU.mult)

                    # k_pe rope (unscaled tables) -> bf16 [TPC, 64]
                    kpe = Pa.tile([TPC, DR], bf16)
                    t1 = Pa.tile([TPC, 32], f32, name="rt1", tag="rt1")
                    t2 = Pa.tile([TPC, 32], f32, name="rt2", tag="rt2")
                    pe_src = kvp_ps[:, KL:].rearrange("p (n two) -> p n two", two=2)
                    x1, x2 = pe_src[:, :, 0], pe_src[:, :, 1]
                    ko = kpe[:].rearrange("p (n two) -> p n two", two=2)
                    nc.vector.tensor_tensor(t1[:], x1, csr[:, :32], op=ALU.mult)
                    nc.vector.tensor_tensor(t2[:], x2, snr[:, :32], op=ALU.mult)
                    nc.vector.tensor_sub(ko[:, :, 0], t1[:], t2[:])
                    nc.vector.tensor_tensor(t1[:], x1, snr[:, :32], op=ALU.mult)
                    nc.vector.tensor_tensor(t2[:], x2, csr[:, :32], op=ALU.mult)
                    nc.vector.tensor_add(ko[:, :, 1], t1[:], t2[:])

                # ---- ik own: layernorm(hn @ Wik) + rope ----
                ikn = Pa.tile([TPC, IHD], bf16)
                with tc.tile_pool(name="ps_ik", bufs=1, space="PSUM") as Pp:
                    wik_sb = Pe.tile([128, KB, IHD], bf16)
                    with tc.high_priority():
                        nc.sync.dma_start(wik_sb[:], WIK[:].rearrange("(k p) n -> p k n", p=128))
                    ik_ps = Pp.tile([TPC, IHD], f32)
                    for k in range(KB):
                        nc.tensor.matmul(ik_ps[:], hnT[:, k, :], wik_sb[:, k, :],
                                         start=(k == 0), stop=(k == KB - 1))
                    negm = Pa.tile([TPC, 1], f32)
                    nc.vector.tensor_reduce(negm[:], ik_ps[:], AX.X, ALU.add, negate=True)
                    nc.vector.tensor_scalar(negm[:], negm[:], 1.0 / IHD, None, op0=ALU.mult)
                    xm = Pa.tile([TPC, IHD], f32)
                    nc.vector.tensor_scalar(xm[:], ik_ps[:], negm[:], None, op0=ALU.add)
                    xms = Pa.tile([TPC, IHD], f32)
                    vss = Pa.tile([TPC, 1], f32)
                    nc.scalar.activation(xms[:], xm[:], AF.Square, accum_out=vss[:])
                    rstd = Pa.tile([TPC, 1], f32)
                    nc.scalar.activation(rstd[:], vss[:], AF.Sqrt, bias=eps_b[:], scale=1.0 / IHD)
                    nc.vector.reciprocal(rstd[:], rstd[:])
                    ikf = Pa.tile([TPC, IHD], f32)
                    nc.vector.scalar_tensor_tensor(ikf[:], xm[:], rstd[:], knw_bc[:],
                                                   op0=ALU.mult, op1=ALU.mult)
                    nc.vector.tensor_add(ikf[:], ikf[:], knb_bc[:])
                    pe2 = ikf[:, :DR].rearrange("p (n two) -> p n two", two=2)
                    iko2 = ikn[:, :DR].rearrange("p (n two) -> p n two", two=2)
                    it1 = Pa.tile([TPC, 32], f32, name="it1", tag="rt1")
                    it2 = Pa.tile([TPC, 32], f32, name="it2", tag="rt2")
                    nc.vector.tensor_tensor(it1[:], pe2[:, :, 0], csr[:, :32], op=ALU.mult)
                    nc.vector.tensor_tensor(it2[:], pe2[:, :, 1], snr[:, :32], op=ALU.mult)
                    nc.vector.tensor_sub(iko2[:, :, 0], it1[:], it2[:])
                    nc.vector.tensor_tensor(it1[:], pe2[:, :, 0], snr[:, :32], op=ALU.mult)
                    nc.vector.tensor_tensor(it2[:], pe2[:, :, 1], csr[:, :32], op=ALU.mult)
                    nc.vector.tensor_add(iko2[:, :, 1], it1[:], it2[:])
                    nc.vector.tensor_copy(ikn[:, DR:], ikf[:, DR:])

                # transposes of kpe, ikn, kvn -> merged CC1 input
                with tc.tile_pool(name="ps_tr2", bufs=2, space="PSUM") as Pp:
                    kpeT_o = Pa.tile([DR, TPC], bf16)
                    tpp = Pp.tile([DR, 128], bf16, name="tpp", tag="tp")
                    nc.tensor.transpose(tpp[:], kpe[:], idb[:])
                    nc.scalar.copy(kpeT_o[:], tpp[:])
                    nc.sync.dma_start(cc1_in[:DR, :], kpeT_o[:])
                    iknT_o = Pa.tile([IHD, TPC], bf16)
                    tpi = Pp.tile([IHD, TPC], bf16, name="tpi", tag="tp")
                    nc.tensor.transpose(tpi[:], ikn[:], idb[:])
                    nc.scalar.copy(iknT_o[:], tpi[:])
                    nc.sync.dma_start(cc1_in[DR:DR + IHD, :], iknT_o[:])
                    kvT_o = Pa.tile([128, 4, TPC], bf16)
                    for k in range(4):
                        tpk = Pp.tile([128, 128], bf16, name="tpk", tag="tp")
                        nc.tensor.transpose(tpk[:], kvn[:, k * 128:(k + 1) * 128], idb[:])
                        nc.scalar.copy(kvT_o[:, k, :], tpk[:])
                    nc.sync.dma_start(
                        cc1_in[DR + IHD:, :].rearrange("(k p) t -> p k t", p=128), kvT_o[:])
                if not SKIP_CC:
                    nc.gpsimd.collective_compute("AllGather", ALU.bypass, replica_groups=RG,
                                                 ins=[cc1_in[:].opt()], outs=[cc1_out[:].opt()])

                # ---- q-part of qkv_a (overlaps CC1) ----
                with tc.tile_pool(name="ps_qp", bufs=1, space="PSUM") as Pp:
                    qc_ps2 = Pp.tile([TPC, QL], f32)
                    for k in range(KB):
                        waq_k = Pw.tile([128, QL], bf16, name="waq", tag="wstream")
                        nc.sync.dma_start(waq_k[:], WA[:].rearrange("(k p) n -> p k n", p=128)[:, k, :QL])
                        for j in range(3):
                            nc.tensor.matmul(qc_ps2[:, j * 512:(j + 1) * 512],
                                             hnT[:, k, :], waq_k[:, j * 512:(j + 1) * 512],
                                             start=(k == 0), stop=(k == KB - 1))
                    qsq = Pa.tile([TPC, QL], f32, name="qsq", tag="sq2")
                    qss = Pa.tile([TPC, 1], f32)
                    nc.scalar.activation(qsq[:], qc_ps2[:], AF.Square, accum_out=qss[:])
                    rq = Pa.tile([TPC, 1], f32)
                    nc.scalar.activation(rq[:], qss[:], AF.Sqrt, bias=eps_b[:], scale=1.0 / QL)
                    nc.vector.reciprocal(rq[:], rq[:])
                    qcn = Pe.tile([TPC, QL], bf16)
                    nc.vector.tensor_scalar(qcn[:], qc_ps2[:], rq[:], None, op0=ALU.mult)
                qcT = Pe.tile([128, QB, TPC], bf16)
                with tc.tile_pool(name="ps_qct", bufs=2, space="PSUM") as Pp:
                    for k in range(QB):
                        tpq = Pp.tile([128, 128], bf16, name="tpq", tag="tp")
                        nc.tensor.transpose(tpq[:], qcn[:, k * 128:(k + 1) * 128], idb[:])
                        nc.scalar.copy(qcT[:, k, :], tpq[:])

                # ---- iq (indexer q) FIRST: it gates the topk long pole ----
                iq_bf = Pe.tile([TPC, INH, IHD], bf16)
                qscale = Pa.tile([TPC, INH], f32)
                with tc.tile_pool(name="ps_iq", bufs=1, space="PSUM") as Pp:
                    iq_ps = Pp.tile([TPC, INH * IHD], f32)
                    for k in range(QB):
                        wiq_k = Pw.tile([128, INH * IHD], bf16, name="wiq", tag="wstream")
                        nc.sync.dma_start(wiq_k[:], WIQ[:].rearrange("(k p) n -> p k n", p=128)[:, k, :])
                        for j in range(4):
                            nc.tensor.matmul(iq_ps[:, j * 512:(j + 1) * 512], qcT[:, k, :],
                                             wiq_k[:, j * 512:(j + 1) * 512],
                                             start=(k == 0), stop=(k == QB - 1))
                    iqv = iq_ps[:].rearrange("p (h d) -> p h d", h=INH)
                    ipe = iqv[:, :, :DR].rearrange("p h (n two) -> p h n two", two=2)
                    ioe = iq_bf[:, :, :DR].rearrange("p h (n two) -> p h n two", two=2)
                    c3r = csr[:].rearrange("p (h n) -> p h n", h=NH)
                    s3r = snr[:].rearrange("p (h n) -> p h n", h=NH)
                    iq1 = Pa.tile([TPC, INH, 32], f32, name="iq1", tag="qt1")
                    iq2 = Pa.tile([TPC, INH, 32], f32, name="iq2", tag="qt2")
                    nc.vector.tensor_tensor(iq1[:], ipe[:, :, :, 0], c3r, op=ALU.mult)
                    nc.vector.tensor_tensor(iq2[:], ipe[:, :, :, 1], s3r, op=ALU.mult)
                    nc.vector.tensor_sub(ioe[:, :, :, 0], iq1[:], iq2[:])
                    nc.vector.tensor_tensor(iq1[:], ipe[:, :, :, 0], s3r, op=ALU.mult)
                    nc.vector.tensor_tensor(iq2[:], ipe[:, :, :, 1], c3r, op=ALU.mult)
                    nc.vector.tensor_add(ioe[:, :, :, 1], iq1[:], iq2[:])
                    nc.vector.tensor_copy(iq_bf[:, :, DR:], iqv[:, :, DR:])
                    nc.vector.tensor_reduce(qscale[:], iq_bf[:], AX.X, ALU.max,
                                            apply_absolute_value=True)
                # q_scale = exp2(ceil(log2(max(amax,1e-12)/448)))
                zz = Pa.tile([TPC, INH], f32)
                nc.vector.tensor_scalar(zz[:], qscale[:], 1e-12, 1.0 / FP8_MAX, op0=ALU.max, op1=ALU.mult)
                man = Pa.tile([TPC, INH], mybir.dt.uint32)
                nc.vector.tensor_scalar(man[:], zz[:].bitcast(mybir.dt.uint32), 0x007FFFFF, None, op0=ALU.bitwise_and)
                exb = Pa.tile([TPC, INH], mybir.dt.uint32)
                nc.vector.tensor_scalar(exb[:], zz[:].bitcast(mybir.dt.uint32), 0xFF800000, None, op0=ALU.bitwise_and)
                nc.vector.tensor_scalar(man[:], man[:], 0, None, op0=ALU.not_equal)
                nc.vector.tensor_scalar(man[:], man[:], 23, None, op0=ALU.logical_shift_left)
                nc.vector.tensor_tensor(exb[:], exb[:], man[:], op=ALU.add)
                nc.vector.tensor_scalar(qscale[:], exb[:].bitcast(f32), IDX_SCALE * (INH ** -0.5), None, op0=ALU.mult)

                iqT = Pe.tile([IHD, INH, TPC], bf16)
                with tc.tile_pool(name="ps_iqt", bufs=2, space="PSUM") as Pp:
                    for h in range(INH):
                        ti = Pp.tile([IHD, TPC], bf16, name="ti", tag="tp")
                        nc.tensor.transpose(ti[:], iq_bf[:, h, :], idb[:])
                        nc.scalar.copy(iqT[:, h, :], ti[:])

                # wts = (hn @ Wip + b) * qscale_scaled ; then diag(wts_h) mats
                wts = Pa.tile([TPC, INH], f32)
                with tc.tile_pool(name="ps_wp", bufs=1, space="PSUM") as Pp:
                    wip_sb = Pe.tile([128, KB, INH], bf16)
                    nc.sync.dma_start(wip_sb[:], WIP[:].rearrange("(k p) n -> p k n", p=128))
                    wp_ps = Pp.tile([TPC, INH], f32)
                    for k in range(KB):
                        nc.tensor.matmul(wp_ps[:], hnT[:, k, :], wip_sb[:, k, :],
                                         start=(k == 0), stop=(k == KB - 1))
                    nc.vector.tensor_add(wts[:], wp_ps[:], wpb_bc[:])
                    nc.vector.tensor_tensor(wts[:], wts[:], qscale[:], op=ALU.mult)
                dgw = Pe.tile([128, INH, 128], bf16)      # diag(wts_h) per head
                for h in range(INH):
                    nc.vector.tensor_scalar(dgw[:, h, :], idb[:], wts[:, h:h + 1], None, op0=ALU.mult)

                # causal additive mask as bf16 (injected into score PSUM via idb matmul)
                cadd_bf = Pe.tile([TPC, T], bf16)
                nc.vector.tensor_scalar(cadd_bf[:], caus[:], 1.0, -NEG, op0=ALU.subtract, op1=ALU.mult)

                # ---- gathered latent -> SBUF (global token order) ----
                kpeT_all = Pa.tile([DR, T], bf16)
                nc.scalar.dma_start(kpeT_all[:].rearrange("d (c t) -> d c t", c=NCORES),
                                    cc1_out[:, :DR, :].rearrange("c d t -> d c t"))
                iknT_all = Pe.tile([IHD, T], bf16)
                nc.scalar.dma_start(iknT_all[:].rearrange("d (c t) -> d c t", c=NCORES),
                                    cc1_out[:, DR:DR + IHD, :].rearrange("c d t -> d c t"))
                kvcT = Pa.tile([128, 4, T], bf16)
                for k in range(4):
                    nc.scalar.dma_start(
                        kvcT[:, k, :].rearrange("p (c t) -> p c t", c=NCORES),
                        cc1_out[:, DR + IHD + k * 128:DR + IHD + (k + 1) * 128, :]
                        .rearrange("c p t -> p c t"))

                # ---- indexer scores on PE: s_acc = mask + sum_h diag(wts_h) @ relu(s_h) ----
                s_acc = Pe.tile([TPC, T], f32)
                with tc.tile_pool(name="ps_s", bufs=1, space="PSUM") as Pp:
                    sa_ps = Pp.tile([TPC, T], f32, name="sa_ps")
                    for j in range(2):
                        nc.tensor.matmul(sa_ps[:, j * 512:(j + 1) * 512], idb[:],
                                         cadd_bf[:, j * 512:(j + 1) * 512],
                                         start=True, stop=False)
                    with tc.tile_pool(name="ps_sh", bufs=3, space="PSUM") as Pp2:
                        for h in range(INH):
                            s_ps = Pp2.tile([TPC, T], f32, name="s_ps", tag="sps")
                            for j in range(2):
                                nc.tensor.matmul(s_ps[:, j * 512:(j + 1) * 512], iqT[:, h, :],
                                                 iknT_all[:, j * 512:(j + 1) * 512],
                                                 start=True, stop=True)
                            rel_h = Pa.tile([TPC, T], bf16, name="rel_h", tag="relh", bufs=3)
                            nc.scalar.activation(rel_h[:], s_ps[:], AF.Relu)
                            for j in range(2):
                                nc.tensor.matmul(sa_ps[:, j * 512:(j + 1) * 512], dgw[:, h, :],
                                                 rel_h[:, j * 512:(j + 1) * 512],
                                                 start=False, stop=(h == INH - 1 and j == 1))
                    nc.scalar.copy(s_acc[:], sa_ps[:])

                # ---- topk threshold scan (DVE serial) ----
                scr = Pe.tile([TPC, T], f32, tag="scrt")
                nc.vector.tensor_copy(scr[:], s_acc[:])
                m8 = Pa.tile([TPC, 8], f32)
                for it in range(1 if SKIP_TOPK else TOPK // 8):
                    nc.vector.max(m8[:], scr[:])
                    nc.vector.match_replace(scr[:], m8[:], scr[:], -3e38)

                # ---- mask from scan threshold ----
                mask01 = Pe.tile([TPC, T], f32, tag="scrt")
                nc.vector.tensor_scalar(mask01[:], s_acc[:], m8[:, 7:8], None, op0=ALU.is_ge)
                nc.vector.tensor_tensor(mask01[:], mask01[:], caus[:], op=ALU.mult)
                madd_bf = Pa.tile([TPC, T], bf16)
                nc.vector.tensor_scalar(madd_bf[:], mask01[:], 1.0, -NEG, op0=ALU.subtract, op1=ALU.mult)


                # ==== work that overlaps the scan: q_b, V, K^T ====
                qtn = Pe.tile([TPC, NH, DN], bf16)    # q_nope * SCALE
                qtp = Pe.tile([TPC, NH, DR], bf16)    # roped q_pe * SCALE
                with tc.tile_pool(name="ps_q", bufs=1, space="PSUM") as Pp:
                    q_ps = Pp.tile([TPC, NH * DQ], f32)
                    for k in range(QB):
                        wqb_k = Pw.tile([128, NH * DQ], bf16, name="wqb", tag="wstream")
                        nc.sync.dma_start(wqb_k[:], WQB[:].rearrange("(k p) n -> p k n", p=128)[:, k, :])
                        for j in range(6):
                            nc.tensor.matmul(q_ps[:, j * 512:(j + 1) * 512], qcT[:, k, :],
                                             wqb_k[:, j * 512:(j + 1) * 512],
                                             start=(k == 0), stop=(k == QB - 1))
                    qv = q_ps[:].rearrange("p (h d) -> p h d", h=NH)
                    nc.vector.tensor_copy(qtn[:], qv[:, :, :DN])
                    pe3 = qv[:, :, DN:].rearrange("p h (n two) -> p h n two", two=2)
                    qo3 = qtp[:].rearrange("p h (n two) -> p h n two", two=2)
                    c3 = csr[:].rearrange("p (h n) -> p h n", h=NH)
                    s3 = snr[:].rearrange("p (h n) -> p h n", h=NH)
                    qt1 = Pa.tile([TPC, NH, 32], f32, name="qt1", tag="qt1")
                    qt2 = Pa.tile([TPC, NH, 32], f32, name="qt2", tag="qt2")
                    nc.vector.tensor_tensor(qt1[:], pe3[:, :, :, 0], c3, op=ALU.mult)
                    nc.vector.tensor_tensor(qt2[:], pe3[:, :, :, 1], s3, op=ALU.mult)
                    nc.vector.tensor_sub(qo3[:, :, :, 0], qt1[:], qt2[:])
                    nc.vector.tensor_tensor(qt1[:], pe3[:, :, :, 0], s3, op=ALU.mult)
                    nc.vector.tensor_tensor(qt2[:], pe3[:, :, :, 1], c3, op=ALU.mult)
                    nc.vector.tensor_add(qo3[:, :, :, 1], qt1[:], qt2[:])

                qtnT = Pa.tile([DN, NH, TPC], bf16)
                qtpT = Pa.tile([DR, NH, TPC], bf16)
                with tc.tile_pool(name="ps_qt", bufs=2, space="PSUM") as Pp:
                    for h in range(NH):
                        tq1 = Pp.tile([DN, TPC], bf16, name="tq1", tag="tp")
                        nc.tensor.transpose(tq1[:], qtn[:, h, :], idb[:])
                        nc.scalar.copy(qtnT[:, h, :], tq1[:])
                        tq2 = Pp.tile([DR, TPC], bf16, name="tq2", tag="tp")
                        nc.tensor.transpose(tq2[:], qtp[:, h, :], idb[:])
                        nc.scalar.copy(qtpT[:, h, :], tq2[:])

                Pe.release()
                # V for all tokens -> DRAM scratch
                v_dram = Pd.tile([NCORES, 128, NH * DV], bf16)
                wv_sb = Pa.tile([128, 4, NH * DV], bf16)
                nc.sync.dma_start(wv_sb[:], WV[:].rearrange("(k p) n -> p k n", p=128))
                with tc.tile_pool(name="ps_vall", bufs=2, space="PSUM") as Pp:
                    for tch in range(NCORES):
                        v_ps = Pp.tile([128, NH * DV], f32, name="v_ps", tag="vps")
                        for k in range(4):
                            for j in range(4):
                                nc.tensor.matmul(v_ps[:, j * 512:(j + 1) * 512],
                                                 kvcT[:, k, tch * 128:(tch + 1) * 128],
                                                 wv_sb[:, k, j * 512:(j + 1) * 512],
                                                 start=(k == 0), stop=(k == 3))
                        v_sb = Pa.tile([128, NH * DV], bf16, name="v_sb", tag="vsb", bufs=2)
                        nc.scalar.copy(v_sb[:], v_ps[:])
                        nc.sync.dma_start(v_dram[:][tch], v_sb[:])

                # K^T for all heads -> DRAM scratch
                kt_dram = Pd.tile([NH, DN, T], bf16)
                with tc.tile_pool(name="ps_ktb", bufs=2, space="PSUM") as Pp:
                    for h in range(NH):
                        wkn_h = Pw.tile([128, 4, DN], bf16, name="wkn_h", tag="wknh", bufs=3)
                        nc.sync.dma_start(
                            wkn_h[:],
                            WKN[:, h * DN:(h + 1) * DN].rearrange("(k p) n -> p k n", p=128))
                        kt_ps = Pp.tile([DN, T], f32, name="kt_ps", tag="ktp")
                        for j in range(2):
                            for k in range(4):
                                nc.tensor.matmul(kt_ps[:, j * 512:(j + 1) * 512],
                                                 wkn_h[:, k, :],
                                                 kvcT[:, k, j * 512:(j + 1) * 512],
                                                 start=(k == 0), stop=(k == 3))
                        kt_sb = Pa.tile([DN, T], bf16, name="kt_sb", tag="kth", bufs=3)
                        nc.scalar.copy(kt_sb[:], kt_ps[:])
                        nc.sync.dma_start(kt_dram[:][h], kt_sb[:])

                # ---- MLA attention ----
                oT = Pa.tile([DV, NH, TPC], bf16)
                with tc.tile_pool(name="ps_att", bufs=1, space="PSUM") as Pp:
                    for h in range(NH):
                        v_h = Pa.tile([128, NCORES, DV], bf16, name="v_h", tag="vh", bufs=3)
                        nc.sync.dma_start(v_h[:], v_dram[:].rearrange("c p d -> p c d")[:, :, h * DV:(h + 1) * DV])
                        kt_h = Pa.tile([DN, T], bf16, name="kt_h", tag="kth2", bufs=3)
                        nc.sync.dma_start(kt_h[:], kt_dram[:][h])
                        a_ps = Pp.tile([TPC, T], f32, name="a_ps", tag="sps", bufs=3)
                        for j in range(2):
                            nc.tensor.matmul(a_ps[:, j * 512:(j + 1) * 512], qtnT[:, h, :],
                                             kt_h[:, j * 512:(j + 1) * 512],
                                             start=True, stop=False)
                            nc.tensor.matmul(a_ps[:, j * 512:(j + 1) * 512], qtpT[:, h, :],
                                             kpeT_all[:, j * 512:(j + 1) * 512],
                                             start=False, stop=False)
                            nc.tensor.matmul(a_ps[:, j * 512:(j + 1) * 512], idb[:],
                                             madd_bf[:, j * 512:(j + 1) * 512],
                                             start=False, stop=True)
                        pex = Pa.tile([TPC, T], bf16, name="pex")
                        rs = Pa.tile([TPC, 1], f32, name="rs")
                        nc.scalar.activation(pex[:], a_ps[:], AF.Exp, accum_out=rs[:])
                        nc.vector.reciprocal(rs[:], rs[:])
                        pb = Pa.tile([TPC, T], bf16, name="pb")
                        nc.vector.tensor_scalar(pb[:], pex[:], rs[:], None, op0=ALU.mult)
                        # transpose P in 8 chunks; copy alternating DVE/Act; accumulate O^T
                        o_ps = Pp.tile([DV, TPC], f32, name="o_ps", tag="ops")
                        for s in range(8):
                            pt = Pp.tile([128, TPC], bf16, name="pt", tag="tp")
                            nc.tensor.transpose(pt[:], pb[:, s * 128:(s + 1) * 128], idb[:])
                            pts = Pa.tile([128, TPC], bf16, name="pts", tag="pts", bufs=4)
                            if s % 2 == 0:
                                nc.vector.tensor_copy(pts[:], pt[:])
                            else:
                                nc.scalar.copy(pts[:], pt[:])
                            nc.tensor.matmul(o_ps[:], v_h[:, s, :], pts[:],
                                             start=(s == 0), stop=(s == 7))
                        nc.vector.tensor_copy(oT[:, h, :], o_ps[:])

                # ---- o_proj + residual ----
                x_own = Pa.tile([TPC, H], f32)
                with tc.tile_pool(name="ps_op", bufs=1, space="PSUM") as Pp:
                    d_ps = Pp.tile([TPC, H], f32)
                    for h in range(NH):
                        wo_k = Pw.tile([128, H], bf16, name="wo_k", tag="wstream")
                        nc.sync.dma_start(wo_k[:], WO[:].rearrange("(k p) n -> p k n", p=128)[:, h, :])
                        for j in range(4):
                            nc.tensor.matmul(d_ps[:, j * 512:(j + 1) * 512], oT[:, h, :],
                                             wo_k[:, j * 512:(j + 1) * 512],
                                             start=(h == 0), stop=(h == NH - 1))
                    nc.vector.tensor_tensor(x_own[:], d_ps[:], xo[:], op=ALU.add)

                # ---- post-LN pieces: r2, gate logits, rw, h2T_own ----
                sq2 = Pa.tile([TPC, H], f32, name="sq2a", tag="sq2")
                ss2 = Pa.tile([TPC, 1], f32)
                nc.scalar.activation(sq2[:], x_own[:], AF.Square, accum_out=ss2[:])
                r2 = Pa.tile([TPC, 1], f32)
                nc.scalar.activation(r2[:], ss2[:], AF.Sqrt, bias=eps_b[:], scale=1.0 / H)
                nc.vector.reciprocal(r2[:], r2[:])
                xT_own = Pa.tile([128, KB, TPC], f32)
                with tc.tile_pool(name="ps_xt", bufs=2, space="PSUM") as Pp:
                    for k in range(KB):
                        tx = Pp.tile([128, TPC], f32, name="tx", tag="tpf")
                        nc.tensor.transpose(tx[:], x_own[:, k * 128:(k + 1) * 128], idf[:])
                        nc.scalar.copy(xT_own[:, k, :], tx[:])
                lg = Pa.tile([TPC, NE], f32)
                with tc.tile_pool(name="ps_g", bufs=1, space="PSUM") as Pp:
                    l_ps = Pp.tile([TPC, NE], f32)
                    for k in range(KB):
                        nc.tensor.matmul(l_ps[:], xT_own[:, k, :], wg_sb[:, k, :],
                                         start=(k == 0), stop=(k == KB - 1))
                    nc.scalar.activation(lg[:], l_ps[:], AF.Copy, scale=r2[:])
                gm8 = Pa.tile([TPC, 8], f32)
                nc.vector.max(gm8[:], lg[:])
                negm0 = Pa.tile([TPC, 1], f32)
                nc.vector.tensor_scalar(negm0[:], gm8[:, 0:1], -1.0, None, op0=ALU.mult)
                el = Pa.tile([TPC, NE], f32)
                nc.scalar.activation(el[:], lg[:], AF.Exp, bias=negm0[:])
                dn1 = Pa.tile([TPC, 1], f32)
                nc.vector.tensor_tensor(dn1[:], gm8[:, 1:2], gm8[:, 0:1], op=ALU.subtract)
                nc.scalar.activation(dn1[:], dn1[:], AF.Exp)
                nc.vector.tensor_scalar(dn1[:], dn1[:], 1.0, None, op0=ALU.add)
                nc.vector.reciprocal(dn1[:], dn1[:])
                sel = Pa.tile([TPC, NE], f32)
                nc.vector.tensor_scalar(sel[:], lg[:], gm8[:, 1:2], None, op0=ALU.is_ge)
                rw = Pa.tile([TPC, NE], f32)
                nc.vector.scalar_tensor_tensor(rw[:], el[:], dn1[:], sel[:],
                                               op0=ALU.mult, op1=ALU.mult)

                # h2T_own in [t', k] layout (feature-major transport)
                r2row = Pa.tile([1, TPC], f32)
                r2bc = Pa.tile([128, TPC], f32)
                with tc.tile_pool(name="ps_r2", bufs=1, space="PSUM") as Pp:
                    r2p = Pp.tile([1, TPC], f32)
                    nc.tensor.transpose(r2p[:], r2[:], idf[:])
                    nc.scalar.copy(r2row[:], r2p[:])
                nc.gpsimd.partition_broadcast(r2bc[:], r2row[:])
                h2T_own = Pa.tile([128, TPC, KB], bf16)
                for k in range(KB):
                    nc.vector.tensor_tensor(h2T_own[:, :, k], xT_own[:, k, :], r2bc[:], op=ALU.mult)

                # ---- CC2 in two half-token slabs (first carries rw) ----
                nc.scalar.dma_start(cch0_in[:, :HLF * KB],
                                    h2T_own[:, :HLF, :].rearrange("p t k -> p (t k)"))
                nc.scalar.dma_start(cch0_in[:, HLF * KB:], rw[:].bitcast(bf16))
                if not SKIP_CC:
                    nc.gpsimd.collective_compute("AllGather", ALU.bypass, replica_groups=RG,
                                                 ins=[cch0_in[:].opt()], outs=[cch0_out[:].opt()])
                nc.scalar.dma_start(cch1_in[:],
                                    h2T_own[:, HLF:, :].rearrange("p t k -> p (t k)"))
                if not SKIP_CC:
                    nc.gpsimd.collective_compute("AllGather", ALU.bypass, replica_groups=RG,
                                                 ins=[cch1_in[:].opt()], outs=[cch1_out[:].opt()])

                # ---- shared expert on own tokens (overlaps CC2) ----
                ss_own = Pa.tile([TPC, SI], bf16)
                with tc.tile_pool(name="ps_shx", bufs=1, space="PSUM") as Pp:
                    gs_ps = Pp.tile([TPC, SI], f32, name="gs_ps")
                    us_ps = Pp.tile([TPC, SI], f32, name="us_ps")
                    for k in range(KB):
                        wsg_k = Pw.tile([128, SI], bf16, name="wsg_k", tag="wstream")
                        nc.sync.dma_start(wsg_k[:], WSG[:].rearrange("(k p) n -> p k n", p=128)[:, k, :])
                        wsu_k = Pw.tile([128, SI], bf16, name="wsu_k", tag="wstream")
                        nc.sync.dma_start(wsu_k[:], WSU[:].rearrange("(k p) n -> p k n", p=128)[:, k, :])
                        for j in range(2):
                            nc.tensor.matmul(gs_ps[:, j * 512:(j + 1) * 512], h2T_own[:, :, k],
                                             wsg_k[:, j * 512:(j + 1) * 512],
                                             start=(k == 0), stop=(k == KB - 1))
                            nc.tensor.matmul(us_ps[:, j * 512:(j + 1) * 512], h2T_own[:, :, k],
                                             wsu_k[:, j * 512:(j + 1) * 512],
                                             start=(k == 0), stop=(k == KB - 1))
                    sgo = Pa.tile([TPC, SI], f32, name="sgo", tag="sq2")
                    nc.scalar.activation(sgo[:], gs_ps[:], AF.Silu)
                    nc.vector.tensor_tensor(ss_own[:], sgo[:], us_ps[:], op=ALU.mult)
                ssT = Pa.tile([128, 8, TPC], bf16)
                with tc.tile_pool(name="ps_st", bufs=2, space="PSUM") as Pp:
                    for m in range(8):
                        tss = Pp.tile([128, TPC], bf16, name="tss", tag="tp")
                        nc.tensor.transpose(tss[:], ss_own[:, m * 128:(m + 1) * 128], idb[:])
                        nc.vector.tensor_copy(ssT[:, m, :], tss[:])
                with tc.tile_pool(name="ps_sd", bufs=1, space="PSUM") as Pp:
                    sh_ps = Pp.tile([TPC, H], f32)
                    for m in range(8):
                        wsd_m = Pw.tile([128, H], bf16, name="wsd_m", tag="wstream")
                        nc.sync.dma_start(wsd_m[:], WSD[:].rearrange("(k p) n -> p k n", p=128)[:, m, :])
                        for j in range(4):
                            nc.tensor.matmul(sh_ps[:, j * 512:(j + 1) * 512], ssT[:, m, :],
                                             wsd_m[:, j * 512:(j + 1) * 512],
                                             start=(m == 0), stop=(m == 7))
                    outx = Pa.tile([TPC, H], f32, name="outx", tag="sq2")
                    nc.vector.tensor_tensor(outx[:], sh_ps[:], x_own[:], op=ALU.add)
                nc.scalar.dma_start(OUT_X[:], outx[:])

            # =================== MoE phase (expert-parallel, dense) ===================
            with tc.tile_pool(name="moe", bufs=1) as Pm:
                weg = Pm.tile([128, MI // 128, KB, 128], bf16)
                weu = Pm.tile([128, MI // 128, KB, 128], bf16)
                for m in range(MI // 128):
                    nc.sync.dma_start(weg[:, m, :, :].rearrange("p k n -> p (k n)"), WEG[:][m])
                    nc.sync.dma_start(weu[:, m, :, :].rearrange("p k n -> p (k n)"), WEU[:][m])
                wed = Pm.tile([128, MI // 128, H], bf16)
                for m in range(MI // 128):
                    nc.sync.dma_start(wed[:, m, :], WED[:][m])
                # gathered h2T halves [p, c, t'(64), k]
                h2h0 = Pm.tile([128, NCORES, HLF, KB], bf16)
                nc.scalar.dma_start(
                    h2h0[:].rearrange("p c t k -> p c (t k)"),
                    cch0_out[:, :, :HLF * KB].rearrange("c p n -> p c n"))
                h2h1 = Pm.tile([128, NCORES, HLF, KB], bf16)
                nc.scalar.dma_start(
                    h2h1[:].rearrange("p c t k -> p c (t k)"),
                    cch1_out[:].rearrange("c p n -> p c n"))
                # rw for all tokens: [p=token-in-chunk, c, 8] f32 (bitcast pairs);
                # select own-expert column via one-hot dot on DVE
                rw_sb = Pm.tile([128, NCORES, 2 * NE], bf16)
                nc.scalar.dma_start(rw_sb[:],
                                    cch0_out[:, :, HLF * KB:].rearrange("c p n -> p c n"))
                rwe = Pm.tile([128, NCORES], f32)
                rwt = Pm.tile([128, NE], f32, name="rwt")
                for tch in range(NCORES):
                    nc.vector.tensor_tensor(rwt[:], rw_sb[:, tch, :].bitcast(f32), oh_bc[:], op=ALU.mult)
                    nc.vector.tensor_reduce(rwe[:, tch:tch + 1], rwt[:], AX.X, ALU.add)

                su = Pm.tile([128, MI // 128, T], bf16)   # silu(g)*u  [mi, (c t')]
                suv = su[:].rearrange("p m (c t) -> p m c t", c=NCORES)
                with tc.tile_pool(name="ps_moe", bufs=2, space="PSUM") as Pp:
                    for half, h2h in ((0, h2h0), (1, h2h1)):
                        for m in range(MI // 128):
                            g_ps = Pp.tile([128, 512], f32, name="g_ps", tag="gps")
                            u_ps = Pp.tile([128, 512], f32, name="u_ps", tag="ups")
                            gv = g_ps[:].rearrange("p (c t) -> p c t", c=NCORES)
                            uv = u_ps[:].rearrange("p (c t) -> p c t", c=NCORES)
                            for k in range(KB):
                                nc.tensor.matmul(g_ps[:], weg[:, m, k, :],
                                                 h2h[:, :, :, k].rearrange("p c t -> p (c t)"),
                                                 start=(k == 0), stop=(k == KB - 1))
                                nc.tensor.matmul(u_ps[:], weu[:, m, k, :],
                                                 h2h[:, :, :, k].rearrange("p c t -> p (c t)"),
                                                 start=(k == 0), stop=(k == KB - 1))
                            sg = Pm.tile([128, 512], f32, name="sg", tag="sgs", bufs=2)
                            nc.scalar.activation(sg[:], g_ps[:], AF.Silu)
                            nc.vector.tensor_tensor(sg[:], sg[:], u_ps[:], op=ALU.mult)
                            nc.vector.tensor_copy(
                                suv[:, m, :, half * HLF:(half + 1) * HLF],
                                sg[:].rearrange("p (c t) -> p c t", c=NCORES))

                with tc.tile_pool(name="ps_dn", bufs=2, space="PSUM") as Pp:
                    for tch in range(8):
                        dn_ps = Pp.tile([128, H], f32, name="dn_ps", tag="dnp")
                        for m in range(8):
                            for j in range(4):
                                nc.tensor.matmul(dn_ps[:, j * 512:(j + 1) * 512],
                                                 su[:, m, tch * 128:(tch + 1) * 128],
                                                 wed[:, m, j * 512:(j + 1) * 512],
                                                 start=(m == 0), stop=(m == 7))
                        ob = Pm.tile([128, H], bf16, name="ob", tag="obs")
                        # scale rows by rw[token, own_expert] (per-partition ptr)
                        nc.scalar.activation(ob[:], dn_ps[:], AF.Copy,
                                             scale=rwe[:, tch:tch + 1])
                        nc.sync.dma_start(OUT_P[:].rearrange("(c p) n -> c p n", p=128)[tch], ob[:])

    nc.compile()
    return nc


_NC = None


def kernel(**inputs):
    global _NC
    inp = {k: np.asarray(v) for k, v in inputs.items()}
    pos = inp["positions"].astype(np.int64)
    x = inp["hidden_states"].astype(np.float32)

    # ---- fold layernorm weights into downstream mats (host prep) ----
    iw = inp["input_ln_w"].astype(np.float32)
    qw = inp["q_a_ln_w"].astype(np.float32)
    kw = inp["kv_a_ln_w"].astype(np.float32)
    pw = inp["post_ln_w"].astype(np.float32)
    Wa = (iw[:, None] * inp["W_qkv_a"]).astype(BF)
    Wik = (iw[:, None] * inp["idx_wk"]).astype(BF)
    Wip = (iw[:, None] * inp["idx_wp_w"]).astype(BF)
    Wqb = (SCALE * qw[:, None] * inp["W_q_b"]).astype(BF)
    Wiq = (qw[:, None] * inp["idx_wq_b"]).astype(BF)
    Wkvb = (kw[:, None] * inp["W_kv_b"]).astype(np.float32).reshape(KL, NH, DN + DV)
    Wkn = np.ascontiguousarray(Wkvb[:, :, :DN].reshape(KL, NH * DN)).astype(BF)
    Wv = np.ascontiguousarray(Wkvb[:, :, DN:].reshape(KL, NH * DV)).astype(BF)
    Wo = inp["W_o"].astype(BF)
    Wg = (pw[:, None] * inp["W_gate"]).astype(np.float32)
    Weg = (pw[None, :, None] * inp["We_gate"]).astype(BF)
    Weu = (pw[None, :, None] * inp["We_up"]).astype(BF)
    Wed = inp["We_down"].astype(BF)
    Wsg = (pw[:, None] * inp["Ws_gate"]).astype(BF)
    Wsu = (pw[:, None] * inp["Ws_up"]).astype(BF)
    Wsd = inp["Ws_down"].astype(BF)

    # relayout expert weights: [H, MI] -> [m][p][k*128+mi'] with H=(k,p)
    def relay_up(W):   # [H, MI] -> [8, 128, 16*128]
        Wr = W.reshape(KB, 128, MI // 128, 128)          # k p m mi'
        return np.ascontiguousarray(Wr.transpose(2, 1, 0, 3).reshape(MI // 128, 128, KB * 128))

    def relay_dn(W):   # [MI, H] -> [8, 128, H]
        return np.ascontiguousarray(W.reshape(MI // 128, 128, H))

    inv = 1.0 / (BASE ** (np.arange(0, DR, 2, dtype=np.float32) / DR))
    ang = pos.astype(np.float32)[:, None] * inv           # [T, 32]
    cs_a, sn_a = np.cos(ang), np.sin(ang)

    in_maps = []
    for c in range(NCORES):
        rows = list(range(c * TPC, (c + 1) * TPC))
        posn = pos[rows]
        causm = (posn[:, None] >= pos[None, :]).astype(np.float32)
        cs = cs_a[rows]; sn = sn_a[rows]
        oh = np.zeros((1, NE), np.float32); oh[0, c] = 1.0
        in_maps.append({
            "OH": oh,
            "XO": np.ascontiguousarray(x[rows]),
            "CAUS": np.ascontiguousarray(causm),
            "CSR": np.ascontiguousarray(np.tile(cs, (1, NH)).astype(np.float32)),
            "SNR": np.ascontiguousarray(np.tile(sn, (1, NH)).astype(np.float32)),
            "KNW": inp["idx_kn_w"].astype(np.float32).reshape(1, IHD),
            "KNB": inp["idx_kn_b"].astype(np.float32).reshape(1, IHD),
            "WPB": inp["idx_wp_b"].astype(np.float32).reshape(1, INH),
            "WA": Wa, "WQB": Wqb, "WIQ": Wiq, "WIK": Wik, "WIP": Wip,
            "WKN": Wkn, "WV": Wv, "WO": Wo, "WG": Wg,
            "WEG": relay_up(Weg[c]),
            "WEU": relay_up(Weu[c]),
            "WED": relay_dn(Wed[c]),
            "WSG": Wsg, "WSU": Wsu, "WSD": Wsd,
        })

    if _NC is None:
        _NC = build()
    try:
        res = run_bass_kernel_spmd(_NC, in_maps, core_ids=list(range(NCORES)))
    except Exception:
        import time as _time
        _time.sleep(2.0)
        res = run_bass_kernel_spmd(_NC, in_maps, core_ids=list(range(NCORES)))

    out = np.zeros((T, H), np.float64)
    for c in range(NCORES):
        out += res.results[c]["OUT_P"].astype(np.float64)
    for c in range(NCORES):
        out[c * TPC:(c + 1) * TPC] += res.results[c]["OUT_X"].astype(np.float64)
    return out.astype(np.float32)


# revision 19
# speedup vs baseline: 1.0715x; 1.0296x over previous
"""Self-contained Trainium2 Bass kernel for the DeepseekV2 decoder layer problem.

Sharding (8 cores): core c owns the contiguous 128-token block [128c, 128c+128).
KV-side projections are computed per-own-token and AllGathered as one bundle
(kpe^T / ik^T / kv_latent^T).  Indexer scores + top-k + MLA attention + o_proj
run on own rows.  h2 is transported feature-major (h2^T) in two half-token
AllGathers (second half carries the router weights); MoE is expert-parallel
(1 routed expert per core, dense over all tokens) plus the shared expert on
own tokens.  Host sums the per-core partials.
"""
import sys
sys.path.insert(0, "/opt/trn_rl_repo")
import numpy as np
import ml_dtypes

import concourse.bass as bass
import concourse.mybir as mybir
from concourse import bacc, tile
from concourse.bass_utils import run_bass_kernel_spmd
from concourse.masks import make_identity

f32 = mybir.dt.float32
bf16 = mybir.dt.bfloat16
AF = mybir.ActivationFunctionType
ALU = mybir.AluOpType
AX = mybir.AxisListType
BF = ml_dtypes.bfloat16

# dims
T = 1024; H = 2048; NH = 16; DN = 128; DR = 64; DQ = DN + DR; DV = 128
QL = 1536; KL = 512
INH = 16; IHD = 128; TOPK = 256
NE = 8; MI = 1024; SI = 1024
BASE = 10000.0; EPS = 1e-6
SCALE = DQ ** -0.5
IDX_SCALE = IHD ** -0.5
FP8_MAX = 448.0
NCORES = 8
TPC = T // NCORES        # 128 tokens per core
NEG = -1e30
import os
SKIP_CC = os.environ.get("SKIP_CC") == "1"
SKIP_TOPK = os.environ.get("SKIP_TOPK") == "1"

KB = 16   # H/128 k-chunks
QB = 12   # QL/128
RG = [list(range(NCORES))]
CCL = DR + IHD + KL          # merged latent collective rows (704)
HLF = TPC // 2               # 64 tokens per h2 half


def build():
    nc = bacc.Bacc("TRN2", target_bir_lowering=False,
                   debug=os.environ.get("BASS_DEBUG") == "1",
                   enable_asserts=False, num_devices=NCORES)

    def din(name, shape, dt=bf16):
        return nc.dram_tensor(name, shape, dt, kind="ExternalInput").ap()

    # ---- per-core inputs ----
    XO = din("XO", [TPC, H], f32)              # x_in own rows
    CAUS = din("CAUS", [TPC, T], f32)          # causal01 over global keys
    CSR = din("CSR", [TPC, 512], f32)          # cos tiled 16x (unscaled)
    SNR = din("SNR", [TPC, 512], f32)
    OH = din("OH", [1, NE], f32)               # own-expert one-hot
    KNW = din("KNW", [1, IHD], f32)            # idx_kn_w
    KNB = din("KNB", [1, IHD], f32)
    WPB = din("WPB", [1, INH], f32)            # idx_wp_b
    WA = din("WA", [H, QL + KL + DR])          # bf16, ln-folded
    WQB = din("WQB", [QL, NH * DQ])
    WIQ = din("WIQ", [QL, INH * IHD])
    WIK = din("WIK", [H, IHD])
    WIP = din("WIP", [H, INH])
    WKN = din("WKN", [KL, NH * DN])
    WV = din("WV", [KL, NH * DV])
    WO = din("WO", [NH * DV, H])
    WG = din("WG", [H, NE], f32)
    WEG = din("WEG", [MI // 128, 128, KB * 128])   # [m][p][k*128+mi'] host-relaid
    WEU = din("WEU", [MI // 128, 128, KB * 128])
    WED = din("WED", [MI // 128, 128, H])          # [m][p=mi-in-chunk][H]
    WSG = din("WSG", [H, SI])
    WSU = din("WSU", [H, SI])
    WSD = din("WSD", [SI, H])

    OUT_P = nc.dram_tensor("OUT_P", [T, H], bf16, kind="ExternalOutput").ap()
    OUT_X = nc.dram_tensor("OUT_X", [TPC, H], f32, kind="ExternalOutput").ap()

    with tile.TileContext(nc) as tc:
        with tc.tile_pool(name="const", bufs=1) as Pc, \
             tc.tile_pool(name="dram", bufs=1, space="DRAM") as Pd:
            idf = Pc.tile([128, 128], f32)
            make_identity(nc, idf[:])
            idb = Pc.tile([128, 128], bf16)
            nc.vector.tensor_copy(idb[:], idf[:])
            eps_b = Pc.tile([128, 1], f32)
            nc.vector.memset(eps_b[:], EPS)

            xo = Pc.tile([TPC, H], f32)
            nc.sync.dma_start(xo[:], XO[:])
            caus = Pc.tile([TPC, T], f32)
            nc.sync.dma_start(caus[:], CAUS[:])
            csr = Pc.tile([TPC, 512], f32); nc.sync.dma_start(csr[:], CSR[:])
            snr = Pc.tile([TPC, 512], f32); nc.sync.dma_start(snr[:], SNR[:])
            knw_r = Pc.tile([1, IHD], f32); nc.sync.dma_start(knw_r[:], KNW[:])
            knb_r = Pc.tile([1, IHD], f32); nc.sync.dma_start(knb_r[:], KNB[:])
            wpb_r = Pc.tile([1, INH], f32); nc.sync.dma_start(wpb_r[:], WPB[:])
            knw_bc = Pc.tile([128, IHD], f32)
            nc.gpsimd.partition_broadcast(knw_bc[:], knw_r[:])
            knb_bc = Pc.tile([128, IHD], f32)
            nc.gpsimd.partition_broadcast(knb_bc[:], knb_r[:])
            wpb_bc = Pc.tile([128, INH], f32)
            nc.gpsimd.partition_broadcast(wpb_bc[:], wpb_r[:])
            wg_sb = Pc.tile([128, KB, NE], f32)
            nc.sync.dma_start(wg_sb[:], WG[:].rearrange("(k p) n -> p k n", p=128))
            oh_r = Pc.tile([1, NE], f32); nc.sync.dma_start(oh_r[:], OH[:])
            oh_bc = Pc.tile([128, NE], f32)
            nc.gpsimd.partition_broadcast(oh_bc[:], oh_r[:])

            # collective buffers
            cc1_in = Pd.tile([CCL, TPC], bf16)
            cc1_out = Pd.tile([NCORES, CCL, TPC], bf16, addr_space="Shared")
            HRW = HLF * KB + 2 * NE        # half-token h2T cols + rw bf16 pairs
            cch0_in = Pd.tile([128, HRW], bf16)
            cch0_out = Pd.tile([NCORES, 128, HRW], bf16, addr_space="Shared")
            cch1_in = Pd.tile([128, HLF * KB], bf16)
            cch1_out = Pd.tile([NCORES, 128, HLF * KB], bf16, addr_space="Shared")

            with tc.tile_pool(name="att", bufs=1) as Pa, \
                 tc.tile_pool(name="wstream", bufs=2) as Pw:
                Pe = tc.alloc_tile_pool(name="early", bufs=1)
                # rmsnorm scale r1 for own rows
                sq = Pa.tile([TPC, H], f32, name="sq_scratch", tag="sq2")
                ssq = Pa.tile([TPC, 1], f32)
                nc.scalar.activation(sq[:], xo[:], AF.Square, accum_out=ssq[:])
                r1 = Pa.tile([TPC, 1], f32)
                nc.scalar.activation(r1[:], ssq[:], AF.Sqrt, bias=eps_b[:], scale=1.0 / H)
                nc.vector.reciprocal(r1[:], r1[:])
                hn_own = Pe.tile([TPC, H], bf16)
                nc.vector.tensor_scalar(hn_own[:], xo[:], r1[:], None, op0=ALU.mult)
                hnT = Pe.tile([128, KB, TPC], bf16)
                with tc.tile_pool(name="ps_tr", bufs=2, space="PSUM") as Pp:
                    for k in range(KB):
                        tp = Pp.tile([128, 128], bf16, name="tp")
                        nc.tensor.transpose(tp[:], hn_own[:, k * 128:(k + 1) * 128], idb[:])
                        nc.scalar.copy(hnT[:, k, :], tp[:])

                # ---- qkv_a: kv+kpe columns FIRST so CC1 can launch early ----
                with tc.tile_pool(name="ps_qkv", bufs=1, space="PSUM") as Pp:
                    kvp_ps = Pp.tile([TPC, KL + DR], f32)
                    for k in range(KB):
                        wakv_k = Pw.tile([128, KL + DR], bf16, name="wakv", tag="wknh", bufs=3)
                        with tc.high_priority():
                            nc.sync.dma_start(wakv_k[:], WA[:].rearrange("(k p) n -> p k n", p=128)[:, k, QL:])
                        nc.tensor.matmul(kvp_ps[:, 0:512], hnT[:, k, :], wakv_k[:, 0:512],
                                         start=(k == 0), stop=(k == KB - 1))
                        nc.tensor.matmul(kvp_ps[:, 512:], hnT[:, k, :], wakv_k[:, 512:],
                                         start=(k == 0), stop=(k == KB - 1))
                    # kv_c rmsnorm -> bf16
                    ksq = Pa.tile([TPC, KL], f32, name="ksq", tag="sq2")
                    kss = Pa.tile([TPC, 1], f32)
                    nc.scalar.activation(ksq[:], kvp_ps[:, :KL], AF.Square, accum_out=kss[:])
                    rkv = Pa.tile([TPC, 1], f32)
                    nc.scalar.activation(rkv[:], kss[:], AF.Sqrt, bias=eps_b[:], scale=1.0 / KL)
                    nc.vector.reciprocal(rkv[:], rkv[:])
                    kvn = Pa.tile([TPC, KL], bf16)
                    nc.vector.tensor_scalar(kvn[:], kvp_ps[:, :KL], rkv[:], None, op0=ALU.mult)

                    # k_pe rope (unscaled tables) -> bf16 [TPC, 64]
                    kpe = Pa.tile([TPC, DR], bf16)
                    t1 = Pa.tile([TPC, 32], f32, name="rt1", tag="rt1")
                    t2 = Pa.tile([TPC, 32], f32, name="rt2", tag="rt2")
                    pe_src = kvp_ps[:, KL:].rearrange("p (n two) -> p n two", two=2)
                    x1, x2 = pe_src[:, :, 0], pe_src[:, :, 1]
                    ko = kpe[:].rearrange("p (n two) -> p n two", two=2)
                    nc.vector.tensor_tensor(t1[:], x1, csr[:, :32], op=ALU.mult)
                    nc.vector.tensor_tensor(t2[:], x2, snr[:, :32], op=ALU.mult)
                    nc.vector.tensor_sub(ko[:, :, 0], t1[:], t2[:])
                    nc.vector.tensor_tensor(t1[:], x1, snr[:, :32], op=ALU.mult)
                    nc.vector.tensor_tensor(t2[:], x2, csr[:, :32], op=ALU.mult)
                    nc.vector.tensor_add(ko[:, :, 1], t1[:], t2[:])

                # ---- ik own: layernorm(hn @ Wik) + rope ----
                ikn = Pa.tile([TPC, IHD], bf16)
                with tc.tile_pool(name="ps_ik", bufs=1, space="PSUM") as Pp:
                    wik_sb = Pe.tile([128, KB, IHD], bf16)
                    with tc.high_priority():
                        nc.sync.dma_start(wik_sb[:], WIK[:].rearrange("(k p) n -> p k n", p=128))
                    ik_ps = Pp.tile([TPC, IHD], f32)
                    for k in range(KB):
                        nc.tensor.matmul(ik_ps[:], hnT[:, k, :], wik_sb[:, k, :],
                                         start=(k == 0), stop=(k == KB - 1))
                    negm = Pa.tile([TPC, 1], f32)
                    nc.vector.tensor_reduce(negm[:], ik_ps[:], AX.X, ALU.add, negate=True)
                    nc.vector.tensor_scalar(negm[:], negm[:], 1.0 / IHD, None, op0=ALU.mult)
                    xm = Pa.tile([TPC, IHD], f32)
                    nc.vector.tensor_scalar(xm[:], ik_ps[:], negm[:], None, op0=ALU.add)
                    xms = Pa.tile([TPC, IHD], f32)
                    vss = Pa.tile([TPC, 1], f32)
                    nc.scalar.activation(xms[:], xm[:], AF.Square, accum_out=vss[:])
                    rstd = Pa.tile([TPC, 1], f32)
                    nc.scalar.activation(rstd[:], vss[:], AF.Sqrt, bias=eps_b[:], scale=1.0 / IHD)
                    nc.vector.reciprocal(rstd[:], rstd[:])
                    ikf = Pa.tile([TPC, IHD], f32)
                    nc.vector.scalar_tensor_tensor(ikf[:], xm[:], rstd[:], knw_bc[:],
                                                   op0=ALU.mult, op1=ALU.mult)
                    nc.vector.tensor_add(ikf[:], ikf[:], knb_bc[:])
                    pe2 = ikf[:, :DR].rearrange("p (n two) -> p n two", two=2)
                    iko2 = ikn[:, :DR].rearrange("p (n two) -> p n two", two=2)
                    it1 = Pa.tile([TPC, 32], f32, name="it1", tag="rt1")
                    it2 = Pa.tile([TPC, 32], f32, name="it2", tag="rt2")
                    nc.vector.tensor_tensor(it1[:], pe2[:, :, 0], csr[:, :32], op=ALU.mult)
                    nc.vector.tensor_tensor(it2[:], pe2[:, :, 1], snr[:, :32], op=ALU.mult)
                    nc.vector.tensor_sub(iko2[:, :, 0], it1[:], it2[:])
                    nc.vector.tensor_tensor(it1[:], pe2[:, :, 0], snr[:, :32], op=ALU.mult)
                    nc.vector.tensor_tensor(it2[:], pe2[:, :, 1], csr[:, :32], op=ALU.mult)
                    nc.vector.tensor_add(iko2[:, :, 1], it1[:], it2[:])
                    nc.vector.tensor_copy(ikn[:, DR:], ikf[:, DR:])

                # transposes of kpe, ikn, kvn -> merged CC1 input
                with tc.tile_pool(name="ps_tr2", bufs=2, space="PSUM") as Pp:
                    kpeT_o = Pa.tile([DR, TPC], bf16)
                    tpp = Pp.tile([DR, 128], bf16, name="tpp", tag="tp")
                    nc.tensor.transpose(tpp[:], kpe[:], idb[:])
                    nc.scalar.copy(kpeT_o[:], tpp[:])
                    nc.sync.dma_start(cc1_in[:DR, :], kpeT_o[:])
                    iknT_o = Pa.tile([IHD, TPC], bf16)
                    tpi = Pp.tile([IHD, TPC], bf16, name="tpi", tag="tp")
                    nc.tensor.transpose(tpi[:], ikn[:], idb[:])
                    nc.scalar.copy(iknT_o[:], tpi[:])
                    nc.sync.dma_start(cc1_in[DR:DR + IHD, :], iknT_o[:])
                    kvT_o = Pa.tile([128, 4, TPC], bf16)
                    for k in range(4):
                        tpk = Pp.tile([128, 128], bf16, name="tpk", tag="tp")
                        nc.tensor.transpose(tpk[:], kvn[:, k * 128:(k + 1) * 128], idb[:])
                        nc.scalar.copy(kvT_o[:, k, :], tpk[:])
                    nc.sync.dma_start(
                        cc1_in[DR + IHD:, :].rearrange("(k p) t -> p k t", p=128), kvT_o[:])
                if not SKIP_CC:
                    nc.gpsimd.collective_compute("AllGather", ALU.bypass, replica_groups=RG,
                                                 ins=[cc1_in[:].opt()], outs=[cc1_out[:].opt()])

                hp_ctx = tc.high_priority(offset=8000); hp_ctx.__enter__()
                # ---- q-part of qkv_a (overlaps CC1) ----
                with tc.tile_pool(name="ps_qp", bufs=1, space="PSUM") as Pp:
                    qc_ps2 = Pp.tile([TPC, QL], f32)
                    for k in range(KB):
                        waq_k = Pw.tile([128, QL], bf16, name="waq", tag="wstream")
                        nc.sync.dma_start(waq_k[:], WA[:].rearrange("(k p) n -> p k n", p=128)[:, k, :QL])
                        for j in range(3):
                            nc.tensor.matmul(qc_ps2[:, j * 512:(j + 1) * 512],
                                             hnT[:, k, :], waq_k[:, j * 512:(j + 1) * 512],
                                             start=(k == 0), stop=(k == KB - 1))
                    qsq = Pa.tile([TPC, QL], f32, name="qsq", tag="sq2")
                    qss = Pa.tile([TPC, 1], f32)
                    nc.scalar.activation(qsq[:], qc_ps2[:], AF.Square, accum_out=qss[:])
                    rq = Pa.tile([TPC, 1], f32)
                    nc.scalar.activation(rq[:], qss[:], AF.Sqrt, bias=eps_b[:], scale=1.0 / QL)
                    nc.vector.reciprocal(rq[:], rq[:])
                    qcn = Pe.tile([TPC, QL], bf16)
                    nc.vector.tensor_scalar(qcn[:], qc_ps2[:], rq[:], None, op0=ALU.mult)
                qcT = Pe.tile([128, QB, TPC], bf16)
                with tc.tile_pool(name="ps_qct", bufs=2, space="PSUM") as Pp:
                    for k in range(QB):
                        tpq = Pp.tile([128, 128], bf16, name="tpq", tag="tp")
                        nc.tensor.transpose(tpq[:], qcn[:, k * 128:(k + 1) * 128], idb[:])
                        nc.scalar.copy(qcT[:, k, :], tpq[:])

                # ---- iq (indexer q) FIRST: it gates the topk long pole ----
                iq_bf = Pe.tile([TPC, INH, IHD], bf16)
                qscale = Pa.tile([TPC, INH], f32)
                with tc.tile_pool(name="ps_iq", bufs=1, space="PSUM") as Pp:
                    iq_ps = Pp.tile([TPC, INH * IHD], f32)
                    for k in range(QB):
                        wiq_k = Pw.tile([128, INH * IHD], bf16, name="wiq", tag="wstream")
                        nc.sync.dma_start(wiq_k[:], WIQ[:].rearrange("(k p) n -> p k n", p=128)[:, k, :])
                        for j in range(4):
                            nc.tensor.matmul(iq_ps[:, j * 512:(j + 1) * 512], qcT[:, k, :],
                                             wiq_k[:, j * 512:(j + 1) * 512],
                                             start=(k == 0), stop=(k == QB - 1))
                    iqv = iq_ps[:].rearrange("p (h d) -> p h d", h=INH)
                    ipe = iqv[:, :, :DR].rearrange("p h (n two) -> p h n two", two=2)
                    ioe = iq_bf[:, :, :DR].rearrange("p h (n two) -> p h n two", two=2)
                    c3r = csr[:].rearrange("p (h n) -> p h n", h=NH)
                    s3r = snr[:].rearrange("p (h n) -> p h n", h=NH)
                    iq1 = Pa.tile([TPC, INH, 32], f32, name="iq1", tag="qt1")
                    iq2 = Pa.tile([TPC, INH, 32], f32, name="iq2", tag="qt2")
                    nc.vector.tensor_tensor(iq1[:], ipe[:, :, :, 0], c3r, op=ALU.mult)
                    nc.vector.tensor_tensor(iq2[:], ipe[:, :, :, 1], s3r, op=ALU.mult)
                    nc.vector.tensor_sub(ioe[:, :, :, 0], iq1[:], iq2[:])
                    nc.vector.tensor_tensor(iq1[:], ipe[:, :, :, 0], s3r, op=ALU.mult)
                    nc.vector.tensor_tensor(iq2[:], ipe[:, :, :, 1], c3r, op=ALU.mult)
                    nc.vector.tensor_add(ioe[:, :, :, 1], iq1[:], iq2[:])
                    nc.vector.tensor_copy(iq_bf[:, :, DR:], iqv[:, :, DR:])
                    nc.vector.tensor_reduce(qscale[:], iq_bf[:], AX.X, ALU.max,
                                            apply_absolute_value=True)
                # q_scale = exp2(ceil(log2(max(amax,1e-12)/448)))
                zz = Pa.tile([TPC, INH], f32)
                nc.vector.tensor_scalar(zz[:], qscale[:], 1e-12, 1.0 / FP8_MAX, op0=ALU.max, op1=ALU.mult)
                man = Pa.tile([TPC, INH], mybir.dt.uint32)
                nc.vector.tensor_scalar(man[:], zz[:].bitcast(mybir.dt.uint32), 0x007FFFFF, None, op0=ALU.bitwise_and)
                exb = Pa.tile([TPC, INH], mybir.dt.uint32)
                nc.vector.tensor_scalar(exb[:], zz[:].bitcast(mybir.dt.uint32), 0xFF800000, None, op0=ALU.bitwise_and)
                nc.vector.tensor_scalar(man[:], man[:], 0, None, op0=ALU.not_equal)
                nc.vector.tensor_scalar(man[:], man[:], 23, None, op0=ALU.logical_shift_left)
                nc.vector.tensor_tensor(exb[:], exb[:], man[:], op=ALU.add)
                nc.vector.tensor_scalar(qscale[:], exb[:].bitcast(f32), IDX_SCALE * (INH ** -0.5), None, op0=ALU.mult)

                iqT = Pe.tile([IHD, INH, TPC], bf16)
                with tc.tile_pool(name="ps_iqt", bufs=2, space="PSUM") as Pp:
                    for h in range(INH):
                        ti = Pp.tile([IHD, TPC], bf16, name="ti", tag="tp")
                        nc.tensor.transpose(ti[:], iq_bf[:, h, :], idb[:])
                        nc.scalar.copy(iqT[:, h, :], ti[:])

                # wts = (hn @ Wip + b) * qscale_scaled ; then diag(wts_h) mats
                wts = Pa.tile([TPC, INH], f32)
                with tc.tile_pool(name="ps_wp", bufs=1, space="PSUM") as Pp:
                    wip_sb = Pe.tile([128, KB, INH], bf16)
                    nc.sync.dma_start(wip_sb[:], WIP[:].rearrange("(k p) n -> p k n", p=128))
                    wp_ps = Pp.tile([TPC, INH], f32)
                    for k in range(KB):
                        nc.tensor.matmul(wp_ps[:], hnT[:, k, :], wip_sb[:, k, :],
                                         start=(k == 0), stop=(k == KB - 1))
                    nc.vector.tensor_add(wts[:], wp_ps[:], wpb_bc[:])
                    nc.vector.tensor_tensor(wts[:], wts[:], qscale[:], op=ALU.mult)
                dgw = Pe.tile([128, INH, 128], bf16)      # diag(wts_h) per head
                for h in range(INH):
                    nc.vector.tensor_scalar(dgw[:, h, :], idb[:], wts[:, h:h + 1], None, op0=ALU.mult)

                # causal additive mask as bf16 (injected into score PSUM via idb matmul)
                cadd_bf = Pe.tile([TPC, T], bf16)
                nc.vector.tensor_scalar(cadd_bf[:], caus[:], 1.0, -NEG, op0=ALU.subtract, op1=ALU.mult)

                # ---- gathered latent -> SBUF (global token order) ----
                kpeT_all = Pa.tile([DR, T], bf16)
                nc.gpsimd.dma_start(kpeT_all[:].rearrange("d (c t) -> d c t", c=NCORES),
                                    cc1_out[:, :DR, :].rearrange("c d t -> d c t"))
                iknT_all = Pe.tile([IHD, T], bf16)
                nc.gpsimd.dma_start(iknT_all[:].rearrange("d (c t) -> d c t", c=NCORES),
                                    cc1_out[:, DR:DR + IHD, :].rearrange("c d t -> d c t"))
                kvcT = Pa.tile([128, 4, T], bf16)
                for k in range(4):
                    nc.gpsimd.dma_start(
                        kvcT[:, k, :].rearrange("p (c t) -> p c t", c=NCORES),
                        cc1_out[:, DR + IHD + k * 128:DR + IHD + (k + 1) * 128, :]
                        .rearrange("c p t -> p c t"))

                # ---- indexer scores on PE: s_acc = mask + sum_h diag(wts_h) @ relu(s_h) ----
                s_acc = Pe.tile([TPC, T], f32)
                with tc.tile_pool(name="ps_s", bufs=1, space="PSUM") as Pp:
                    sa_ps = Pp.tile([TPC, T], f32, name="sa_ps")
                    for j in range(2):
                        nc.tensor.matmul(sa_ps[:, j * 512:(j + 1) * 512], idb[:],
                                         cadd_bf[:, j * 512:(j + 1) * 512],
                                         start=True, stop=False)
                    with tc.tile_pool(name="ps_sh", bufs=3, space="PSUM") as Pp2:
                        for h in range(INH):
                            s_ps = Pp2.tile([TPC, T], f32, name="s_ps", tag="sps")
                            for j in range(2):
                                nc.tensor.matmul(s_ps[:, j * 512:(j + 1) * 512], iqT[:, h, :],
                                                 iknT_all[:, j * 512:(j + 1) * 512],
                                                 start=True, stop=True)
                            rel_h = Pa.tile([TPC, T], bf16, name="rel_h", tag="relh", bufs=3)
                            nc.scalar.activation(rel_h[:], s_ps[:], AF.Relu)
                            for j in range(2):
                                nc.tensor.matmul(sa_ps[:, j * 512:(j + 1) * 512], dgw[:, h, :],
                                                 rel_h[:, j * 512:(j + 1) * 512],
                                                 start=False, stop=(h == INH - 1 and j == 1))
                    nc.scalar.copy(s_acc[:], sa_ps[:])

                # ---- topk threshold scan (DVE serial) ----
                scr = Pe.tile([TPC, T], f32, tag="scrt")
                nc.vector.tensor_copy(scr[:], s_acc[:])
                m8 = Pa.tile([TPC, 8], f32)
                for it in range(1 if SKIP_TOPK else TOPK // 8):
                    nc.vector.max(m8[:], scr[:])
                    nc.vector.match_replace(scr[:], m8[:], scr[:], -3e38)

                # ---- mask from scan threshold ----
                mask01 = Pe.tile([TPC, T], f32, tag="scrt")
                nc.vector.tensor_scalar(mask01[:], s_acc[:], m8[:, 7:8], None, op0=ALU.is_ge)
                nc.vector.tensor_tensor(mask01[:], mask01[:], caus[:], op=ALU.mult)
                madd_bf = Pa.tile([TPC, T], bf16)
                nc.vector.tensor_scalar(madd_bf[:], mask01[:], 1.0, -NEG, op0=ALU.subtract, op1=ALU.mult)


                hp_ctx.__exit__(None, None, None)

                # ==== work that overlaps the scan: q_b, V, K^T ====
                qtn = Pe.tile([TPC, NH, DN], bf16)    # q_nope * SCALE
                qtp = Pe.tile([TPC, NH, DR], bf16)    # roped q_pe * SCALE
                with tc.tile_pool(name="ps_q", bufs=1, space="PSUM") as Pp:
                    q_ps = Pp.tile([TPC, NH * DQ], f32)
                    for k in range(QB):
                        wqb_k = Pw.tile([128, NH * DQ], bf16, name="wqb", tag="wstream")
                        nc.sync.dma_start(wqb_k[:], WQB[:].rearrange("(k p) n -> p k n", p=128)[:, k, :])
                        for j in range(6):
                            nc.tensor.matmul(q_ps[:, j * 512:(j + 1) * 512], qcT[:, k, :],
                                             wqb_k[:, j * 512:(j + 1) * 512],
                                             start=(k == 0), stop=(k == QB - 1))
                    qv = q_ps[:].rearrange("p (h d) -> p h d", h=NH)
                    nc.vector.tensor_copy(qtn[:], qv[:, :, :DN])
                    pe3 = qv[:, :, DN:].rearrange("p h (n two) -> p h n two", two=2)
                    qo3 = qtp[:].rearrange("p h (n two) -> p h n two", two=2)
                    c3 = csr[:].rearrange("p (h n) -> p h n", h=NH)
                    s3 = snr[:].rearrange("p (h n) -> p h n", h=NH)
                    qt1 = Pa.tile([TPC, NH, 32], f32, name="qt1", tag="qt1")
                    qt2 = Pa.tile([TPC, NH, 32], f32, name="qt2", tag="qt2")
                    nc.vector.tensor_tensor(qt1[:], pe3[:, :, :, 0], c3, op=ALU.mult)
                    nc.vector.tensor_tensor(qt2[:], pe3[:, :, :, 1], s3, op=ALU.mult)
                    nc.vector.tensor_sub(qo3[:, :, :, 0], qt1[:], qt2[:])
                    nc.vector.tensor_tensor(qt1[:], pe3[:, :, :, 0], s3, op=ALU.mult)
                    nc.vector.tensor_tensor(qt2[:], pe3[:, :, :, 1], c3, op=ALU.mult)
                    nc.vector.tensor_add(qo3[:, :, :, 1], qt1[:], qt2[:])

                qtnT = Pa.tile([DN, NH, TPC], bf16)
                qtpT = Pa.tile([DR, NH, TPC], bf16)
                with tc.tile_pool(name="ps_qt", bufs=2, space="PSUM") as Pp:
                    for h in range(NH):
                        tq1 = Pp.tile([DN, TPC], bf16, name="tq1", tag="tp")
                        nc.tensor.transpose(tq1[:], qtn[:, h, :], idb[:])
                        nc.scalar.copy(qtnT[:, h, :], tq1[:])
                        tq2 = Pp.tile([DR, TPC], bf16, name="tq2", tag="tp")
                        nc.tensor.transpose(tq2[:], qtp[:, h, :], idb[:])
                        nc.scalar.copy(qtpT[:, h, :], tq2[:])

                Pe.release()
                # V for all tokens -> DRAM scratch
                v_dram = Pd.tile([NCORES, 128, NH * DV], bf16)
                wv_sb = Pa.tile([128, 4, NH * DV], bf16)
                nc.sync.dma_start(wv_sb[:], WV[:].rearrange("(k p) n -> p k n", p=128))
                with tc.tile_pool(name="ps_vall", bufs=2, space="PSUM") as Pp:
                    for tch in range(NCORES):
                        v_ps = Pp.tile([128, NH * DV], f32, name="v_ps", tag="vps")
                        for k in range(4):
                            for j in range(4):
                                nc.tensor.matmul(v_ps[:, j * 512:(j + 1) * 512],
                                                 kvcT[:, k, tch * 128:(tch + 1) * 128],
                                                 wv_sb[:, k, j * 512:(j + 1) * 512],
                                                 start=(k == 0), stop=(k == 3))
                        v_sb = Pa.tile([128, NH * DV], bf16, name="v_sb", tag="vsb", bufs=2)
                        nc.scalar.copy(v_sb[:], v_ps[:])
                        nc.sync.dma_start(v_dram[:][tch], v_sb[:])

                # K^T for all heads -> DRAM scratch
                kt_dram = Pd.tile([NH, DN, T], bf16)
                with tc.tile_pool(name="ps_ktb", bufs=2, space="PSUM") as Pp:
                    for h in range(NH):
                        wkn_h = Pw.tile([128, 4, DN], bf16, name="wkn_h", tag="wknh", bufs=3)
                        nc.sync.dma_start(
                            wkn_h[:],
                            WKN[:, h * DN:(h + 1) * DN].rearrange("(k p) n -> p k n", p=128))
                        kt_ps = Pp.tile([DN, T], f32, name="kt_ps", tag="ktp")
                        for j in range(2):
                            for k in range(4):
                                nc.tensor.matmul(kt_ps[:, j * 512:(j + 1) * 512],
                                                 wkn_h[:, k, :],
                                                 kvcT[:, k, j * 512:(j + 1) * 512],
                                                 start=(k == 0), stop=(k == 3))
                        kt_sb = Pa.tile([DN, T], bf16, name="kt_sb", tag="kth", bufs=3)
                        nc.scalar.copy(kt_sb[:], kt_ps[:])
                        nc.sync.dma_start(kt_dram[:][h], kt_sb[:])

                # ---- MLA attention ----
                oT = Pa.tile([DV, NH, TPC], bf16)
                with tc.tile_pool(name="ps_att", bufs=1, space="PSUM") as Pp:
                    for h in range(NH):
                        v_h = Pa.tile([128, NCORES, DV], bf16, name="v_h", tag="vh", bufs=3)
                        nc.sync.dma_start(v_h[:], v_dram[:].rearrange("c p d -> p c d")[:, :, h * DV:(h + 1) * DV])
                        kt_h = Pa.tile([DN, T], bf16, name="kt_h", tag="kth2", bufs=3)
                        nc.sync.dma_start(kt_h[:], kt_dram[:][h])
                        a_ps = Pp.tile([TPC, T], f32, name="a_ps", tag="sps", bufs=3)
                        for j in range(2):
                            nc.tensor.matmul(a_ps[:, j * 512:(j + 1) * 512], qtnT[:, h, :],
                                             kt_h[:, j * 512:(j + 1) * 512],
                                             start=True, stop=False)
                            nc.tensor.matmul(a_ps[:, j * 512:(j + 1) * 512], qtpT[:, h, :],
                                             kpeT_all[:, j * 512:(j + 1) * 512],
                                             start=False, stop=False)
                            nc.tensor.matmul(a_ps[:, j * 512:(j + 1) * 512], idb[:],
                                             madd_bf[:, j * 512:(j + 1) * 512],
                                             start=False, stop=True)
                        pex = Pa.tile([TPC, T], bf16, name="pex")
                        rs = Pa.tile([TPC, 1], f32, name="rs")
                        nc.scalar.activation(pex[:], a_ps[:], AF.Exp, accum_out=rs[:])
                        nc.vector.reciprocal(rs[:], rs[:])
                        pb = Pa.tile([TPC, T], bf16, name="pb")
                        nc.vector.tensor_scalar(pb[:], pex[:], rs[:], None, op0=ALU.mult)
                        # transpose P in 8 chunks; copy alternating DVE/Act; accumulate O^T
                        o_ps = Pp.tile([DV, TPC], f32, name="o_ps", tag="ops")
                        for s in range(8):
                            pt = Pp.tile([128, TPC], bf16, name="pt", tag="tp")
                            nc.tensor.transpose(pt[:], pb[:, s * 128:(s + 1) * 128], idb[:])
                            pts = Pa.tile([128, TPC], bf16, name="pts", tag="pts", bufs=4)
                            if s % 2 == 0:
                                nc.vector.tensor_copy(pts[:], pt[:])
                            else:
                                nc.scalar.copy(pts[:], pt[:])
                            nc.tensor.matmul(o_ps[:], v_h[:, s, :], pts[:],
                                             start=(s == 0), stop=(s == 7))
                        nc.vector.tensor_copy(oT[:, h, :], o_ps[:])

                # ---- o_proj + residual ----
                x_own = Pa.tile([TPC, H], f32)
                with tc.tile_pool(name="ps_op", bufs=1, space="PSUM") as Pp:
                    d_ps = Pp.tile([TPC, H], f32)
                    for h in range(NH):
                        wo_k = Pw.tile([128, H], bf16, name="wo_k", tag="wstream")
                        nc.sync.dma_start(wo_k[:], WO[:].rearrange("(k p) n -> p k n", p=128)[:, h, :])
                        for j in range(4):
                            nc.tensor.matmul(d_ps[:, j * 512:(j + 1) * 512], oT[:, h, :],
                                             wo_k[:, j * 512:(j + 1) * 512],
                                             start=(h == 0), stop=(h == NH - 1))
                    nc.vector.tensor_tensor(x_own[:], d_ps[:], xo[:], op=ALU.add)

                # ---- post-LN pieces: r2, gate logits, rw, h2T_own ----
                sq2 = Pa.tile([TPC, H], f32, name="sq2a", tag="sq2")
                ss2 = Pa.tile([TPC, 1], f32)
                nc.scalar.activation(sq2[:], x_own[:], AF.Square, accum_out=ss2[:])
                r2 = Pa.tile([TPC, 1], f32)
                nc.scalar.activation(r2[:], ss2[:], AF.Sqrt, bias=eps_b[:], scale=1.0 / H)
                nc.vector.reciprocal(r2[:], r2[:])
                xT_own = Pa.tile([128, KB, TPC], f32)
                with tc.tile_pool(name="ps_xt", bufs=2, space="PSUM") as Pp:
                    for k in range(KB):
                        tx = Pp.tile([128, TPC], f32, name="tx", tag="tpf")
                        nc.tensor.transpose(tx[:], x_own[:, k * 128:(k + 1) * 128], idf[:])
                        nc.scalar.copy(xT_own[:, k, :], tx[:])
                lg = Pa.tile([TPC, NE], f32)
                with tc.tile_pool(name="ps_g", bufs=1, space="PSUM") as Pp:
                    l_ps = Pp.tile([TPC, NE], f32)
                    for k in range(KB):
                        nc.tensor.matmul(l_ps[:], xT_own[:, k, :], wg_sb[:, k, :],
                                         start=(k == 0), stop=(k == KB - 1))
                    nc.scalar.activation(lg[:], l_ps[:], AF.Copy, scale=r2[:])
                gm8 = Pa.tile([TPC, 8], f32)
                nc.vector.max(gm8[:], lg[:])
                negm0 = Pa.tile([TPC, 1], f32)
                nc.vector.tensor_scalar(negm0[:], gm8[:, 0:1], -1.0, None, op0=ALU.mult)
                el = Pa.tile([TPC, NE], f32)
                nc.scalar.activation(el[:], lg[:], AF.Exp, bias=negm0[:])
                dn1 = Pa.tile([TPC, 1], f32)
                nc.vector.tensor_tensor(dn1[:], gm8[:, 1:2], gm8[:, 0:1], op=ALU.subtract)
                nc.scalar.activation(dn1[:], dn1[:], AF.Exp)
                nc.vector.tensor_scalar(dn1[:], dn1[:], 1.0, None, op0=ALU.add)
                nc.vector.reciprocal(dn1[:], dn1[:])
                sel = Pa.tile([TPC, NE], f32)
                nc.vector.tensor_scalar(sel[:], lg[:], gm8[:, 1:2], None, op0=ALU.is_ge)
                rw = Pa.tile([TPC, NE], f32)
                nc.vector.scalar_tensor_tensor(rw[:], el[:], dn1[:], sel[:],
                                               op0=ALU.mult, op1=ALU.mult)

                # h2T_own in [t', k] layout (feature-major transport)
                r2row = Pa.tile([1, TPC], f32)
                r2bc = Pa.tile([128, TPC], f32)
                with tc.tile_pool(name="ps_r2", bufs=1, space="PSUM") as Pp:
                    r2p = Pp.tile([1, TPC], f32)
                    nc.tensor.transpose(r2p[:], r2[:], idf[:])
                    nc.scalar.copy(r2row[:], r2p[:])
                nc.gpsimd.partition_broadcast(r2bc[:], r2row[:])
                h2T_own = Pa.tile([128, TPC, KB], bf16)
                for k in range(KB):
                    nc.vector.tensor_tensor(h2T_own[:, :, k], xT_own[:, k, :], r2bc[:], op=ALU.mult)

                # ---- CC2 in two half-token slabs (first carries rw) ----
                nc.scalar.dma_start(cch0_in[:, :HLF * KB],
                                    h2T_own[:, :HLF, :].rearrange("p t k -> p (t k)"))
                nc.scalar.dma_start(cch0_in[:, HLF * KB:], rw[:].bitcast(bf16))
                if not SKIP_CC:
                    nc.gpsimd.collective_compute("AllGather", ALU.bypass, replica_groups=RG,
                                                 ins=[cch0_in[:].opt()], outs=[cch0_out[:].opt()])
                nc.scalar.dma_start(cch1_in[:],
                                    h2T_own[:, HLF:, :].rearrange("p t k -> p (t k)"))
                if not SKIP_CC:
                    nc.gpsimd.collective_compute("AllGather", ALU.bypass, replica_groups=RG,
                                                 ins=[cch1_in[:].opt()], outs=[cch1_out[:].opt()])

                # ---- shared expert on own tokens (overlaps CC2) ----
                ss_own = Pa.tile([TPC, SI], bf16)
                with tc.tile_pool(name="ps_shx", bufs=1, space="PSUM") as Pp:
                    gs_ps = Pp.tile([TPC, SI], f32, name="gs_ps")
                    us_ps = Pp.tile([TPC, SI], f32, name="us_ps")
                    for k in range(KB):
                        wsg_k = Pw.tile([128, SI], bf16, name="wsg_k", tag="wstream")
                        nc.sync.dma_start(wsg_k[:], WSG[:].rearrange("(k p) n -> p k n", p=128)[:, k, :])
                        wsu_k = Pw.tile([128, SI], bf16, name="wsu_k", tag="wstream")
                        nc.sync.dma_start(wsu_k[:], WSU[:].rearrange("(k p) n -> p k n", p=128)[:, k, :])
                        for j in range(2):
                            nc.tensor.matmul(gs_ps[:, j * 512:(j + 1) * 512], h2T_own[:, :, k],
                                             wsg_k[:, j * 512:(j + 1) * 512],
                                             start=(k == 0), stop=(k == KB - 1))
                            nc.tensor.matmul(us_ps[:, j * 512:(j + 1) * 512], h2T_own[:, :, k],
                                             wsu_k[:, j * 512:(j + 1) * 512],
                                             start=(k == 0), stop=(k == KB - 1))
                    sgo = Pa.tile([TPC, SI], f32, name="sgo", tag="sq2")
                    nc.scalar.activation(sgo[:], gs_ps[:], AF.Silu)
                    nc.vector.tensor_tensor(ss_own[:], sgo[:], us_ps[:], op=ALU.mult)
                ssT = Pa.tile([128, 8, TPC], bf16)
                with tc.tile_pool(name="ps_st", bufs=2, space="PSUM") as Pp:
                    for m in range(8):
                        tss = Pp.tile([128, TPC], bf16, name="tss", tag="tp")
                        nc.tensor.transpose(tss[:], ss_own[:, m * 128:(m + 1) * 128], idb[:])
                        nc.vector.tensor_copy(ssT[:, m, :], tss[:])
                with tc.tile_pool(name="ps_sd", bufs=1, space="PSUM") as Pp:
                    sh_ps = Pp.tile([TPC, H], f32)
                    for m in range(8):
                        wsd_m = Pw.tile([128, H], bf16, name="wsd_m", tag="wstream")
                        nc.sync.dma_start(wsd_m[:], WSD[:].rearrange("(k p) n -> p k n", p=128)[:, m, :])
                        for j in range(4):
                            nc.tensor.matmul(sh_ps[:, j * 512:(j + 1) * 512], ssT[:, m, :],
                                             wsd_m[:, j * 512:(j + 1) * 512],
                                             start=(m == 0), stop=(m == 7))
                    outx = Pa.tile([TPC, H], f32, name="outx", tag="sq2")
                    nc.vector.tensor_tensor(outx[:], sh_ps[:], x_own[:], op=ALU.add)
                nc.scalar.dma_start(OUT_X[:], outx[:])

            # =================== MoE phase (expert-parallel, dense) ===================
            with tc.tile_pool(name="moe", bufs=1) as Pm:
                weg = Pm.tile([128, MI // 128, KB, 128], bf16)
                weu = Pm.tile([128, MI // 128, KB, 128], bf16)
                for m in range(MI // 128):
                    nc.sync.dma_start(weg[:, m, :, :].rearrange("p k n -> p (k n)"), WEG[:][m])
                    nc.sync.dma_start(weu[:, m, :, :].rearrange("p k n -> p (k n)"), WEU[:][m])
                wed = Pm.tile([128, MI // 128, H], bf16)
                for m in range(MI // 128):
                    nc.sync.dma_start(wed[:, m, :], WED[:][m])
                # gathered h2T halves [p, c, t'(64), k]
                h2h0 = Pm.tile([128, NCORES, HLF, KB], bf16)
                nc.gpsimd.dma_start(
                    h2h0[:].rearrange("p c t k -> p c (t k)"),
                    cch0_out[:, :, :HLF * KB].rearrange("c p n -> p c n"))
                h2h1 = Pm.tile([128, NCORES, HLF, KB], bf16)
                nc.gpsimd.dma_start(
                    h2h1[:].rearrange("p c t k -> p c (t k)"),
                    cch1_out[:].rearrange("c p n -> p c n"))
                # rw for all tokens: [p=token-in-chunk, c, 8] f32 (bitcast pairs);
                # select own-expert column via one-hot dot on DVE
                rw_sb = Pm.tile([128, NCORES, 2 * NE], bf16)
                nc.gpsimd.dma_start(rw_sb[:],
                                    cch0_out[:, :, HLF * KB:].rearrange("c p n -> p c n"))
                rwe = Pm.tile([128, NCORES], f32)
                rwt = Pm.tile([128, NE], f32, name="rwt")
                for tch in range(NCORES):
                    nc.vector.tensor_tensor(rwt[:], rw_sb[:, tch, :].bitcast(f32), oh_bc[:], op=ALU.mult)
                    nc.vector.tensor_reduce(rwe[:, tch:tch + 1], rwt[:], AX.X, ALU.add)

                su = Pm.tile([128, MI // 128, T], bf16)   # silu(g)*u  [mi, (c t')]
                suv = su[:].rearrange("p m (c t) -> p m c t", c=NCORES)
                with tc.tile_pool(name="ps_moe", bufs=2, space="PSUM") as Pp:
                    for half, h2h in ((0, h2h0), (1, h2h1)):
                        for m in range(MI // 128):
                            g_ps = Pp.tile([128, 512], f32, name="g_ps", tag="gps")
                            u_ps = Pp.tile([128, 512], f32, name="u_ps", tag="ups")
                            gv = g_ps[:].rearrange("p (c t) -> p c t", c=NCORES)
                            uv = u_ps[:].rearrange("p (c t) -> p c t", c=NCORES)
                            for k in range(KB):
                                nc.tensor.matmul(g_ps[:], weg[:, m, k, :],
                                                 h2h[:, :, :, k].rearrange("p c t -> p (c t)"),
                                                 start=(k == 0), stop=(k == KB - 1))
                                nc.tensor.matmul(u_ps[:], weu[:, m, k, :],
                                                 h2h[:, :, :, k].rearrange("p c t -> p (c t)"),
                                                 start=(k == 0), stop=(k == KB - 1))
                            sg = Pm.tile([128, 512], f32, name="sg", tag="sgs", bufs=2)
                            nc.scalar.activation(sg[:], g_ps[:], AF.Silu)
                            nc.vector.tensor_tensor(sg[:], sg[:], u_ps[:], op=ALU.mult)
                            nc.vector.tensor_copy(
                                suv[:, m, :, half * HLF:(half + 1) * HLF],
                                sg[:].rearrange("p (c t) -> p c t", c=NCORES))

                with tc.tile_pool(name="ps_dn", bufs=2, space="PSUM") as Pp:
                    for tch in range(8):
                        dn_ps = Pp.tile([128, H], f32, name="dn_ps", tag="dnp")
                        for m in range(8):
                            for j in range(4):
                                nc.tensor.matmul(dn_ps[:, j * 512:(j + 1) * 512],
                                                 su[:, m, tch * 128:(tch + 1) * 128],
                                                 wed[:, m, j * 512:(j + 1) * 512],
                                                 start=(m == 0), stop=(m == 7))
                        ob = Pm.tile([128, H], bf16, name="ob", tag="obs")
                        # scale rows by rw[token, own_expert] (per-partition ptr)
                        nc.scalar.activation(ob[:], dn_ps[:], AF.Copy,
                                             scale=rwe[:, tch:tch + 1])
                        nc.sync.dma_start(OUT_P[:].rearrange("(c p) n -> c p n", p=128)[tch], ob[:])

    nc.compile()
    return nc


_NC = None


def kernel(**inputs):
    global _NC
    inp = {k: np.asarray(v) for k, v in inputs.items()}
    pos = inp["positions"].astype(np.int64)
    x = inp["hidden_states"].astype(np.float32)

    # ---- fold layernorm weights into downstream mats (host prep) ----
    iw = inp["input_ln_w"].astype(np.float32)
    qw = inp["q_a_ln_w"].astype(np.float32)
    kw = inp["kv_a_ln_w"].astype(np.float32)
    pw = inp["post_ln_w"].astype(np.float32)
    Wa = (iw[:, None] * inp["W_qkv_a"]).astype(BF)
    Wik = (iw[:, None] * inp["idx_wk"]).astype(BF)
    Wip = (iw[:, None] * inp["idx_wp_w"]).astype(BF)
    Wqb = (SCALE * qw[:, None] * inp["W_q_b"]).astype(BF)
    Wiq = (qw[:, None] * inp["idx_wq_b"]).astype(BF)
    Wkvb = (kw[:, None] * inp["W_kv_b"]).astype(np.float32).reshape(KL, NH, DN + DV)
    Wkn = np.ascontiguousarray(Wkvb[:, :, :DN].reshape(KL, NH * DN)).astype(BF)
    Wv = np.ascontiguousarray(Wkvb[:, :, DN:].reshape(KL, NH * DV)).astype(BF)
    Wo = inp["W_o"].astype(BF)
    Wg = (pw[:, None] * inp["W_gate"]).astype(np.float32)
    Weg = (pw[None, :, None] * inp["We_gate"]).astype(BF)
    Weu = (pw[None, :, None] * inp["We_up"]).astype(BF)
    Wed = inp["We_down"].astype(BF)
    Wsg = (pw[:, None] * inp["Ws_gate"]).astype(BF)
    Wsu = (pw[:, None] * inp["Ws_up"]).astype(BF)
    Wsd = inp["Ws_down"].astype(BF)

    # relayout expert weights: [H, MI] -> [m][p][k*128+mi'] with H=(k,p)
    def relay_up(W):   # [H, MI] -> [8, 128, 16*128]
        Wr = W.reshape(KB, 128, MI // 128, 128)          # k p m mi'
        return np.ascontiguousarray(Wr.transpose(2, 1, 0, 3).reshape(MI // 128, 128, KB * 128))

    def relay_dn(W):   # [MI, H] -> [8, 128, H]
        return np.ascontiguousarray(W.reshape(MI // 128, 128, H))

    inv = 1.0 / (BASE ** (np.arange(0, DR, 2, dtype=np.float32) / DR))
    ang = pos.astype(np.float32)[:, None] * inv           # [T, 32]
    cs_a, sn_a = np.cos(ang), np.sin(ang)

    in_maps = []
    for c in range(NCORES):
        rows = list(range(c * TPC, (c + 1) * TPC))
        posn = pos[rows]
        causm = (posn[:, None] >= pos[None, :]).astype(np.float32)
        cs = cs_a[rows]; sn = sn_a[rows]
        oh = np.zeros((1, NE), np.float32); oh[0, c] = 1.0
        in_maps.append({
            "OH": oh,
            "XO": np.ascontiguousarray(x[rows]),
            "CAUS": np.ascontiguousarray(causm),
            "CSR": np.ascontiguousarray(np.tile(cs, (1, NH)).astype(np.float32)),
            "SNR": np.ascontiguousarray(np.tile(sn, (1, NH)).astype(np.float32)),
            "KNW": inp["idx_kn_w"].astype(np.float32).reshape(1, IHD),
            "KNB": inp["idx_kn_b"].astype(np.float32).reshape(1, IHD),
            "WPB": inp["idx_wp_b"].astype(np.float32).reshape(1, INH),
            "WA": Wa, "WQB": Wqb, "WIQ": Wiq, "WIK": Wik, "WIP": Wip,
            "WKN": Wkn, "WV": Wv, "WO": Wo, "WG": Wg,
            "WEG": relay_up(Weg[c]),
            "WEU": relay_up(Weu[c]),
            "WED": relay_dn(Wed[c]),
            "WSG": Wsg, "WSU": Wsu, "WSD": Wsd,
        })

    if _NC is None:
        _NC = build()
    try:
        res = run_bass_kernel_spmd(_NC, in_maps, core_ids=list(range(NCORES)))
    except Exception:
        import time as _time
        _time.sleep(2.0)
        res = run_bass_kernel_spmd(_NC, in_maps, core_ids=list(range(NCORES)))

    out = np.zeros((T, H), np.float64)
    for c in range(NCORES):
        out += res.results[c]["OUT_P"].astype(np.float64)
    for c in range(NCORES):
        out[c * TPC:(c + 1) * TPC] += res.results[c]["OUT_X"].astype(np.float64)
    return out.astype(np.float32)


# revision 20
# speedup vs baseline: 1.1452x; 1.0688x over previous
"""Self-contained Trainium2 Bass kernel for the DeepseekV2 decoder layer problem.

Sharding (8 cores): core c owns the contiguous 128-token block [128c, 128c+128).
KV-side projections are computed per-own-token and AllGathered as one bundle
(kpe^T / ik^T / kv_latent^T).  Indexer scores + top-k + MLA attention + o_proj
run on own rows.  h2 is transported feature-major (h2^T) in two half-token
AllGathers (second half carries the router weights); MoE is expert-parallel
(1 routed expert per core, dense over all tokens) plus the shared expert on
own tokens.  Host sums the per-core partials.
"""
import sys
sys.path.insert(0, "/opt/trn_rl_repo")
import numpy as np
import ml_dtypes

import concourse.bass as bass
import concourse.mybir as mybir
from concourse import bacc, tile
from concourse.bass_utils import run_bass_kernel_spmd
from concourse.masks import make_identity

f32 = mybir.dt.float32
bf16 = mybir.dt.bfloat16
AF = mybir.ActivationFunctionType
ALU = mybir.AluOpType
AX = mybir.AxisListType
BF = ml_dtypes.bfloat16

# dims
T = 1024; H = 2048; NH = 16; DN = 128; DR = 64; DQ = DN + DR; DV = 128
QL = 1536; KL = 512
INH = 16; IHD = 128; TOPK = 256
NE = 8; MI = 1024; SI = 1024
BASE = 10000.0; EPS = 1e-6
SCALE = DQ ** -0.5
IDX_SCALE = IHD ** -0.5
FP8_MAX = 448.0
NCORES = 8
TPC = T // NCORES        # 128 tokens per core
NEG = -1e30
import os
SKIP_CC = os.environ.get("SKIP_CC") == "1"
SKIP_TOPK = os.environ.get("SKIP_TOPK") == "1"

KB = 16   # H/128 k-chunks
QB = 12   # QL/128
RG = [list(range(NCORES))]
CCL = DR + IHD + KL          # merged latent collective rows (704)
HLF = TPC // 2               # 64 tokens per h2 half


def build():
    nc = bacc.Bacc("TRN2", target_bir_lowering=False,
                   debug=os.environ.get("BASS_DEBUG") == "1",
                   enable_asserts=False, num_devices=NCORES)

    def din(name, shape, dt=bf16):
        return nc.dram_tensor(name, shape, dt, kind="ExternalInput").ap()

    # ---- per-core inputs ----
    XO = din("XO", [TPC, H], f32)              # x_in own rows
    CAUS = din("CAUS", [TPC, T], f32)          # causal01 over global keys
    CSR = din("CSR", [TPC, 512], f32)          # cos tiled 16x (unscaled)
    SNR = din("SNR", [TPC, 512], f32)
    OH = din("OH", [1, NE], f32)               # own-expert one-hot
    KNW = din("KNW", [1, IHD], f32)            # idx_kn_w
    KNB = din("KNB", [1, IHD], f32)
    WPB = din("WPB", [1, INH], f32)            # idx_wp_b
    WA = din("WA", [H, QL + KL + DR])          # bf16, ln-folded
    WQB = din("WQB", [QL, NH * DQ])
    WIQ = din("WIQ", [QL, INH * IHD])
    WIK = din("WIK", [H, IHD])
    WIP = din("WIP", [H, INH])
    WKN = din("WKN", [KL, NH * DN])
    WV = din("WV", [KL, NH * DV])
    WO = din("WO", [NH * DV, H])
    WG = din("WG", [H, NE], f32)
    WEG = din("WEG", [MI // 128, 128, KB * 128])   # [m][p][k*128+mi'] host-relaid
    WEU = din("WEU", [MI // 128, 128, KB * 128])
    WED = din("WED", [MI // 128, 128, H])          # [m][p=mi-in-chunk][H]
    WSG = din("WSG", [H, SI])
    WSU = din("WSU", [H, SI])
    WSD = din("WSD", [SI, H])

    OUT_P = nc.dram_tensor("OUT_P", [T, H], bf16, kind="ExternalOutput").ap()
    OUT_X = nc.dram_tensor("OUT_X", [TPC, H], f32, kind="ExternalOutput").ap()

    with tile.TileContext(nc) as tc:
        with tc.tile_pool(name="const", bufs=1) as Pc, \
             tc.tile_pool(name="dram", bufs=1, space="DRAM") as Pd:
            idf = Pc.tile([128, 128], f32)
            make_identity(nc, idf[:])
            idb = Pc.tile([128, 128], bf16)
            nc.vector.tensor_copy(idb[:], idf[:])
            eps_b = Pc.tile([128, 1], f32)
            nc.vector.memset(eps_b[:], EPS)

            xo = Pc.tile([TPC, H], f32)
            nc.sync.dma_start(xo[:], XO[:])
            caus = Pc.tile([TPC, T], f32)
            nc.sync.dma_start(caus[:], CAUS[:])
            csr = Pc.tile([TPC, 512], f32); nc.sync.dma_start(csr[:], CSR[:])
            snr = Pc.tile([TPC, 512], f32); nc.sync.dma_start(snr[:], SNR[:])
            knw_r = Pc.tile([1, IHD], f32); nc.sync.dma_start(knw_r[:], KNW[:])
            knb_r = Pc.tile([1, IHD], f32); nc.sync.dma_start(knb_r[:], KNB[:])
            wpb_r = Pc.tile([1, INH], f32); nc.sync.dma_start(wpb_r[:], WPB[:])
            knw_bc = Pc.tile([128, IHD], f32)
            nc.gpsimd.partition_broadcast(knw_bc[:], knw_r[:])
            knb_bc = Pc.tile([128, IHD], f32)
            nc.gpsimd.partition_broadcast(knb_bc[:], knb_r[:])
            wpb_bc = Pc.tile([128, INH], f32)
            nc.gpsimd.partition_broadcast(wpb_bc[:], wpb_r[:])
            wg_sb = Pc.tile([128, KB, NE], f32)
            nc.sync.dma_start(wg_sb[:], WG[:].rearrange("(k p) n -> p k n", p=128))
            oh_r = Pc.tile([1, NE], f32); nc.sync.dma_start(oh_r[:], OH[:])
            oh_bc = Pc.tile([128, NE], f32)
            nc.gpsimd.partition_broadcast(oh_bc[:], oh_r[:])

            # collective buffers
            cc1_in = Pd.tile([CCL, TPC], bf16)
            cc1_out = Pd.tile([NCORES, CCL, TPC], bf16, addr_space="Shared")
            HRW = HLF * KB + 2 * NE        # half-token h2T cols + rw bf16 pairs
            cch0_in = Pd.tile([128, HRW], bf16)
            cch0_out = Pd.tile([NCORES, 128, HRW], bf16, addr_space="Shared")
            cch1_in = Pd.tile([128, HLF * KB], bf16)
            cch1_out = Pd.tile([NCORES, 128, HLF * KB], bf16, addr_space="Shared")

            with tc.tile_pool(name="att", bufs=1) as Pa, \
                 tc.tile_pool(name="wstream", bufs=2) as Pw:
                Pe = tc.alloc_tile_pool(name="early", bufs=1)
                # rmsnorm scale r1 for own rows
                sq = Pa.tile([TPC, H], f32, name="sq_scratch", tag="sq2")
                ssq = Pa.tile([TPC, 1], f32)
                nc.scalar.activation(sq[:], xo[:], AF.Square, accum_out=ssq[:])
                r1 = Pa.tile([TPC, 1], f32)
                nc.scalar.activation(r1[:], ssq[:], AF.Sqrt, bias=eps_b[:], scale=1.0 / H)
                nc.vector.reciprocal(r1[:], r1[:])
                hn_own = Pe.tile([TPC, H], bf16)
                nc.vector.tensor_scalar(hn_own[:], xo[:], r1[:], None, op0=ALU.mult)
                hnT = Pe.tile([128, KB, TPC], bf16)
                with tc.tile_pool(name="ps_tr", bufs=2, space="PSUM") as Pp:
                    for k in range(KB):
                        tp = Pp.tile([128, 128], bf16, name="tp")
                        nc.tensor.transpose(tp[:], hn_own[:, k * 128:(k + 1) * 128], idb[:])
                        if k % 2 == 0:
                            nc.vector.tensor_copy(hnT[:, k, :], tp[:])
                        else:
                            nc.scalar.copy(hnT[:, k, :], tp[:])

                # ---- qkv_a: kv+kpe columns FIRST so CC1 can launch early ----
                with tc.tile_pool(name="ps_qkv", bufs=1, space="PSUM") as Pp:
                    kvp_ps = Pp.tile([TPC, KL + DR], f32)
                    for k in range(KB):
                        wakv_k = Pw.tile([128, KL + DR], bf16, name="wakv", tag="wknh", bufs=3)
                        with tc.high_priority():
                            nc.sync.dma_start(wakv_k[:], WA[:].rearrange("(k p) n -> p k n", p=128)[:, k, QL:])
                        nc.tensor.matmul(kvp_ps[:, 0:512], hnT[:, k, :], wakv_k[:, 0:512],
                                         start=(k == 0), stop=(k == KB - 1))
                        nc.tensor.matmul(kvp_ps[:, 512:], hnT[:, k, :], wakv_k[:, 512:],
                                         start=(k == 0), stop=(k == KB - 1))
                    # kv_c rmsnorm -> bf16
                    ksq = Pa.tile([TPC, KL], f32, name="ksq", tag="sq2")
                    kss = Pa.tile([TPC, 1], f32)
                    nc.scalar.activation(ksq[:], kvp_ps[:, :KL], AF.Square, accum_out=kss[:])
                    rkv = Pa.tile([TPC, 1], f32)
                    nc.scalar.activation(rkv[:], kss[:], AF.Sqrt, bias=eps_b[:], scale=1.0 / KL)
                    nc.vector.reciprocal(rkv[:], rkv[:])
                    kvn = Pa.tile([TPC, KL], bf16)
                    nc.vector.tensor_scalar(kvn[:], kvp_ps[:, :KL], rkv[:], None, op0=ALU.mult)

                    # k_pe rope (unscaled tables) -> bf16 [TPC, 64]
                    kpe = Pa.tile([TPC, DR], bf16)
                    t1 = Pa.tile([TPC, 32], f32, name="rt1", tag="rt1")
                    t2 = Pa.tile([TPC, 32], f32, name="rt2", tag="rt2")
                    pe_src = kvp_ps[:, KL:].rearrange("p (n two) -> p n two", two=2)
                    x1, x2 = pe_src[:, :, 0], pe_src[:, :, 1]
                    ko = kpe[:].rearrange("p (n two) -> p n two", two=2)
                    nc.vector.tensor_tensor(t1[:], x1, csr[:, :32], op=ALU.mult)
                    nc.vector.tensor_tensor(t2[:], x2, snr[:, :32], op=ALU.mult)
                    nc.vector.tensor_sub(ko[:, :, 0], t1[:], t2[:])
                    nc.vector.tensor_tensor(t1[:], x1, snr[:, :32], op=ALU.mult)
                    nc.vector.tensor_tensor(t2[:], x2, csr[:, :32], op=ALU.mult)
                    nc.vector.tensor_add(ko[:, :, 1], t1[:], t2[:])

                # ---- ik own: layernorm(hn @ Wik) + rope ----
                ikn = Pa.tile([TPC, IHD], bf16)
                with tc.tile_pool(name="ps_ik", bufs=1, space="PSUM") as Pp:
                    wik_sb = Pe.tile([128, KB, IHD], bf16)
                    with tc.high_priority():
                        nc.sync.dma_start(wik_sb[:], WIK[:].rearrange("(k p) n -> p k n", p=128))
                    ik_ps = Pp.tile([TPC, IHD], f32)
                    for k in range(KB):
                        nc.tensor.matmul(ik_ps[:], hnT[:, k, :], wik_sb[:, k, :],
                                         start=(k == 0), stop=(k == KB - 1))
                    negm = Pa.tile([TPC, 1], f32)
                    nc.vector.tensor_reduce(negm[:], ik_ps[:], AX.X, ALU.add, negate=True)
                    nc.vector.tensor_scalar(negm[:], negm[:], 1.0 / IHD, None, op0=ALU.mult)
                    xm = Pa.tile([TPC, IHD], f32)
                    nc.vector.tensor_scalar(xm[:], ik_ps[:], negm[:], None, op0=ALU.add)
                    xms = Pa.tile([TPC, IHD], f32)
                    vss = Pa.tile([TPC, 1], f32)
                    nc.scalar.activation(xms[:], xm[:], AF.Square, accum_out=vss[:])
                    rstd = Pa.tile([TPC, 1], f32)
                    nc.scalar.activation(rstd[:], vss[:], AF.Sqrt, bias=eps_b[:], scale=1.0 / IHD)
                    nc.vector.reciprocal(rstd[:], rstd[:])
                    ikf = Pa.tile([TPC, IHD], f32)
                    nc.vector.scalar_tensor_tensor(ikf[:], xm[:], rstd[:], knw_bc[:],
                                                   op0=ALU.mult, op1=ALU.mult)
                    nc.vector.tensor_add(ikf[:], ikf[:], knb_bc[:])
                    pe2 = ikf[:, :DR].rearrange("p (n two) -> p n two", two=2)
                    iko2 = ikn[:, :DR].rearrange("p (n two) -> p n two", two=2)
                    it1 = Pa.tile([TPC, 32], f32, name="it1", tag="rt1")
                    it2 = Pa.tile([TPC, 32], f32, name="it2", tag="rt2")
                    nc.vector.tensor_tensor(it1[:], pe2[:, :, 0], csr[:, :32], op=ALU.mult)
                    nc.vector.tensor_tensor(it2[:], pe2[:, :, 1], snr[:, :32], op=ALU.mult)
                    nc.vector.tensor_sub(iko2[:, :, 0], it1[:], it2[:])
                    nc.vector.tensor_tensor(it1[:], pe2[:, :, 0], snr[:, :32], op=ALU.mult)
                    nc.vector.tensor_tensor(it2[:], pe2[:, :, 1], csr[:, :32], op=ALU.mult)
                    nc.vector.tensor_add(iko2[:, :, 1], it1[:], it2[:])
                    nc.vector.tensor_copy(ikn[:, DR:], ikf[:, DR:])

                # transposes of kpe, ikn, kvn -> merged CC1 input
                with tc.tile_pool(name="ps_tr2", bufs=2, space="PSUM") as Pp:
                    kpeT_o = Pa.tile([DR, TPC], bf16)
                    tpp = Pp.tile([DR, 128], bf16, name="tpp", tag="tp")
                    nc.tensor.transpose(tpp[:], kpe[:], idb[:])
                    nc.vector.tensor_copy(kpeT_o[:], tpp[:])
                    nc.sync.dma_start(cc1_in[:DR, :], kpeT_o[:])
                    iknT_o = Pa.tile([IHD, TPC], bf16)
                    tpi = Pp.tile([IHD, TPC], bf16, name="tpi", tag="tp")
                    nc.tensor.transpose(tpi[:], ikn[:], idb[:])
                    nc.vector.tensor_copy(iknT_o[:], tpi[:])
                    nc.sync.dma_start(cc1_in[DR:DR + IHD, :], iknT_o[:])
                    kvT_o = Pa.tile([128, 4, TPC], bf16)
                    for k in range(4):
                        tpk = Pp.tile([128, 128], bf16, name="tpk", tag="tp")
                        nc.tensor.transpose(tpk[:], kvn[:, k * 128:(k + 1) * 128], idb[:])
                        nc.vector.tensor_copy(kvT_o[:, k, :], tpk[:])
                    nc.sync.dma_start(
                        cc1_in[DR + IHD:, :].rearrange("(k p) t -> p k t", p=128), kvT_o[:])
                if not SKIP_CC:
                    nc.gpsimd.collective_compute("AllGather", ALU.bypass, replica_groups=RG,
                                                 ins=[cc1_in[:].opt()], outs=[cc1_out[:].opt()])

                hp_ctx = tc.high_priority(offset=8000); hp_ctx.__enter__()
                # ---- q-part of qkv_a (overlaps CC1) ----
                with tc.tile_pool(name="ps_qp", bufs=1, space="PSUM") as Pp:
                    qc_ps2 = Pp.tile([TPC, QL], f32)
                    for k in range(KB):
                        waq_k = Pw.tile([128, QL], bf16, name="waq", tag="wstream")
                        nc.sync.dma_start(waq_k[:], WA[:].rearrange("(k p) n -> p k n", p=128)[:, k, :QL])
                        for j in range(3):
                            nc.tensor.matmul(qc_ps2[:, j * 512:(j + 1) * 512],
                                             hnT[:, k, :], waq_k[:, j * 512:(j + 1) * 512],
                                             start=(k == 0), stop=(k == KB - 1))
                    qsq = Pa.tile([TPC, QL], f32, name="qsq", tag="sq2")
                    qss = Pa.tile([TPC, 1], f32)
                    nc.scalar.activation(qsq[:], qc_ps2[:], AF.Square, accum_out=qss[:])
                    rq = Pa.tile([TPC, 1], f32)
                    nc.scalar.activation(rq[:], qss[:], AF.Sqrt, bias=eps_b[:], scale=1.0 / QL)
                    nc.vector.reciprocal(rq[:], rq[:])
                    qcn = Pe.tile([TPC, QL], bf16)
                    nc.vector.tensor_scalar(qcn[:], qc_ps2[:], rq[:], None, op0=ALU.mult)
                qcT = Pe.tile([128, QB, TPC], bf16)
                with tc.tile_pool(name="ps_qct", bufs=2, space="PSUM") as Pp:
                    for k in range(QB):
                        tpq = Pp.tile([128, 128], bf16, name="tpq", tag="tp")
                        nc.tensor.transpose(tpq[:], qcn[:, k * 128:(k + 1) * 128], idb[:])
                        nc.scalar.copy(qcT[:, k, :], tpq[:])

                # ---- iq (indexer q) FIRST: it gates the topk long pole ----
                iq_bf = Pe.tile([TPC, INH, IHD], bf16)
                qscale = Pa.tile([TPC, INH], f32)
                with tc.tile_pool(name="ps_iq", bufs=1, space="PSUM") as Pp:
                    iq_ps = Pp.tile([TPC, INH * IHD], f32)
                    for k in range(QB):
                        wiq_k = Pw.tile([128, INH * IHD], bf16, name="wiq", tag="wstream")
                        nc.sync.dma_start(wiq_k[:], WIQ[:].rearrange("(k p) n -> p k n", p=128)[:, k, :])
                        for j in range(4):
                            nc.tensor.matmul(iq_ps[:, j * 512:(j + 1) * 512], qcT[:, k, :],
                                             wiq_k[:, j * 512:(j + 1) * 512],
                                             start=(k == 0), stop=(k == QB - 1))
                    iqv = iq_ps[:].rearrange("p (h d) -> p h d", h=INH)
                    ipe = iqv[:, :, :DR].rearrange("p h (n two) -> p h n two", two=2)
                    ioe = iq_bf[:, :, :DR].rearrange("p h (n two) -> p h n two", two=2)
                    c3r = csr[:].rearrange("p (h n) -> p h n", h=NH)
                    s3r = snr[:].rearrange("p (h n) -> p h n", h=NH)
                    iq1 = Pa.tile([TPC, INH, 32], f32, name="iq1", tag="qt1")
                    iq2 = Pa.tile([TPC, INH, 32], f32, name="iq2", tag="qt2")
                    nc.vector.tensor_tensor(iq1[:], ipe[:, :, :, 0], c3r, op=ALU.mult)
                    nc.vector.tensor_tensor(iq2[:], ipe[:, :, :, 1], s3r, op=ALU.mult)
                    nc.vector.tensor_sub(ioe[:, :, :, 0], iq1[:], iq2[:])
                    nc.vector.tensor_tensor(iq1[:], ipe[:, :, :, 0], s3r, op=ALU.mult)
                    nc.vector.tensor_tensor(iq2[:], ipe[:, :, :, 1], c3r, op=ALU.mult)
                    nc.vector.tensor_add(ioe[:, :, :, 1], iq1[:], iq2[:])
                    nc.vector.tensor_copy(iq_bf[:, :, DR:], iqv[:, :, DR:])
                    nc.vector.tensor_reduce(qscale[:], iq_bf[:], AX.X, ALU.max,
                                            apply_absolute_value=True)
                # q_scale = exp2(ceil(log2(max(amax,1e-12)/448)))
                zz = Pa.tile([TPC, INH], f32)
                nc.vector.tensor_scalar(zz[:], qscale[:], 1e-12, 1.0 / FP8_MAX, op0=ALU.max, op1=ALU.mult)
                man = Pa.tile([TPC, INH], mybir.dt.uint32)
                nc.vector.tensor_scalar(man[:], zz[:].bitcast(mybir.dt.uint32), 0x007FFFFF, None, op0=ALU.bitwise_and)
                exb = Pa.tile([TPC, INH], mybir.dt.uint32)
                nc.vector.tensor_scalar(exb[:], zz[:].bitcast(mybir.dt.uint32), 0xFF800000, None, op0=ALU.bitwise_and)
                nc.vector.tensor_scalar(man[:], man[:], 0, None, op0=ALU.not_equal)
                nc.vector.tensor_scalar(man[:], man[:], 23, None, op0=ALU.logical_shift_left)
                nc.vector.tensor_tensor(exb[:], exb[:], man[:], op=ALU.add)
                nc.vector.tensor_scalar(qscale[:], exb[:].bitcast(f32), IDX_SCALE * (INH ** -0.5), None, op0=ALU.mult)

                iqT = Pe.tile([IHD, INH, TPC], bf16)
                with tc.tile_pool(name="ps_iqt", bufs=2, space="PSUM") as Pp:
                    for h in range(INH):
                        ti = Pp.tile([IHD, TPC], bf16, name="ti", tag="tp")
                        nc.tensor.transpose(ti[:], iq_bf[:, h, :], idb[:])
                        nc.scalar.copy(iqT[:, h, :], ti[:])

                # wts = (hn @ Wip + b) * qscale_scaled ; then diag(wts_h) mats
                wts = Pa.tile([TPC, INH], f32)
                with tc.tile_pool(name="ps_wp", bufs=1, space="PSUM") as Pp:
                    wip_sb = Pe.tile([128, KB, INH], bf16)
                    nc.sync.dma_start(wip_sb[:], WIP[:].rearrange("(k p) n -> p k n", p=128))
                    wp_ps = Pp.tile([TPC, INH], f32)
                    for k in range(KB):
                        nc.tensor.matmul(wp_ps[:], hnT[:, k, :], wip_sb[:, k, :],
                                         start=(k == 0), stop=(k == KB - 1))
                    nc.vector.tensor_add(wts[:], wp_ps[:], wpb_bc[:])
                    nc.vector.tensor_tensor(wts[:], wts[:], qscale[:], op=ALU.mult)
                dgw = Pe.tile([128, INH, 128], bf16)      # diag(wts_h) per head
                for h in range(INH):
                    nc.vector.tensor_scalar(dgw[:, h, :], idb[:], wts[:, h:h + 1], None, op0=ALU.mult)

                # causal additive mask as bf16 (injected into score PSUM via idb matmul)
                cadd_bf = Pe.tile([TPC, T], bf16)
                nc.vector.tensor_scalar(cadd_bf[:], caus[:], 1.0, -NEG, op0=ALU.subtract, op1=ALU.mult)

                # ---- gathered latent -> SBUF (global token order) ----
                kpeT_all = Pa.tile([DR, T], bf16)
                nc.gpsimd.dma_start(kpeT_all[:].rearrange("d (c t) -> d c t", c=NCORES),
                                    cc1_out[:, :DR, :].rearrange("c d t -> d c t"))
                iknT_all = Pe.tile([IHD, T], bf16)
                nc.gpsimd.dma_start(iknT_all[:].rearrange("d (c t) -> d c t", c=NCORES),
                                    cc1_out[:, DR:DR + IHD, :].rearrange("c d t -> d c t"))
                kvcT = Pa.tile([128, 4, T], bf16)
                for k in range(4):
                    nc.gpsimd.dma_start(
                        kvcT[:, k, :].rearrange("p (c t) -> p c t", c=NCORES),
                        cc1_out[:, DR + IHD + k * 128:DR + IHD + (k + 1) * 128, :]
                        .rearrange("c p t -> p c t"))

                # ---- indexer scores on PE: s_acc = mask + sum_h diag(wts_h) @ relu(s_h) ----
                s_acc = Pe.tile([TPC, T], f32)
                with tc.tile_pool(name="ps_s", bufs=1, space="PSUM") as Pp:
                    sa_ps = Pp.tile([TPC, T], f32, name="sa_ps")
                    for j in range(2):
                        nc.tensor.matmul(sa_ps[:, j * 512:(j + 1) * 512], idb[:],
                                         cadd_bf[:, j * 512:(j + 1) * 512],
                                         start=True, stop=False)
                    with tc.tile_pool(name="ps_sh", bufs=3, space="PSUM") as Pp2:
                        for h in range(INH):
                            s_ps = Pp2.tile([TPC, T], f32, name="s_ps", tag="sps")
                            for j in range(2):
                                nc.tensor.matmul(s_ps[:, j * 512:(j + 1) * 512], iqT[:, h, :],
                                                 iknT_all[:, j * 512:(j + 1) * 512],
                                                 start=True, stop=True)
                            rel_h = Pa.tile([TPC, T], bf16, name="rel_h", tag="relh", bufs=3)
                            nc.scalar.activation(rel_h[:], s_ps[:], AF.Relu)
                            for j in range(2):
                                nc.tensor.matmul(sa_ps[:, j * 512:(j + 1) * 512], dgw[:, h, :],
                                                 rel_h[:, j * 512:(j + 1) * 512],
                                                 start=False, stop=(h == INH - 1 and j == 1))
                    nc.scalar.copy(s_acc[:], sa_ps[:])

                # ---- topk threshold scan (DVE serial) ----
                scr = Pe.tile([TPC, T], f32, tag="scrt")
                nc.vector.tensor_copy(scr[:], s_acc[:])
                m8 = Pa.tile([TPC, 8], f32)
                for it in range(1 if SKIP_TOPK else TOPK // 8):
                    nc.vector.max(m8[:], scr[:])
                    nc.vector.match_replace(scr[:], m8[:], scr[:], -3e38)

                # ---- mask from scan threshold ----
                mask01 = Pe.tile([TPC, T], f32, tag="scrt")
                nc.vector.tensor_scalar(mask01[:], s_acc[:], m8[:, 7:8], None, op0=ALU.is_ge)
                nc.vector.tensor_tensor(mask01[:], mask01[:], caus[:], op=ALU.mult)
                madd_bf = Pa.tile([TPC, T], bf16)
                nc.vector.tensor_scalar(madd_bf[:], mask01[:], 1.0, -NEG, op0=ALU.subtract, op1=ALU.mult)


                hp_ctx.__exit__(None, None, None)

                # ==== work that overlaps the scan: q_b, V, K^T ====
                qtn = Pe.tile([TPC, NH, DN], bf16)    # q_nope * SCALE
                qtp = Pe.tile([TPC, NH, DR], bf16)    # roped q_pe * SCALE
                with tc.tile_pool(name="ps_q", bufs=1, space="PSUM") as Pp:
                    q_ps = Pp.tile([TPC, NH * DQ], f32)
                    for k in range(QB):
                        wqb_k = Pw.tile([128, NH * DQ], bf16, name="wqb", tag="wstream")
                        nc.sync.dma_start(wqb_k[:], WQB[:].rearrange("(k p) n -> p k n", p=128)[:, k, :])
                        for j in range(6):
                            nc.tensor.matmul(q_ps[:, j * 512:(j + 1) * 512], qcT[:, k, :],
                                             wqb_k[:, j * 512:(j + 1) * 512],
                                             start=(k == 0), stop=(k == QB - 1))
                    qv = q_ps[:].rearrange("p (h d) -> p h d", h=NH)
                    nc.vector.tensor_copy(qtn[:], qv[:, :, :DN])
                    pe3 = qv[:, :, DN:].rearrange("p h (n two) -> p h n two", two=2)
                    qo3 = qtp[:].rearrange("p h (n two) -> p h n two", two=2)
                    c3 = csr[:].rearrange("p (h n) -> p h n", h=NH)
                    s3 = snr[:].rearrange("p (h n) -> p h n", h=NH)
                    qt1 = Pa.tile([TPC, NH, 32], f32, name="qt1", tag="qt1")
                    qt2 = Pa.tile([TPC, NH, 32], f32, name="qt2", tag="qt2")
                    nc.vector.tensor_tensor(qt1[:], pe3[:, :, :, 0], c3, op=ALU.mult)
                    nc.vector.tensor_tensor(qt2[:], pe3[:, :, :, 1], s3, op=ALU.mult)
                    nc.vector.tensor_sub(qo3[:, :, :, 0], qt1[:], qt2[:])
                    nc.vector.tensor_tensor(qt1[:], pe3[:, :, :, 0], s3, op=ALU.mult)
                    nc.vector.tensor_tensor(qt2[:], pe3[:, :, :, 1], c3, op=ALU.mult)
                    nc.vector.tensor_add(qo3[:, :, :, 1], qt1[:], qt2[:])

                qtnT = Pa.tile([DN, NH, TPC], bf16)
                qtpT = Pa.tile([DR, NH, TPC], bf16)
                with tc.tile_pool(name="ps_qt", bufs=2, space="PSUM") as Pp:
                    for h in range(NH):
                        tq1 = Pp.tile([DN, TPC], bf16, name="tq1", tag="tp")
                        nc.tensor.transpose(tq1[:], qtn[:, h, :], idb[:])
                        nc.scalar.copy(qtnT[:, h, :], tq1[:])
                        tq2 = Pp.tile([DR, TPC], bf16, name="tq2", tag="tp")
                        nc.tensor.transpose(tq2[:], qtp[:, h, :], idb[:])
                        nc.scalar.copy(qtpT[:, h, :], tq2[:])

                Pe.release()
                # V for all tokens -> DRAM scratch
                v_dram = Pd.tile([NCORES, 128, NH * DV], bf16)
                wv_sb = Pa.tile([128, 4, NH * DV], bf16)
                nc.sync.dma_start(wv_sb[:], WV[:].rearrange("(k p) n -> p k n", p=128))
                with tc.tile_pool(name="ps_vall", bufs=2, space="PSUM") as Pp:
                    for tch in range(NCORES):
                        v_ps = Pp.tile([128, NH * DV], f32, name="v_ps", tag="vps")
                        for k in range(4):
                            for j in range(4):
                                nc.tensor.matmul(v_ps[:, j * 512:(j + 1) * 512],
                                                 kvcT[:, k, tch * 128:(tch + 1) * 128],
                                                 wv_sb[:, k, j * 512:(j + 1) * 512],
                                                 start=(k == 0), stop=(k == 3))
                        v_sb = Pa.tile([128, NH * DV], bf16, name="v_sb", tag="vsb", bufs=2)
                        nc.scalar.copy(v_sb[:], v_ps[:])
                        nc.sync.dma_start(v_dram[:][tch], v_sb[:])

                # K^T for all heads -> DRAM scratch
                kt_dram = Pd.tile([NH, DN, T], bf16)
                with tc.tile_pool(name="ps_ktb", bufs=2, space="PSUM") as Pp:
                    for h in range(NH):
                        wkn_h = Pw.tile([128, 4, DN], bf16, name="wkn_h", tag="wknh", bufs=3)
                        nc.sync.dma_start(
                            wkn_h[:],
                            WKN[:, h * DN:(h + 1) * DN].rearrange("(k p) n -> p k n", p=128))
                        kt_ps = Pp.tile([DN, T], f32, name="kt_ps", tag="ktp")
                        for j in range(2):
                            for k in range(4):
                                nc.tensor.matmul(kt_ps[:, j * 512:(j + 1) * 512],
                                                 wkn_h[:, k, :],
                                                 kvcT[:, k, j * 512:(j + 1) * 512],
                                                 start=(k == 0), stop=(k == 3))
                        kt_sb = Pa.tile([DN, T], bf16, name="kt_sb", tag="kth", bufs=3)
                        nc.scalar.copy(kt_sb[:], kt_ps[:])
                        nc.sync.dma_start(kt_dram[:][h], kt_sb[:])

                # ---- MLA attention ----
                oT = Pa.tile([DV, NH, TPC], bf16)
                with tc.tile_pool(name="ps_att", bufs=1, space="PSUM") as Pp:
                    for h in range(NH):
                        v_h = Pa.tile([128, NCORES, DV], bf16, name="v_h", tag="vh", bufs=3)
                        nc.sync.dma_start(v_h[:], v_dram[:].rearrange("c p d -> p c d")[:, :, h * DV:(h + 1) * DV])
                        kt_h = Pa.tile([DN, T], bf16, name="kt_h", tag="kth2", bufs=3)
                        nc.sync.dma_start(kt_h[:], kt_dram[:][h])
                        a_ps = Pp.tile([TPC, T], f32, name="a_ps", tag="sps", bufs=3)
                        for j in range(2):
                            nc.tensor.matmul(a_ps[:, j * 512:(j + 1) * 512], qtnT[:, h, :],
                                             kt_h[:, j * 512:(j + 1) * 512],
                                             start=True, stop=False)
                            nc.tensor.matmul(a_ps[:, j * 512:(j + 1) * 512], qtpT[:, h, :],
                                             kpeT_all[:, j * 512:(j + 1) * 512],
                                             start=False, stop=False)
                            nc.tensor.matmul(a_ps[:, j * 512:(j + 1) * 512], idb[:],
                                             madd_bf[:, j * 512:(j + 1) * 512],
                                             start=False, stop=True)
                        pex = Pa.tile([TPC, T], bf16, name="pex")
                        rs = Pa.tile([TPC, 1], f32, name="rs")
                        nc.scalar.activation(pex[:], a_ps[:], AF.Exp, accum_out=rs[:])
                        nc.vector.reciprocal(rs[:], rs[:])
                        pb = Pa.tile([TPC, T], bf16, name="pb")
                        nc.vector.tensor_scalar(pb[:], pex[:], rs[:], None, op0=ALU.mult)
                        # transpose P in 8 chunks; copy alternating DVE/Act; accumulate O^T
                        o_ps = Pp.tile([DV, TPC], f32, name="o_ps", tag="ops")
                        for s in range(8):
                            pt = Pp.tile([128, TPC], bf16, name="pt", tag="tp")
                            nc.tensor.transpose(pt[:], pb[:, s * 128:(s + 1) * 128], idb[:])
                            pts = Pa.tile([128, TPC], bf16, name="pts", tag="pts", bufs=4)
                            if s % 2 == 0:
                                nc.vector.tensor_copy(pts[:], pt[:])
                            else:
                                nc.scalar.copy(pts[:], pt[:])
                            nc.tensor.matmul(o_ps[:], v_h[:, s, :], pts[:],
                                             start=(s == 0), stop=(s == 7))
                        nc.vector.tensor_copy(oT[:, h, :], o_ps[:])

                # ---- o_proj + residual ----
                x_own = Pa.tile([TPC, H], f32)
                with tc.tile_pool(name="ps_op", bufs=1, space="PSUM") as Pp:
                    d_ps = Pp.tile([TPC, H], f32)
                    for h in range(NH):
                        wo_k = Pw.tile([128, H], bf16, name="wo_k", tag="wstream")
                        nc.sync.dma_start(wo_k[:], WO[:].rearrange("(k p) n -> p k n", p=128)[:, h, :])
                        for j in range(4):
                            nc.tensor.matmul(d_ps[:, j * 512:(j + 1) * 512], oT[:, h, :],
                                             wo_k[:, j * 512:(j + 1) * 512],
                                             start=(h == 0), stop=(h == NH - 1))
                    nc.vector.tensor_tensor(x_own[:], d_ps[:], xo[:], op=ALU.add)

                # ---- post-LN pieces: r2, gate logits, rw, h2T_own ----
                sq2 = Pa.tile([TPC, H], f32, name="sq2a", tag="sq2")
                ss2 = Pa.tile([TPC, 1], f32)
                nc.scalar.activation(sq2[:], x_own[:], AF.Square, accum_out=ss2[:])
                r2 = Pa.tile([TPC, 1], f32)
                nc.scalar.activation(r2[:], ss2[:], AF.Sqrt, bias=eps_b[:], scale=1.0 / H)
                nc.vector.reciprocal(r2[:], r2[:])
                xT_own = Pa.tile([128, KB, TPC], f32)
                with tc.tile_pool(name="ps_xt", bufs=2, space="PSUM") as Pp:
                    for k in range(KB):
                        tx = Pp.tile([128, TPC], f32, name="tx", tag="tpf")
                        nc.tensor.transpose(tx[:], x_own[:, k * 128:(k + 1) * 128], idf[:])
                        nc.scalar.copy(xT_own[:, k, :], tx[:])
                lg = Pa.tile([TPC, NE], f32)
                with tc.tile_pool(name="ps_g", bufs=1, space="PSUM") as Pp:
                    l_ps = Pp.tile([TPC, NE], f32)
                    for k in range(KB):
                        nc.tensor.matmul(l_ps[:], xT_own[:, k, :], wg_sb[:, k, :],
                                         start=(k == 0), stop=(k == KB - 1))
                    nc.scalar.activation(lg[:], l_ps[:], AF.Copy, scale=r2[:])
                gm8 = Pa.tile([TPC, 8], f32)
                nc.vector.max(gm8[:], lg[:])
                negm0 = Pa.tile([TPC, 1], f32)
                nc.vector.tensor_scalar(negm0[:], gm8[:, 0:1], -1.0, None, op0=ALU.mult)
                el = Pa.tile([TPC, NE], f32)
                nc.scalar.activation(el[:], lg[:], AF.Exp, bias=negm0[:])
                dn1 = Pa.tile([TPC, 1], f32)
                nc.vector.tensor_tensor(dn1[:], gm8[:, 1:2], gm8[:, 0:1], op=ALU.subtract)
                nc.scalar.activation(dn1[:], dn1[:], AF.Exp)
                nc.vector.tensor_scalar(dn1[:], dn1[:], 1.0, None, op0=ALU.add)
                nc.vector.reciprocal(dn1[:], dn1[:])
                sel = Pa.tile([TPC, NE], f32)
                nc.vector.tensor_scalar(sel[:], lg[:], gm8[:, 1:2], None, op0=ALU.is_ge)
                rw = Pa.tile([TPC, NE], f32)
                nc.vector.scalar_tensor_tensor(rw[:], el[:], dn1[:], sel[:],
                                               op0=ALU.mult, op1=ALU.mult)

                # h2T_own in [t', k] layout (feature-major transport)
                r2row = Pa.tile([1, TPC], f32)
                r2bc = Pa.tile([128, TPC], f32)
                with tc.tile_pool(name="ps_r2", bufs=1, space="PSUM") as Pp:
                    r2p = Pp.tile([1, TPC], f32)
                    nc.tensor.transpose(r2p[:], r2[:], idf[:])
                    nc.scalar.copy(r2row[:], r2p[:])
                nc.gpsimd.partition_broadcast(r2bc[:], r2row[:])
                h2T_own = Pa.tile([128, TPC, KB], bf16)
                for k in range(KB):
                    nc.vector.tensor_tensor(h2T_own[:, :, k], xT_own[:, k, :], r2bc[:], op=ALU.mult)

                # ---- CC2 in two half-token slabs (first carries rw) ----
                nc.scalar.dma_start(cch0_in[:, :HLF * KB],
                                    h2T_own[:, :HLF, :].rearrange("p t k -> p (t k)"))
                nc.scalar.dma_start(cch0_in[:, HLF * KB:], rw[:].bitcast(bf16))
                if not SKIP_CC:
                    nc.gpsimd.collective_compute("AllGather", ALU.bypass, replica_groups=RG,
                                                 ins=[cch0_in[:].opt()], outs=[cch0_out[:].opt()])
                nc.scalar.dma_start(cch1_in[:],
                                    h2T_own[:, HLF:, :].rearrange("p t k -> p (t k)"))
                if not SKIP_CC:
                    nc.gpsimd.collective_compute("AllGather", ALU.bypass, replica_groups=RG,
                                                 ins=[cch1_in[:].opt()], outs=[cch1_out[:].opt()])

                # ---- shared expert on own tokens (overlaps CC2) ----
                ss_own = Pa.tile([TPC, SI], bf16)
                with tc.tile_pool(name="ps_shx", bufs=1, space="PSUM") as Pp:
                    gs_ps = Pp.tile([TPC, SI], f32, name="gs_ps")
                    us_ps = Pp.tile([TPC, SI], f32, name="us_ps")
                    for k in range(KB):
                        wsg_k = Pw.tile([128, SI], bf16, name="wsg_k", tag="wstream")
                        nc.sync.dma_start(wsg_k[:], WSG[:].rearrange("(k p) n -> p k n", p=128)[:, k, :])
                        wsu_k = Pw.tile([128, SI], bf16, name="wsu_k", tag="wstream")
                        nc.sync.dma_start(wsu_k[:], WSU[:].rearrange("(k p) n -> p k n", p=128)[:, k, :])
                        for j in range(2):
                            nc.tensor.matmul(gs_ps[:, j * 512:(j + 1) * 512], h2T_own[:, :, k],
                                             wsg_k[:, j * 512:(j + 1) * 512],
                                             start=(k == 0), stop=(k == KB - 1))
                            nc.tensor.matmul(us_ps[:, j * 512:(j + 1) * 512], h2T_own[:, :, k],
                                             wsu_k[:, j * 512:(j + 1) * 512],
                                             start=(k == 0), stop=(k == KB - 1))
                    sgo = Pa.tile([TPC, SI], f32, name="sgo", tag="sq2")
                    nc.scalar.activation(sgo[:], gs_ps[:], AF.Silu)
                    nc.vector.tensor_tensor(ss_own[:], sgo[:], us_ps[:], op=ALU.mult)
                ssT = Pa.tile([128, 8, TPC], bf16)
                with tc.tile_pool(name="ps_st", bufs=2, space="PSUM") as Pp:
                    for m in range(8):
                        tss = Pp.tile([128, TPC], bf16, name="tss", tag="tp")
                        nc.tensor.transpose(tss[:], ss_own[:, m * 128:(m + 1) * 128], idb[:])
                        nc.vector.tensor_copy(ssT[:, m, :], tss[:])
                with tc.tile_pool(name="ps_sd", bufs=1, space="PSUM") as Pp:
                    sh_ps = Pp.tile([TPC, H], f32)
                    for m in range(8):
                        wsd_m = Pw.tile([128, H], bf16, name="wsd_m", tag="wstream")
                        nc.sync.dma_start(wsd_m[:], WSD[:].rearrange("(k p) n -> p k n", p=128)[:, m, :])
                        for j in range(4):
                            nc.tensor.matmul(sh_ps[:, j * 512:(j + 1) * 512], ssT[:, m, :],
                                             wsd_m[:, j * 512:(j + 1) * 512],
                                             start=(m == 0), stop=(m == 7))
                    outx = Pa.tile([TPC, H], f32, name="outx", tag="sq2")
                    nc.vector.tensor_tensor(outx[:], sh_ps[:], x_own[:], op=ALU.add)
                nc.scalar.dma_start(OUT_X[:], outx[:])

            # =================== MoE phase (expert-parallel, dense) ===================
            with tc.tile_pool(name="moe", bufs=1) as Pm:
                weg = Pm.tile([128, MI // 128, KB, 128], bf16)
                weu = Pm.tile([128, MI // 128, KB, 128], bf16)
                for m in range(MI // 128):
                    nc.sync.dma_start(weg[:, m, :, :].rearrange("p k n -> p (k n)"), WEG[:][m])
                    nc.sync.dma_start(weu[:, m, :, :].rearrange("p k n -> p (k n)"), WEU[:][m])
                wed = Pm.tile([128, MI // 128, H], bf16)
                for m in range(MI // 128):
                    nc.sync.dma_start(wed[:, m, :], WED[:][m])
                # gathered h2T halves [p, c, t'(64), k]
                h2h0 = Pm.tile([128, NCORES, HLF, KB], bf16)
                nc.gpsimd.dma_start(
                    h2h0[:].rearrange("p c t k -> p c (t k)"),
                    cch0_out[:, :, :HLF * KB].rearrange("c p n -> p c n"))
                rw_sb = Pm.tile([128, NCORES, 2 * NE], bf16)
                nc.gpsimd.dma_start(rw_sb[:],
                                    cch0_out[:, :, HLF * KB:].rearrange("c p n -> p c n"))
                h2h1 = Pm.tile([128, NCORES, HLF, KB], bf16)
                nc.gpsimd.dma_start(
                    h2h1[:].rearrange("p c t k -> p c (t k)"),
                    cch1_out[:].rearrange("c p n -> p c n"))

                su = Pm.tile([128, MI // 128, T], bf16)   # silu(g)*u  [mi, (c t')]
                suv = su[:].rearrange("p m (c t) -> p m c t", c=NCORES)
                with tc.tile_pool(name="ps_moe", bufs=2, space="PSUM") as Pp:
                    for half, h2h in ((0, h2h0), (1, h2h1)):
                        for m in range(MI // 128):
                            g_ps = Pp.tile([128, 512], f32, name="g_ps", tag="gps")
                            u_ps = Pp.tile([128, 512], f32, name="u_ps", tag="ups")
                            gv = g_ps[:].rearrange("p (c t) -> p c t", c=NCORES)
                            uv = u_ps[:].rearrange("p (c t) -> p c t", c=NCORES)
                            for k in range(KB):
                                nc.tensor.matmul(g_ps[:], weg[:, m, k, :],
                                                 h2h[:, :, :, k].rearrange("p c t -> p (c t)"),
                                                 start=(k == 0), stop=(k == KB - 1))
                                nc.tensor.matmul(u_ps[:], weu[:, m, k, :],
                                                 h2h[:, :, :, k].rearrange("p c t -> p (c t)"),
                                                 start=(k == 0), stop=(k == KB - 1))
                            sg = Pm.tile([128, 512], f32, name="sg", tag="sgs", bufs=2)
                            nc.scalar.activation(sg[:], g_ps[:], AF.Silu)
                            nc.vector.tensor_tensor(sg[:], sg[:], u_ps[:], op=ALU.mult)
                            nc.vector.tensor_copy(
                                suv[:, m, :, half * HLF:(half + 1) * HLF],
                                sg[:].rearrange("p (c t) -> p c t", c=NCORES))

                # own-expert rw column selection (needed only by down outputs)
                rwe = Pm.tile([128, NCORES], f32)
                rwt = Pm.tile([128, NE], f32, name="rwt")
                for tch in range(NCORES):
                    nc.vector.tensor_tensor(rwt[:], rw_sb[:, tch, :].bitcast(f32), oh_bc[:], op=ALU.mult)
                    nc.vector.tensor_reduce(rwe[:, tch:tch + 1], rwt[:], AX.X, ALU.add)

                with tc.tile_pool(name="ps_dn", bufs=2, space="PSUM") as Pp:
                    for tch in range(8):
                        dn_ps = Pp.tile([128, H], f32, name="dn_ps", tag="dnp")
                        for m in range(8):
                            for j in range(4):
                                nc.tensor.matmul(dn_ps[:, j * 512:(j + 1) * 512],
                                                 su[:, m, tch * 128:(tch + 1) * 128],
                                                 wed[:, m, j * 512:(j + 1) * 512],
                                                 start=(m == 0), stop=(m == 7))
                        ob = Pm.tile([128, H], bf16, name="ob", tag="obs")
                        # scale rows by rw[token, own_expert] (per-partition ptr)
                        nc.scalar.activation(ob[:], dn_ps[:], AF.Copy,
                                             scale=rwe[:, tch:tch + 1])
                        nc.sync.dma_start(OUT_P[:].rearrange("(c p) n -> c p n", p=128)[tch], ob[:])

    nc.compile()
    return nc


_NC = None


def kernel(**inputs):
    global _NC
    inp = {k: np.asarray(v) for k, v in inputs.items()}
    pos = inp["positions"].astype(np.int64)
    x = inp["hidden_states"].astype(np.float32)

    # ---- fold layernorm weights into downstream mats (host prep) ----
    iw = inp["input_ln_w"].astype(np.float32)
    qw = inp["q_a_ln_w"].astype(np.float32)
    kw = inp["kv_a_ln_w"].astype(np.float32)
    pw = inp["post_ln_w"].astype(np.float32)
    Wa = (iw[:, None] * inp["W_qkv_a"]).astype(BF)
    Wik = (iw[:, None] * inp["idx_wk"]).astype(BF)
    Wip = (iw[:, None] * inp["idx_wp_w"]).astype(BF)
    Wqb = (SCALE * qw[:, None] * inp["W_q_b"]).astype(BF)
    Wiq = (qw[:, None] * inp["idx_wq_b"]).astype(BF)
    Wkvb = (kw[:, None] * inp["W_kv_b"]).astype(np.float32).reshape(KL, NH, DN + DV)
    Wkn = np.ascontiguousarray(Wkvb[:, :, :DN].reshape(KL, NH * DN)).astype(BF)
    Wv = np.ascontiguousarray(Wkvb[:, :, DN:].reshape(KL, NH * DV)).astype(BF)
    Wo = inp["W_o"].astype(BF)
    Wg = (pw[:, None] * inp["W_gate"]).astype(np.float32)
    Weg = (pw[None, :, None] * inp["We_gate"]).astype(BF)
    Weu = (pw[None, :, None] * inp["We_up"]).astype(BF)
    Wed = inp["We_down"].astype(BF)
    Wsg = (pw[:, None] * inp["Ws_gate"]).astype(BF)
    Wsu = (pw[:, None] * inp["Ws_up"]).astype(BF)
    Wsd = inp["Ws_down"].astype(BF)

    # relayout expert weights: [H, MI] -> [m][p][k*128+mi'] with H=(k,p)
    def relay_up(W):   # [H, MI] -> [8, 128, 16*128]
        Wr = W.reshape(KB, 128, MI // 128, 128)          # k p m mi'
        return np.ascontiguousarray(Wr.transpose(2, 1, 0, 3).reshape(MI // 128, 128, KB * 128))

    def relay_dn(W):   # [MI, H] -> [8, 128, H]
        return np.ascontiguousarray(W.reshape(MI // 128, 128, H))

    inv = 1.0 / (BASE ** (np.arange(0, DR, 2, dtype=np.float32) / DR))
    ang = pos.astype(np.float32)[:, None] * inv           # [T, 32]
    cs_a, sn_a = np.cos(ang), np.sin(ang)

    in_maps = []
    for c in range(NCORES):
        rows = list(range(c * TPC, (c + 1) * TPC))
        posn = pos[rows]
        causm = (posn[:, None] >= pos[None, :]).astype(np.float32)
        cs = cs_a[rows]; sn = sn_a[rows]
        oh = np.zeros((1, NE), np.float32); oh[0, c] = 1.0
        in_maps.append({
            "OH": oh,
            "XO": np.ascontiguousarray(x[rows]),
            "CAUS": np.ascontiguousarray(causm),
            "CSR": np.ascontiguousarray(np.tile(cs, (1, NH)).astype(np.float32)),
            "SNR": np.ascontiguousarray(np.tile(sn, (1, NH)).astype(np.float32)),
            "KNW": inp["idx_kn_w"].astype(np.float32).reshape(1, IHD),
            "KNB": inp["idx_kn_b"].astype(np.float32).reshape(1, IHD),
            "WPB": inp["idx_wp_b"].astype(np.float32).reshape(1, INH),
            "WA": Wa, "WQB": Wqb, "WIQ": Wiq, "WIK": Wik, "WIP": Wip,
            "WKN": Wkn, "WV": Wv, "WO": Wo, "WG": Wg,
            "WEG": relay_up(Weg[c]),
            "WEU": relay_up(Weu[c]),
            "WED": relay_dn(Wed[c]),
            "WSG": Wsg, "WSU": Wsu, "WSD": Wsd,
        })

    if _NC is None:
        _NC = build()
    try:
        res = run_bass_kernel_spmd(_NC, in_maps, core_ids=list(range(NCORES)))
    except Exception:
        import time as _time
        _time.sleep(2.0)
        res = run_bass_kernel_spmd(_NC, in_maps, core_ids=list(range(NCORES)))

    out = np.zeros((T, H), np.float64)
    for c in range(NCORES):
        out += res.results[c]["OUT_P"].astype(np.float64)
    for c in range(NCORES):
        out[c * TPC:(c + 1) * TPC] += res.results[c]["OUT_X"].astype(np.float64)
    return out.astype(np.float32)
